# revision 2
# baseline (speedup 1.0000x reference)
"""Trainium2 Bass kernel for MultiHeadAttention (B=2, S=2048, D=1024, H=16).

Sharding: 8 cores = 2 (batch) x 4 (head groups of 4 heads / 256 proj cols).
Each core computes attention for its batch + head group and a partial
output projection [S, D]; host sums the 4 partials per batch and adds
bo' = bo + bv @ Wo.T (the V bias is folded into the host-side constant).

v2 pipeline (fp8e4m3 DoubleRow matmuls wherever the cost permits):
  1. Projections: 3-pass error-compensated fp8 DoubleRow
     (x_hi@W_hi + x_lo@W_hi + x_hi@W_lo), weights pre-scaled by 32 on the
     host so the fp8 residuals stay inside e4m3's dynamic range. Q/K are
     descaled (x1/32) + biased on DVE and written straight to fp8 SBUF in
     a permuted (head, halfdim) layout: psum partition p = head*32 +
     (dim%32), slot dim = dim//32. V keeps the 32x scale (the softmax
     ones-row is 32 so the reciprocal absorbs it) and is split hi/lo on
     device for an error-compensated 2-pass PV.
  2. QK^T: fp8 DoubleRow per head over 32 partitions at base head*32
     (contraction 64 = 32x2 slots), output S.T psum [sk, 2 heads, sq].
     Additive -3e4 mask bias on partial blocks as before.
  3. exp (Act, scale=1/8) -> fp8 P.T tiles with a j-pair slot dim; PV is
     a 2-pass (V hi/lo) fp8 DoubleRow over j-tile pairs. Row 64 of the
     PV psum is 32*denominator.
  4. Per head pair: DVE reciprocal of both denominators -> [2, CH], one
     PE broadcast matmul with a 0/1 selector -> [128, CH], DVE multiply
     (psum x psum) -> scaled Z.T in SBUF (f32r).
  5. Out-proj per s-tile: f32r matmuls, psum -> bf16 SBUF -> DMA out.

PSUM: "st" tag [128,2,CH] f32 (2 banks) x2 bufs + "rot" tag [128,CH]
f32 (1 bank) x4 bufs rotating over proj psums, PV accumulators, the
broadcast, and out-proj psums = 8 banks exactly.
"""

import math
import os
import sys

import numpy as np

sys.path.insert(0, "/opt/trn_rl_repo")
sys.path.insert(0, "/opt/trn_rl_repo/concourse")

B, S, D, H = 2, 2048, 1024, 16
HD = D // H  # 64
G = 4  # head groups (cores per batch)
OG = D // G  # 256 proj cols per core
HPG = H // G  # 4 heads per core
P = 128
NT = S // P  # 16 s-tiles
CH = 512  # sq chunk width
NCH = S // CH  # 4 chunks
KT2 = 4  # fp8 DoubleRow contraction steps (256 dims each)
WSCALE = 32.0  # host pre-scale on all projection weights
NEG = -30000.0  # additive mask bias (pre-scale)

_cache = {}


def _block_structure(mask, key_padding_mask):
    """Classify each 128x128 block of the [S,S] score matrix per batch.

    Returns (process, biased, bias_data) where
      process[i,j]  : bool  -- any batch needs block (sq-tile i, sk-tile j)
      biased[i,j]   : bool  -- some processed batch needs a bias on (i,j)
      bias_data[b]  : {(i,j): [128,128] f32 bias (TRANSPOSED: [sk,sq])}
    """
    mask = np.asarray(mask)
    kpm = np.asarray(key_padding_mask)
    full = np.zeros((B, NT, NT), dtype=bool)
    partial = np.zeros((B, NT, NT), dtype=bool)
    blocks = {}
    for b in range(B):
        for i in range(NT):
            mrow = mask[i * P:(i + 1) * P]
            for j in range(NT):
                mb = mrow[:, j * P:(j + 1) * P] | kpm[b, None, j * P:(j + 1) * P]
                if mb.all():
                    full[b, i, j] = True
                elif mb.any():
                    partial[b, i, j] = True
                    blocks[(b, i, j)] = mb
                else:
                    blocks[(b, i, j)] = None
    process = (~full).any(axis=0)
    biased = process & (full | partial).any(axis=0)
    bias_data = []
    for b in range(B):
        d = {}
        for i in range(NT):
            for j in range(NT):
                if not (process[i, j] and biased[i, j]):
                    continue
                if full[b, i, j]:
                    d[(i, j)] = np.full((P, P), NEG, np.float32)
                elif partial[b, i, j]:
                    d[(i, j)] = (blocks[(b, i, j)].T * NEG).astype(np.float32)
                else:
                    d[(i, j)] = np.zeros((P, P), np.float32)
        bias_data.append(d)
    return process, biased, bias_data


def _build_bass(process, biased, bias_slots):
    """Trace the Tile kernel. bias_slots: {(i,j): slot} for biased blocks."""
    import concourse.bass as bass
    import concourse.tile as tile
    from concourse import bacc, mybir

    f32 = mybir.dt.float32
    f32r = mybir.dt.float32r
    f8 = mybir.dt.float8e4
    bf16 = mybir.dt.bfloat16
    DR = mybir.MatmulPerfMode.DoubleRow
    AL = mybir.AluOpType
    EXPS = 1.0 / math.sqrt(HD)
    nc = bacc.Bacc("TRN2", target_bir_lowering=False, debug=False,
                   enable_asserts=False)

    # Host supplies fp8 hi/lo splits, contraction-interleaved:
    # x*: [p, t, slot, s] with input dim d = t*256 + slot*128 + p
    # w*: [p, t, slot, o] same d mapping; o permuted for Q/K (head*32+halfdim)
    xqh = nc.dram_tensor("xqh", [P, KT2, 2, S], f8, kind="ExternalInput").ap()
    xql = nc.dram_tensor("xql", [P, KT2, 2, S], f8, kind="ExternalInput").ap()
    xkh = nc.dram_tensor("xkh", [P, KT2, 2, S], f8, kind="ExternalInput").ap()
    xkl = nc.dram_tensor("xkl", [P, KT2, 2, S], f8, kind="ExternalInput").ap()
    xvh = nc.dram_tensor("xvh", [P, KT2, 2, S], f8, kind="ExternalInput").ap()
    xvl = nc.dram_tensor("xvl", [P, KT2, 2, S], f8, kind="ExternalInput").ap()
    wqh = nc.dram_tensor("wqh", [P, KT2, 2, OG], f8, kind="ExternalInput").ap()
    wql = nc.dram_tensor("wql", [P, KT2, 2, OG], f8, kind="ExternalInput").ap()
    wkh = nc.dram_tensor("wkh", [P, KT2, 2, OG], f8, kind="ExternalInput").ap()
    wkl = nc.dram_tensor("wkl", [P, KT2, 2, OG], f8, kind="ExternalInput").ap()
    wvh = nc.dram_tensor("wvh", [P, KT2, 2, OG], f8, kind="ExternalInput").ap()
    wvl = nc.dram_tensor("wvl", [P, KT2, 2, OG], f8, kind="ExternalInput").ap()
    woT = nc.dram_tensor("woT", [OG, D], f32r, kind="ExternalInput").ap()
    bqd = nc.dram_tensor("bqd", [P, 2], f32, kind="ExternalInput").ap()
    bkd = nc.dram_tensor("bkd", [P, 2], f32, kind="ExternalInput").ap()
    idd = nc.dram_tensor("idd", [P, P], bf16, kind="ExternalInput").ap()
    nbias = max(1, len(bias_slots))
    biasT = nc.dram_tensor("biasT", [nbias, P, P], bf16,
                           kind="ExternalInput").ap()
    out = nc.dram_tensor("out", [S, D], bf16, kind="ExternalOutput").ap()

    with tile.TileContext(nc) as tc:
        with tc.tile_pool(name="persist", bufs=1) as persist, \
             tc.tile_pool(name="const", bufs=1) as const:
            # Persistent SBUF tensors. Q/K layout: 4 slots; head h lives at
            # partition base HBASE[h], slot pair HSP[h]..HSP[h]+2 (matmul
            # operand bases must be in {0,32,64}, so head 3 wraps to base 0
            # on the second slot pair).
            HBASE = [0, 32, 64, 0]
            HSP = [0, 0, 0, 2]
            qT8 = persist.tile([P, 4, S], f8)
            kT8 = persist.tile([P, 4, S], f8)
            # V tiles padded to 128 output dims per head: cols 0:64 = v,
            # col 64 = 32.0 (denominator ones-row), cols 65:128 = 0 — the
            # dual-fp8 ldweights ISA check requires M=128 (M=65 is illegal)
            vaug_h = persist.tile([P, NT, HPG, P], f8)
            vaug_l = persist.tile([P, NT, HPG, P], f8)
            zt01 = persist.tile([P, S], f32r)    # heads 0,1 Z.T scaled
            zt23 = persist.tile([P, S], f32r)
            woT_sb = persist.tile([P, 2, D], f32r)
            bias_sb = persist.tile([P, nbias, P], bf16)

            bqs = const.tile([P, 2], f32)
            bks = const.tile([P, 2], f32)
            ident = const.tile([P, P], bf16)
            nc.sync.dma_start(bqs, bqd)
            nc.sync.dma_start(bks, bkd)
            nc.sync.dma_start(ident, idd)
            # ones-row: 32.0 in the hi V (absorbed by the reciprocal), 0 in lo
            nc.gpsimd.memset(vaug_h[:, :, :, HD:P], 0.0)
            nc.gpsimd.memset(vaug_h[:, :, :, HD:HD + 1], WSCALE)
            nc.vector.memset(vaug_l[:, :, :, HD:P], 0.0)

            # ---- Flat pools ----
            osb = tc.alloc_tile_pool(name="osb", bufs=3)
            xTp = tc.alloc_tile_pool(name="xT", bufs=4)
            wsb = tc.alloc_tile_pool(name="wsb", bufs=1)
            psum = tc.alloc_tile_pool(name="psum", bufs=1, space="PSUM")
            ptp = tc.alloc_tile_pool(name="pt", bufs=4)
            small = tc.alloc_tile_pool(name="small", bufs=2)

            wqh_sb = wsb.tile([P, KT2, 2, OG], f8, tag="wqh")
            wql_sb = wsb.tile([P, KT2, 2, OG], f8, tag="wql")
            wkh_sb = wsb.tile([P, KT2, 2, OG], f8, tag="wkh")
            wkl_sb = wsb.tile([P, KT2, 2, OG], f8, tag="wkl")
            wvh_sb = wsb.tile([P, KT2, 2, OG], f8, tag="wvh")
            wvl_sb = wsb.tile([P, KT2, 2, OG], f8, tag="wvl")
            # K weights first (first projections), split for early start
            nc.sync.dma_start(wkh_sb[:, 0:2], wkh[:, 0:2])
            nc.sync.dma_start(wkh_sb[:, 2:4], wkh[:, 2:4])

            # PSUM: "st" tag [P,2,CH] f32 (2 banks) x2 bufs for attention
            # scores; "rot" tag [P,CH] f32 (1 bank) x4 bufs rotating over
            # proj psums, PV accumulators, and out-proj psums = 8 banks.
            def st_tile(name):
                return psum.tile([P, 2, CH], f32, tag="st", bufs=2, name=name)

            def rot(name):
                return psum.tile([P, CH], f32, tag="rot", bufs=4, name=name)

            srcs = {0: (xkh, xkl, wkh_sb, wkl_sb),
                    1: (xvh, xvl, wvh_sb, wvl_sb),
                    2: (xqh, xql, wqh_sb, wql_sb)}

            def emit_proj(which, c, step):
                # deferred constant loads, spread through the stream
                if step == 0:
                    nc.sync.dma_start(wkl_sb, wkl)
                    nc.sync.dma_start(wqh_sb, wqh)
                elif step == 1:
                    nc.sync.dma_start(wql_sb, wql)
                    nc.sync.dma_start(wvh_sb, wvh)
                    nc.sync.dma_start(bias_sb,
                                      biasT.rearrange("n p q -> p n q"))
                elif step == 2:
                    nc.sync.dma_start(wvl_sb, wvl)
                elif step == 3:
                    nc.sync.dma_start(
                        woT_sb, woT.rearrange("(t p) d -> p t d", p=P))
                xh_dr, xl_dr, w_h, w_l = srcs[which]
                xh_t = xTp.tile([P, KT2, 2, CH], f8, tag="xT", name="xh")
                xl_t = xTp.tile([P, KT2, 2, CH], f8, tag="xT", name="xl")
                csl = slice(c * CH, (c + 1) * CH)
                if step == 0:
                    # fine-grained for an early first matmul
                    nc.sync.dma_start(xh_t[:, 0:2], xh_dr[:, 0:2, :, csl])
                    nc.sync.dma_start(xh_t[:, 2:4], xh_dr[:, 2:4, :, csl])
                else:
                    nc.sync.dma_start(xh_t, xh_dr[:, :, :, csl])
                nc.sync.dma_start(xl_t, xl_dr[:, :, :, csl])
                if which != 1:
                    # Q/K: out psum [perm-dim, s-chunk]; 3-pass DoubleRow
                    dst8 = kT8 if which == 0 else qT8
                    bias_ap = bks if which == 0 else bqs
                    for ot in range(2):
                        ps = rot("psqk")
                        osl = slice(ot * P, (ot + 1) * P)
                        for t in range(KT2):
                            nc.tensor.matmul(ps, w_h[:, t, :, osl],
                                             xh_t[:, t], start=(t == 0),
                                             stop=False, perf_mode=DR)
                        for t in range(KT2):
                            nc.tensor.matmul(ps, w_h[:, t, :, osl],
                                             xl_t[:, t], start=False,
                                             stop=False, perf_mode=DR)
                        for t in range(KT2):
                            nc.tensor.matmul(ps, w_l[:, t, :, osl],
                                             xh_t[:, t], start=False,
                                             stop=(t == KT2 - 1),
                                             perf_mode=DR)
                        # heads 0-2 (psum partitions 0:96) -> slot ot;
                        # head 3 (96:128) -> base 0, slot 2+ot
                        nc.vector.tensor_scalar(
                            dst8[0:96, ot, csl], ps[0:96, :], 1.0 / WSCALE,
                            bias_ap[0:96, ot:ot + 1],
                            op0=AL.mult, op1=AL.add)
                        nc.vector.tensor_scalar(
                            dst8[0:32, 2 + ot, csl], ps[96:128, :],
                            1.0 / WSCALE, bias_ap[96:128, ot:ot + 1],
                            op0=AL.mult, op1=AL.add)
                else:
                    # V: out psum [s-tile, o] (x stationary); keep 32x scale
                    for st in range(CH // P):
                        pv = rot("psv")[:, 0:OG]
                        ssl = slice(st * P, (st + 1) * P)
                        for t in range(KT2):
                            nc.tensor.matmul(pv, xh_t[:, t, :, ssl],
                                             w_h[:, t], start=(t == 0),
                                             stop=False, perf_mode=DR)
                        for t in range(KT2):
                            nc.tensor.matmul(pv, xl_t[:, t, :, ssl],
                                             w_h[:, t], start=False,
                                             stop=False, perf_mode=DR)
                        for t in range(KT2):
                            nc.tensor.matmul(pv, xh_t[:, t, :, ssl],
                                             w_l[:, t], start=False,
                                             stop=(t == KT2 - 1),
                                             perf_mode=DR)
                        pv_re = pv.rearrange("p (h d) -> p h d", h=HPG)
                        vh_view = vaug_h[:, c * 4 + st, :, 0:HD]
                        nc.vector.tensor_copy(vh_view, pv_re)
                        nc.vector.tensor_tensor(
                            vaug_l[:, c * 4 + st, :, 0:HD], pv_re, vh_view,
                            op=AL.subtract)

            # ---- Attention + out-proj, per sq-chunk ----
            # Out-proj for chunk c-1 is emitted mid-way through chunk c so
            # the (in-order) PE stream never stalls on the epilogue; the
            # epilogue itself is PE-free (DVE recip -> Pool partition
            # broadcast -> DVE multiply). Mask biases are added on the PE
            # (identity-matmul accumulate) to keep DVE off the exp path.
            def emit_oproj(c):
                for st in range(4):
                    sg = c * 4 + st
                    ob = osb.tile([P, D], bf16, tag="ob", name="ob")
                    for nk in range(2):
                        ps = rot("psop")
                        for kk, zsrc in enumerate((zt01, zt23)):
                            nc.tensor.matmul(
                                ps, zsrc[:, sg * P:(sg + 1) * P],
                                woT_sb[:, kk, nk * CH:(nk + 1) * CH],
                                start=(kk == 0), stop=(kk == 1))
                        osl = slice(nk * CH, (nk + 1) * CH)
                        if nk == 0:
                            nc.scalar.copy(ob[:, osl], ps)
                        else:
                            nc.vector.tensor_copy(ob[:, osl], ps)
                    nc.sync.dma_start(out[sg * P:(sg + 1) * P, :], ob)

            def emit_attn(c):
                tiles_i = list(range(c * 4, c * 4 + 4))
                jlist = []
                for j in range(NT):
                    ii = [i for i in tiles_i if process[i, j]]
                    if ii:
                        jlist.append((j, min(ii) - c * 4,
                                      max(ii) - c * 4 + 1))
                # pair adjacent j's for the 2-slot PV DoubleRow
                jpairs = []
                idx = 0
                while idx < len(jlist):
                    if (idx + 1 < len(jlist)
                            and jlist[idx + 1][0] == jlist[idx][0] + 1):
                        jpairs.append((jlist[idx], jlist[idx + 1]))
                        idx += 2
                    else:
                        jpairs.append((jlist[idx], None))
                        idx += 1
                for hp in range(2):  # head pairs (2*hp, 2*hp+1)
                    h0, h1 = 2 * hp, 2 * hp + 1
                    zta = {h: rot(f"zta{h}") for h in (h0, h1)}
                    first = True
                    for pi, (pa, pb) in enumerate(jpairs):
                        ja, loa, hia = pa
                        if pb is not None:
                            jb, lob, hib = pb
                            lo_u, hi_u = min(loa, lob), max(hia, hib)
                        else:
                            lo_u, hi_u = loa, hia
                        offu, wu = lo_u * P, (hi_u - lo_u) * P
                        pt = ptp.tile([P, 2, 2, CH], f8, tag="pt", name="pt")
                        for jj, ent in enumerate([pa] + ([pb] if pb else [])):
                            j_, lo_, hi_ = ent
                            off, w = lo_ * P, (hi_ - lo_) * P
                            st_ = st_tile("st_")
                            bis = [i for i in range(c * 4 + lo_, c * 4 + hi_)
                                   if biased[i, j_]]
                            for hh, h in enumerate((h0, h1)):
                                pb_, sp = HBASE[h], HSP[h]
                                nc.tensor.matmul(
                                    st_[:, hh, off:off + w],
                                    kT8[pb_:pb_ + 32, sp:sp + 2,
                                        j_ * P:(j_ + 1) * P],
                                    qT8[pb_:pb_ + 32, sp:sp + 2,
                                        c * CH + off:c * CH + off + w],
                                    start=True, stop=(not bis),
                                    perf_mode=DR)
                            # mask biases via PE identity-matmul accumulate
                            for bn, i in enumerate(bis):
                                sl = bias_slots[(i, j_)]
                                so = (i - c * 4) * P
                                lastb = bn == len(bis) - 1
                                for hh in range(2):
                                    nc.tensor.matmul(
                                        st_[:, hh, so:so + P], ident,
                                        bias_sb[:, sl, :], start=False,
                                        stop=(lastb and hh == 1),
                                        skip_group_check=True)
                            # zero pt where this j's band is narrower than
                            # the pair's union (PV reads the union)
                            if off > offu:
                                nc.gpsimd.memset(
                                    pt[:, jj, :, offu:off], 0.0)
                            if off + w < offu + wu:
                                nc.gpsimd.memset(
                                    pt[:, jj, :, off + w:offu + wu], 0.0)
                            nc.scalar.activation(
                                pt[:, jj, :, off:off + w],
                                st_[:, :, off:off + w],
                                mybir.ActivationFunctionType.Exp,
                                scale=EXPS)
                        last = pi == len(jpairs) - 1
                        for hh, h in enumerate((h0, h1)):
                            zo = zta[h]
                            if pb is not None:
                                nc.tensor.matmul(
                                    zo[:, offu:offu + wu],
                                    vaug_h[:, ja:ja + 2, h, :],
                                    pt[:, :, hh, offu:offu + wu],
                                    start=first, stop=False, perf_mode=DR)
                                nc.tensor.matmul(
                                    zo[:, offu:offu + wu],
                                    vaug_l[:, ja:ja + 2, h, :],
                                    pt[:, :, hh, offu:offu + wu],
                                    start=False, stop=last, perf_mode=DR)
                            else:
                                nc.tensor.matmul(
                                    zo[:, offu:offu + wu],
                                    vaug_h[:, ja, h, :],
                                    pt[:, 0, hh, offu:offu + wu],
                                    start=first, stop=False)
                                nc.tensor.matmul(
                                    zo[:, offu:offu + wu],
                                    vaug_l[:, ja, h, :],
                                    pt[:, 0, hh, offu:offu + wu],
                                    start=False, stop=last)
                        first = False
                    # epilogue (PE-free): reciprocal of 32*denominators,
                    # Pool partition-broadcast, DVE scale into SBUF
                    recs = small.tile([1, 2, CH], f32, tag="recs", bufs=2,
                                      name="recs")
                    bcs0 = small.tile([HD, CH], f32, tag="bcs0", bufs=2,
                                      name="bcs0")
                    bcs1 = small.tile([HD, CH], f32, tag="bcs1", bufs=2,
                                      name="bcs1")
                    with nc.allow_low_precision(reason="fp22 recip"):
                        nc.vector.reciprocal(recs[0:1, 0, :],
                                             zta[h0][HD:HD + 1, :])
                        nc.vector.reciprocal(recs[0:1, 1, :],
                                             zta[h1][HD:HD + 1, :])
                    # (partition_broadcast only writes at base partition 0)
                    nc.gpsimd.partition_broadcast(bcs0, recs[0:1, 0, :],
                                                  channels=HD)
                    nc.gpsimd.partition_broadcast(bcs1, recs[0:1, 1, :],
                                                  channels=HD)
                    zdst = zt01 if hp == 0 else zt23
                    for hh, h in enumerate((h0, h1)):
                        zpo = hh * HD
                        nc.vector.tensor_mul(
                            zdst[zpo:zpo + HD, c * CH:(c + 1) * CH],
                            zta[h][0:HD, :], (bcs0 if hh == 0 else bcs1))
                    if hp == 0 and c > 0:
                        emit_oproj(c - 1)

            # Interleaved schedule: causal attention chunk c needs only
            # K/V chunks 0..c and Q chunk c — emit it as soon as those
            # projections are in the stream so the Act engine (exp, the
            # critical resource) starts as early as possible.
            step = 0
            for c in range(NCH):
                for which in (0, 2, 1):  # K, Q, V of chunk c
                    emit_proj(which, c, step)
                    step += 1
                emit_attn(c)
            emit_oproj(NCH - 1)
            for pool_ in (small, ptp, psum, wsb, xTp, osb):
                pool_.release()
    nc.compile()
    # Belt-and-braces: any write-only preamble registers that survive DCE
    # but never get ids from alloc_regs would fail walrus birverifier
    # (reg_id == -1). They are write-only, so engine-unique ids are safe;
    # keep _lo/_hi pairs adjacent and even-aligned.
    from collections import defaultdict
    from concourse import mybir
    ctr = defaultdict(int)
    for f_ in nc.m.functions:
        for a in f_.allocations:
            if isinstance(a, mybir.Register) and a.reg_id >= 0:
                ctr[a.engine] = max(ctr[a.engine], a.reg_id + 1)
    for f_ in nc.m.functions:
        for a in f_.allocations:
            if isinstance(a, mybir.Register) and a.reg_id == -1:
                if a.name.endswith("_lo") and ctr[a.engine] % 2:
                    ctr[a.engine] += 1
                a.reg_id = ctr[a.engine]
                ctr[a.engine] += 1
    return nc


def _interleave_kdim(arr):
    """[1024 in-dim, N] -> [128 p, 4 t, 2 slot, N] with d = t*256+slot*128+p."""
    n = arr.shape[1]
    return np.ascontiguousarray(
        arr.reshape(KT2, 2, P, n).transpose(2, 0, 1, 3))


def _split8(arr):
    import ml_dtypes
    e4 = ml_dtypes.float8_e4m3
    hi = arr.astype(e4)
    lo = (arr - hi.astype(np.float32)).astype(e4)
    return np.ascontiguousarray(hi), np.ascontiguousarray(lo)


def kernel(query, key, value, mask, key_padding_mask,
           Wq, bq, Wk, bk, Wv, bv, Wo, bo, _return_perf=False):
    import ml_dtypes
    from concourse import bass_utils

    query = np.asarray(query, np.float32)
    key_ = np.asarray(key, np.float32)
    value = np.asarray(value, np.float32)
    Wq, Wk, Wv, Wo = (np.asarray(w, np.float32) for w in (Wq, Wk, Wv, Wo))
    bq, bk, bv, bo = (np.asarray(b_, np.float32) for b_ in (bq, bk, bv, bo))

    process, biased, bias_data = _block_structure(mask, key_padding_mask)
    bias_slots = {}
    for i in range(NT):
        for j in range(NT):
            if process[i, j] and biased[i, j]:
                bias_slots[(i, j)] = len(bias_slots)

    key_struct = (process.tobytes(), biased.tobytes())
    if key_struct not in _cache:
        _cache[key_struct] = _build_bass(process, biased, bias_slots)
    nc = _cache[key_struct]

    nbias = max(1, len(bias_slots))
    # x splits: shared across the 4 cores of each batch
    xsp = {}
    for b in range(B):
        for nm, x in (("q", query[b]), ("k", key_[b]), ("v", value[b])):
            xsp[(nm, b)] = _split8(_interleave_kdim(
                np.ascontiguousarray(x.T)))

    # Q/K output-dim permutation: psum partition p = head*32 + dim%32,
    # slot ot = dim//32  (head/dim within this core's 4-head group)
    perm = np.zeros((2, P), np.int64)
    for ot in range(2):
        for p_ in range(P):
            perm[ot, p_] = (p_ // 32) * HD + ot * 32 + (p_ % 32)

    in_maps = []
    for core in range(8):
        b, g = core // G, core % G
        gsl = np.arange(g * OG, (g + 1) * OG)
        qk_rows = gsl.reshape(1, OG)[0][perm.reshape(-1)]  # [256] perm'd
        wq_s = _split8(_interleave_kdim(WSCALE * Wq[qk_rows, :].T))
        wk_s = _split8(_interleave_kdim(WSCALE * Wk[qk_rows, :].T))
        wv_s = _split8(_interleave_kdim(WSCALE * Wv[gsl, :].T))
        bt = np.zeros((nbias, P, P), np.float32)
        for (i, j), slot in bias_slots.items():
            bt[slot] = bias_data[b][(i, j)]
        bt = bt.astype(ml_dtypes.bfloat16)
        in_maps.append({
            "xqh": xsp[("q", b)][0], "xql": xsp[("q", b)][1],
            "xkh": xsp[("k", b)][0], "xkl": xsp[("k", b)][1],
            "xvh": xsp[("v", b)][0], "xvl": xsp[("v", b)][1],
            "wqh": wq_s[0], "wql": wq_s[1],
            "wkh": wk_s[0], "wkl": wk_s[1],
            "wvh": wv_s[0], "wvl": wv_s[1],
            "woT": np.ascontiguousarray(Wo[:, gsl].T),
            "bqd": np.ascontiguousarray(bq[qk_rows].reshape(2, P).T),
            "bkd": np.ascontiguousarray(bk[qk_rows].reshape(2, P).T),
            "idd": np.eye(P, dtype=ml_dtypes.bfloat16),
            "biasT": bt,
        })

    trace = bool(int(os.environ.get("KERNEL_TRACE", "0")))
    res = bass_utils.run_bass_kernel_spmd(
        nc, in_maps, core_ids=list(range(8)), trace=trace)

    out = np.zeros((B, S, D), np.float32)
    for core in range(8):
        out[core // G] += res.results[core]["out"].astype(np.float32)
    out += (bo + bv @ Wo.T)[None, None, :]
    if _return_perf:
        return out, res
    return out


# revision 3
# speedup vs baseline: 1.0456x; 1.0456x over previous
"""Trainium2 Bass kernel for MultiHeadAttention (B=2, S=2048, D=1024, H=16).

Sharding: 8 cores = 2 (batch) x 4 (head groups of 4 heads / 256 proj cols).
Each core computes attention for its batch + head group and a partial
output projection [S, D]; host sums the 4 partials per batch and adds
bo' = bo + bv @ Wo.T (the V bias is folded into the host-side constant).

v2 pipeline (fp8e4m3 DoubleRow matmuls wherever the cost permits):
  1. Projections: 3-pass error-compensated fp8 DoubleRow
     (x_hi@W_hi + x_lo@W_hi + x_hi@W_lo), weights pre-scaled by 32 on the
     host so the fp8 residuals stay inside e4m3's dynamic range. Q/K are
     descaled (x1/32) + biased on DVE and written straight to fp8 SBUF in
     a permuted (head, halfdim) layout: psum partition p = head*32 +
     (dim%32), slot dim = dim//32. V keeps the 32x scale (the softmax
     ones-row is 32 so the reciprocal absorbs it) and is split hi/lo on
     device for an error-compensated 2-pass PV.
  2. QK^T: fp8 DoubleRow per head over 32 partitions at base head*32
     (contraction 64 = 32x2 slots), output S.T psum [sk, 2 heads, sq].
     Additive -3e4 mask bias on partial blocks as before.
  3. exp (Act, scale=1/8) -> fp8 P.T tiles with a j-pair slot dim; PV is
     a 2-pass (V hi/lo) fp8 DoubleRow over j-tile pairs. Row 64 of the
     PV psum is 32*denominator.
  4. Per head pair: DVE reciprocal of both denominators -> [2, CH], one
     PE broadcast matmul with a 0/1 selector -> [128, CH], DVE multiply
     (psum x psum) -> scaled Z.T in SBUF (f32r).
  5. Out-proj per s-tile: f32r matmuls, psum -> bf16 SBUF -> DMA out.

PSUM: "st" tag [128,2,CH] f32 (2 banks) x2 bufs + "rot" tag [128,CH]
f32 (1 bank) x4 bufs rotating over proj psums, PV accumulators, the
broadcast, and out-proj psums = 8 banks exactly.
"""

import math
import os
import sys

import numpy as np

sys.path.insert(0, "/opt/trn_rl_repo")
sys.path.insert(0, "/opt/trn_rl_repo/concourse")

B, S, D, H = 2, 2048, 1024, 16
HD = D // H  # 64
G = 4  # head groups (cores per batch)
OG = D // G  # 256 proj cols per core
HPG = H // G  # 4 heads per core
P = 128
NT = S // P  # 16 s-tiles
CH = 512  # sq chunk width
NCH = S // CH  # 4 chunks
KT2 = 4  # fp8 DoubleRow contraction steps (256 dims each)
WSCALE = 32.0  # host pre-scale on all projection weights
NEG = -30000.0  # additive mask bias (pre-scale)

_cache = {}


def _block_structure(mask, key_padding_mask):
    """Classify each 128x128 block of the [S,S] score matrix per batch.

    Returns (process, biased, bias_data) where
      process[i,j]  : bool  -- any batch needs block (sq-tile i, sk-tile j)
      biased[i,j]   : bool  -- some processed batch needs a bias on (i,j)
      bias_data[b]  : {(i,j): [128,128] f32 bias (TRANSPOSED: [sk,sq])}
    """
    mask = np.asarray(mask)
    kpm = np.asarray(key_padding_mask)
    full = np.zeros((B, NT, NT), dtype=bool)
    partial = np.zeros((B, NT, NT), dtype=bool)
    blocks = {}
    for b in range(B):
        for i in range(NT):
            mrow = mask[i * P:(i + 1) * P]
            for j in range(NT):
                mb = mrow[:, j * P:(j + 1) * P] | kpm[b, None, j * P:(j + 1) * P]
                if mb.all():
                    full[b, i, j] = True
                elif mb.any():
                    partial[b, i, j] = True
                    blocks[(b, i, j)] = mb
                else:
                    blocks[(b, i, j)] = None
    process = (~full).any(axis=0)
    biased = process & (full | partial).any(axis=0)
    bias_data = []
    for b in range(B):
        d = {}
        for i in range(NT):
            for j in range(NT):
                if not (process[i, j] and biased[i, j]):
                    continue
                if full[b, i, j]:
                    d[(i, j)] = np.full((P, P), NEG, np.float32)
                elif partial[b, i, j]:
                    d[(i, j)] = (blocks[(b, i, j)].T * NEG).astype(np.float32)
                else:
                    d[(i, j)] = np.zeros((P, P), np.float32)
        bias_data.append(d)
    return process, biased, bias_data


def _build_bass(process, biased, bias_slots):
    """Trace the Tile kernel. bias_slots: {(i,j): slot} for biased blocks."""
    import concourse.bass as bass
    import concourse.tile as tile
    from concourse import bacc, mybir

    f32 = mybir.dt.float32
    f32r = mybir.dt.float32r
    f8 = mybir.dt.float8e4
    bf16 = mybir.dt.bfloat16
    DR = mybir.MatmulPerfMode.DoubleRow
    AL = mybir.AluOpType
    EXPS = 1.0 / math.sqrt(HD)
    nc = bacc.Bacc("TRN2", target_bir_lowering=False, debug=False,
                   enable_asserts=False)

    # Host supplies fp8 hi/lo splits, contraction-interleaved:
    # x*: [p, t, slot, s] with input dim d = t*256 + slot*128 + p
    # w*: [p, t, slot, o] same d mapping; o permuted for Q/K (head*32+halfdim)
    xqh = nc.dram_tensor("xqh", [P, KT2, 2, S], f8, kind="ExternalInput").ap()
    xql = nc.dram_tensor("xql", [P, KT2, 2, S], f8, kind="ExternalInput").ap()
    xkh = nc.dram_tensor("xkh", [P, KT2, 2, S], f8, kind="ExternalInput").ap()
    xkl = nc.dram_tensor("xkl", [P, KT2, 2, S], f8, kind="ExternalInput").ap()
    xvh = nc.dram_tensor("xvh", [P, KT2, 2, S], f8, kind="ExternalInput").ap()
    xvl = nc.dram_tensor("xvl", [P, KT2, 2, S], f8, kind="ExternalInput").ap()
    wqh = nc.dram_tensor("wqh", [P, KT2, 2, OG], f8, kind="ExternalInput").ap()
    wql = nc.dram_tensor("wql", [P, KT2, 2, OG], f8, kind="ExternalInput").ap()
    wkh = nc.dram_tensor("wkh", [P, KT2, 2, OG], f8, kind="ExternalInput").ap()
    wkl = nc.dram_tensor("wkl", [P, KT2, 2, OG], f8, kind="ExternalInput").ap()
    wvh = nc.dram_tensor("wvh", [P, KT2, 2, OG], f8, kind="ExternalInput").ap()
    wvl = nc.dram_tensor("wvl", [P, KT2, 2, OG], f8, kind="ExternalInput").ap()
    woT = nc.dram_tensor("woT", [OG, D], bf16, kind="ExternalInput").ap()
    bqd = nc.dram_tensor("bqd", [P, 2], f32, kind="ExternalInput").ap()
    bkd = nc.dram_tensor("bkd", [P, 2], f32, kind="ExternalInput").ap()
    idd = nc.dram_tensor("idd", [P, P], bf16, kind="ExternalInput").ap()
    nbias = max(1, len(bias_slots))
    biasT = nc.dram_tensor("biasT", [nbias, P, P], bf16,
                           kind="ExternalInput").ap()
    out = nc.dram_tensor("out", [S, D], bf16, kind="ExternalOutput").ap()

    with tile.TileContext(nc) as tc:
        with tc.tile_pool(name="persist", bufs=1) as persist, \
             tc.tile_pool(name="const", bufs=1) as const:
            # Persistent SBUF tensors. Q/K layout: 4 slots; head h lives at
            # partition base HBASE[h], slot pair HSP[h]..HSP[h]+2 (matmul
            # operand bases must be in {0,32,64}, so head 3 wraps to base 0
            # on the second slot pair).
            HBASE = [0, 32, 64, 0]
            HSP = [0, 0, 0, 2]
            qT8 = persist.tile([P, 4, S], f8)
            kT8 = persist.tile([P, 4, S], f8)
            # V tiles padded to 128 output dims per head: cols 0:64 = v,
            # col 64 = 32.0 (denominator ones-row), cols 65:128 = 0 — the
            # dual-fp8 ldweights ISA check requires M=128 (M=65 is illegal)
            vaug_h = persist.tile([P, NT, HPG, P], f8)
            vaug_l = persist.tile([P, NT, HPG, P], f8)
            zt01 = persist.tile([P, S], bf16)    # heads 0,1 Z.T scaled
            zt23 = persist.tile([P, S], bf16)
            woT_sb = persist.tile([P, 2, D], bf16)
            bias_sb = persist.tile([P, nbias, P], bf16)

            bqs = const.tile([P, 2], f32)
            bks = const.tile([P, 2], f32)
            ident = const.tile([P, P], bf16)
            # ones-row: 32.0 in the hi V (absorbed by the reciprocal), 0 in lo
            nc.gpsimd.memset(vaug_h[:, :, :, HD:P], 0.0)
            nc.gpsimd.memset(vaug_h[:, :, :, HD:HD + 1], WSCALE)
            nc.vector.memset(vaug_l[:, :, :, HD:P], 0.0)

            # ---- Flat pools ----
            osb = tc.alloc_tile_pool(name="osb", bufs=3)
            xTp = tc.alloc_tile_pool(name="xT", bufs=4)
            wsb = tc.alloc_tile_pool(name="wsb", bufs=1)
            psum = tc.alloc_tile_pool(name="psum", bufs=1, space="PSUM")
            ptp = tc.alloc_tile_pool(name="pt", bufs=4)
            small = tc.alloc_tile_pool(name="small", bufs=2)

            wqh_sb = wsb.tile([P, KT2, 2, OG], f8, tag="wqh")
            wql_sb = wsb.tile([P, KT2, 2, OG], f8, tag="wql")
            wkh_sb = wsb.tile([P, KT2, 2, OG], f8, tag="wkh")
            wkl_sb = wsb.tile([P, KT2, 2, OG], f8, tag="wkl")
            wvh_sb = wsb.tile([P, KT2, 2, OG], f8, tag="wvh")
            wvl_sb = wsb.tile([P, KT2, 2, OG], f8, tag="wvl")
            # K weights first (first projections), split for early start;
            # tiny const loads are deferred behind the first matmul's deps
            nc.sync.dma_start(wkh_sb[:, 0:2], wkh[:, 0:2])

            # PSUM: "st" tag [P,2,CH] f32 (2 banks) x2 bufs for attention
            # scores; "rot" tag [P,CH] f32 (1 bank) x4 bufs rotating over
            # proj psums, PV accumulators, and out-proj psums = 8 banks.
            def st_tile(name):
                return psum.tile([P, 2, CH], f32, tag="st", bufs=2, name=name)

            def rot(name):
                return psum.tile([P, CH], f32, tag="rot", bufs=4, name=name)

            srcs = {0: (xkh, xkl, wkh_sb, wkl_sb),
                    1: (xvh, xvl, wvh_sb, wvl_sb),
                    2: (xqh, xql, wqh_sb, wql_sb)}

            def emit_proj(which, c, step):
                # deferred constant loads, spread through the stream
                if step == 0:
                    nc.sync.dma_start(wkl_sb, wkl)
                    nc.sync.dma_start(bks, bkd)
                    nc.sync.dma_start(wqh_sb, wqh)
                    nc.sync.dma_start(bqs, bqd)
                    nc.sync.dma_start(ident, idd)
                elif step == 1:
                    nc.sync.dma_start(wql_sb, wql)
                    nc.sync.dma_start(wvh_sb, wvh)
                    nc.sync.dma_start(bias_sb,
                                      biasT.rearrange("n p q -> p n q"))
                elif step == 2:
                    nc.sync.dma_start(wvl_sb, wvl)
                elif step == 3:
                    nc.sync.dma_start(
                        woT_sb, woT.rearrange("(t p) d -> p t d", p=P))
                xh_dr, xl_dr, w_h, w_l = srcs[which]
                xh_t = xTp.tile([P, KT2, 2, CH], f8, tag="xT", name="xh")
                xl_t = xTp.tile([P, KT2, 2, CH], f8, tag="xT", name="xl")
                csl = slice(c * CH, (c + 1) * CH)
                if step == 0:
                    # fine-grained + issued on the idle Act queue so the
                    # first matmul's two DMAs run in parallel
                    nc.scalar.dma_start(xh_t[:, 0:2], xh_dr[:, 0:2, :, csl])
                    nc.sync.dma_start(wkh_sb[:, 2:4], wkh[:, 2:4])
                    nc.sync.dma_start(xh_t[:, 2:4], xh_dr[:, 2:4, :, csl])
                else:
                    nc.sync.dma_start(xh_t, xh_dr[:, :, :, csl])
                nc.sync.dma_start(xl_t, xl_dr[:, :, :, csl])
                if which != 1:
                    # Q/K: out psum [perm-dim, s-chunk]; 3-pass DoubleRow
                    dst8 = kT8 if which == 0 else qT8
                    bias_ap = bks if which == 0 else bqs
                    for ot in range(2):
                        ps = rot("psqk")
                        osl = slice(ot * P, (ot + 1) * P)
                        for t in range(KT2):
                            nc.tensor.matmul(ps, w_h[:, t, :, osl],
                                             xh_t[:, t], start=(t == 0),
                                             stop=False, perf_mode=DR)
                        for t in range(KT2):
                            nc.tensor.matmul(ps, w_l[:, t, :, osl],
                                             xh_t[:, t], start=False,
                                             stop=False, perf_mode=DR)
                        for t in range(KT2):
                            nc.tensor.matmul(ps, w_h[:, t, :, osl],
                                             xl_t[:, t], start=False,
                                             stop=(t == KT2 - 1),
                                             perf_mode=DR)
                        # heads 0-2 (psum partitions 0:96) -> slot ot;
                        # head 3 (96:128) -> base 0, slot 2+ot
                        nc.vector.tensor_scalar(
                            dst8[0:96, ot, csl], ps[0:96, :], 1.0 / WSCALE,
                            bias_ap[0:96, ot:ot + 1],
                            op0=AL.mult, op1=AL.add)
                        nc.vector.tensor_scalar(
                            dst8[0:32, 2 + ot, csl], ps[96:128, :],
                            1.0 / WSCALE, bias_ap[96:128, ot:ot + 1],
                            op0=AL.mult, op1=AL.add)
                else:
                    # V: out psum [s-tile, o] (x stationary); keep 32x scale
                    for st in range(CH // P):
                        pv = rot("psv")[:, 0:OG]
                        ssl = slice(st * P, (st + 1) * P)
                        for t in range(KT2):
                            nc.tensor.matmul(pv, xh_t[:, t, :, ssl],
                                             w_h[:, t], start=(t == 0),
                                             stop=False, perf_mode=DR)
                        for t in range(KT2):
                            nc.tensor.matmul(pv, xh_t[:, t, :, ssl],
                                             w_l[:, t], start=False,
                                             stop=False, perf_mode=DR)
                        for t in range(KT2):
                            nc.tensor.matmul(pv, xl_t[:, t, :, ssl],
                                             w_h[:, t], start=False,
                                             stop=(t == KT2 - 1),
                                             perf_mode=DR)
                        pv_re = pv.rearrange("p (h d) -> p h d", h=HPG)
                        vh_view = vaug_h[:, c * 4 + st, :, 0:HD]
                        nc.vector.tensor_copy(vh_view, pv_re)
                        nc.vector.tensor_tensor(
                            vaug_l[:, c * 4 + st, :, 0:HD], pv_re, vh_view,
                            op=AL.subtract)

            # ---- Attention + out-proj, per sq-chunk ----
            # Out-proj for chunk c-1 is emitted mid-way through chunk c so
            # the (in-order) PE stream never stalls on the epilogue; the
            # epilogue itself is PE-free (DVE recip -> Pool partition
            # broadcast -> DVE multiply). Mask biases are added on the PE
            # (identity-matmul accumulate) to keep DVE off the exp path.
            ATTN_OFF = [0, 512, 1024, 1536]
            ATTN_W = [512, 512, 512, 512]
            NAC = len(ATTN_W)

            def emit_oproj(ci):
                i0 = ATTN_OFF[ci] // P
                for sg in range(i0, i0 + ATTN_W[ci] // P):
                    ob = osb.tile([P, D], bf16, tag="ob", name="ob")
                    for nk in range(2):
                        ps = rot("psop")
                        for kk, zsrc in enumerate((zt01, zt23)):
                            nc.tensor.matmul(
                                ps, zsrc[:, sg * P:(sg + 1) * P],
                                woT_sb[:, kk, nk * CH:(nk + 1) * CH],
                                start=(kk == 0), stop=(kk == 1))
                        osl = slice(nk * CH, (nk + 1) * CH)
                        if nk == 0:
                            nc.scalar.copy(ob[:, osl], ps)
                        else:
                            nc.vector.tensor_copy(ob[:, osl], ps)
                    nc.sync.dma_start(out[sg * P:(sg + 1) * P, :], ob)

            def emit_attn(ci, between=None):
                coff, cw = ATTN_OFF[ci], ATTN_W[ci]
                i0 = coff // P
                tiles_i = list(range(i0, i0 + cw // P))
                jlist = []
                for j in range(NT):
                    ii = [i for i in tiles_i if process[i, j]]
                    if ii:
                        jlist.append((j, min(ii) - i0,
                                      max(ii) - i0 + 1))
                # pair adjacent j's for the 2-slot PV DoubleRow
                jpairs = []
                idx = 0
                while idx < len(jlist):
                    if (idx + 1 < len(jlist)
                            and jlist[idx + 1][0] == jlist[idx][0] + 1):
                        jpairs.append((jlist[idx], jlist[idx + 1]))
                        idx += 2
                    else:
                        jpairs.append((jlist[idx], None))
                        idx += 1
                for hp in range(2):  # head pairs (2*hp, 2*hp+1)
                    h0, h1 = 2 * hp, 2 * hp + 1
                    zta = {h: rot(f"zta{h}") for h in (h0, h1)}
                    first = True
                    for pi, (pa, pb) in enumerate(jpairs):
                        ja, loa, hia = pa
                        if pb is not None:
                            jb, lob, hib = pb
                            lo_u, hi_u = min(loa, lob), max(hia, hib)
                        else:
                            lo_u, hi_u = loa, hia
                        offu, wu = lo_u * P, (hi_u - lo_u) * P
                        pt = ptp.tile([P, 2, 2, CH], f8, tag="pt", name="pt")
                        for jj, ent in enumerate([pa] + ([pb] if pb else [])):
                            j_, lo_, hi_ = ent
                            off, w = lo_ * P, (hi_ - lo_) * P
                            st_ = st_tile("st_")
                            bis = [i for i in range(i0 + lo_, i0 + hi_)
                                   if biased[i, j_]]
                            for hh, h in enumerate((h0, h1)):
                                pb_, sp = HBASE[h], HSP[h]
                                nc.tensor.matmul(
                                    st_[:, hh, off:off + w],
                                    kT8[pb_:pb_ + 32, sp:sp + 2,
                                        j_ * P:(j_ + 1) * P],
                                    qT8[pb_:pb_ + 32, sp:sp + 2,
                                        coff + off:coff + off + w],
                                    start=True, stop=(not bis),
                                    perf_mode=DR)
                            # mask biases via PE identity-matmul accumulate
                            for bn, i in enumerate(bis):
                                sl = bias_slots[(i, j_)]
                                so = (i - i0) * P
                                lastb = bn == len(bis) - 1
                                for hh in range(2):
                                    nc.tensor.matmul(
                                        st_[:, hh, so:so + P], ident,
                                        bias_sb[:, sl, :], start=False,
                                        stop=(lastb and hh == 1),
                                        skip_group_check=True)
                            # zero pt where this j's band is narrower than
                            # the pair's union (PV reads the union)
                            if off > offu:
                                nc.gpsimd.memset(
                                    pt[:, jj, :, offu:off], 0.0)
                            if off + w < offu + wu:
                                nc.gpsimd.memset(
                                    pt[:, jj, :, off + w:offu + wu], 0.0)
                            nc.scalar.activation(
                                pt[:, jj, :, off:off + w],
                                st_[:, :, off:off + w],
                                mybir.ActivationFunctionType.Exp,
                                scale=EXPS)
                        last = pi == len(jpairs) - 1
                        for hh, h in enumerate((h0, h1)):
                            zo = zta[h]
                            if pb is not None:
                                nc.tensor.matmul(
                                    zo[:, offu:offu + wu],
                                    vaug_h[:, ja:ja + 2, h, :],
                                    pt[:, :, hh, offu:offu + wu],
                                    start=first, stop=False, perf_mode=DR)
                                nc.tensor.matmul(
                                    zo[:, offu:offu + wu],
                                    vaug_l[:, ja:ja + 2, h, :],
                                    pt[:, :, hh, offu:offu + wu],
                                    start=False, stop=last, perf_mode=DR)
                            else:
                                nc.tensor.matmul(
                                    zo[:, offu:offu + wu],
                                    vaug_h[:, ja, h, :],
                                    pt[:, 0, hh, offu:offu + wu],
                                    start=first, stop=False)
                                nc.tensor.matmul(
                                    zo[:, offu:offu + wu],
                                    vaug_l[:, ja, h, :],
                                    pt[:, 0, hh, offu:offu + wu],
                                    start=False, stop=last)
                        first = False
                    # epilogue (PE-free): reciprocal of 32*denominators,
                    # Pool partition-broadcast, DVE scale into SBUF
                    recs = small.tile([1, 2, CH], f32, tag="recs", bufs=2,
                                      name="recs")
                    bcs0 = small.tile([HD, CH], f32, tag="bcs0", bufs=2,
                                      name="bcs0")
                    bcs1 = small.tile([HD, CH], f32, tag="bcs1", bufs=2,
                                      name="bcs1")
                    with nc.allow_low_precision(reason="fp22 recip"):
                        nc.vector.reciprocal(recs[0:1, 0, 0:cw],
                                             zta[h0][HD:HD + 1, 0:cw])
                        nc.vector.reciprocal(recs[0:1, 1, 0:cw],
                                             zta[h1][HD:HD + 1, 0:cw])
                    # (partition_broadcast only writes at base partition 0)
                    nc.gpsimd.partition_broadcast(bcs0[:, 0:cw],
                                                  recs[0:1, 0, 0:cw],
                                                  channels=HD)
                    nc.gpsimd.partition_broadcast(bcs1[:, 0:cw],
                                                  recs[0:1, 1, 0:cw],
                                                  channels=HD)
                    zdst = zt01 if hp == 0 else zt23
                    for hh, h in enumerate((h0, h1)):
                        zpo = hh * HD
                        nc.vector.tensor_mul(
                            zdst[zpo:zpo + HD, coff:coff + cw],
                            zta[h][0:HD, 0:cw],
                            (bcs0 if hh == 0 else bcs1)[:, 0:cw])
                    if hp == 0 and between is not None:
                        between()

            # Interleaved schedule: causal attention chunk c needs only
            # K/V chunks 0..c and Q chunk c. Chunk c+1's projections are
            # emitted BETWEEN chunk c's head pairs so the PE keeps working
            # while the Act engine (exp, the critical resource) chews on
            # chunk c, and Act never starves at chunk boundaries.
            emit_proj(0, 0, 0)
            emit_proj(2, 0, 1)
            emit_proj(1, 0, 2)

            def make_between(ci):
                def between():
                    if ci + 1 < NCH:
                        emit_proj(0, ci + 1, 3 + ci)
                    if ci > 0:
                        emit_oproj(ci - 1)
                return between

            for ci in range(NAC):
                emit_attn(ci, between=make_between(ci))
                if ci + 1 < NCH:
                    emit_proj(2, ci + 1, 99)
                    emit_proj(1, ci + 1, 99)
            emit_oproj(NAC - 1)
            for pool_ in (small, ptp, psum, wsb, xTp, osb):
                pool_.release()
    nc.compile()
    # Belt-and-braces: any write-only preamble registers that survive DCE
    # but never get ids from alloc_regs would fail walrus birverifier
    # (reg_id == -1). They are write-only, so engine-unique ids are safe;
    # keep _lo/_hi pairs adjacent and even-aligned.
    from collections import defaultdict
    from concourse import mybir
    ctr = defaultdict(int)
    for f_ in nc.m.functions:
        for a in f_.allocations:
            if isinstance(a, mybir.Register) and a.reg_id >= 0:
                ctr[a.engine] = max(ctr[a.engine], a.reg_id + 1)
    for f_ in nc.m.functions:
        for a in f_.allocations:
            if isinstance(a, mybir.Register) and a.reg_id == -1:
                if a.name.endswith("_lo") and ctr[a.engine] % 2:
                    ctr[a.engine] += 1
                a.reg_id = ctr[a.engine]
                ctr[a.engine] += 1
    return nc


def _interleave_kdim(arr):
    """[1024 in-dim, N] -> [128 p, 4 t, 2 slot, N] with d = t*256+slot*128+p."""
    n = arr.shape[1]
    return np.ascontiguousarray(
        arr.reshape(KT2, 2, P, n).transpose(2, 0, 1, 3))


def _split8(arr):
    import ml_dtypes
    e4 = ml_dtypes.float8_e4m3
    hi = arr.astype(e4)
    lo = (arr - hi.astype(np.float32)).astype(e4)
    return np.ascontiguousarray(hi), np.ascontiguousarray(lo)


def kernel(query, key, value, mask, key_padding_mask,
           Wq, bq, Wk, bk, Wv, bv, Wo, bo, _return_perf=False):
    import ml_dtypes
    from concourse import bass_utils

    query = np.asarray(query, np.float32)
    key_ = np.asarray(key, np.float32)
    value = np.asarray(value, np.float32)
    Wq, Wk, Wv, Wo = (np.asarray(w, np.float32) for w in (Wq, Wk, Wv, Wo))
    bq, bk, bv, bo = (np.asarray(b_, np.float32) for b_ in (bq, bk, bv, bo))

    process, biased, bias_data = _block_structure(mask, key_padding_mask)
    bias_slots = {}
    for i in range(NT):
        for j in range(NT):
            if process[i, j] and biased[i, j]:
                bias_slots[(i, j)] = len(bias_slots)

    key_struct = (process.tobytes(), biased.tobytes())
    if key_struct not in _cache:
        _cache[key_struct] = _build_bass(process, biased, bias_slots)
    nc = _cache[key_struct]

    nbias = max(1, len(bias_slots))
    # x splits: shared across the 4 cores of each batch
    xsp = {}
    for b in range(B):
        for nm, x in (("q", query[b]), ("k", key_[b]), ("v", value[b])):
            xsp[(nm, b)] = _split8(_interleave_kdim(
                np.ascontiguousarray(x.T)))

    # Q/K output-dim permutation: psum partition p = head*32 + dim%32,
    # slot ot = dim//32  (head/dim within this core's 4-head group)
    perm = np.zeros((2, P), np.int64)
    for ot in range(2):
        for p_ in range(P):
            perm[ot, p_] = (p_ // 32) * HD + ot * 32 + (p_ % 32)

    in_maps = []
    for core in range(8):
        b, g = core // G, core % G
        gsl = np.arange(g * OG, (g + 1) * OG)
        qk_rows = gsl.reshape(1, OG)[0][perm.reshape(-1)]  # [256] perm'd
        wq_s = _split8(_interleave_kdim(WSCALE * Wq[qk_rows, :].T))
        wk_s = _split8(_interleave_kdim(WSCALE * Wk[qk_rows, :].T))
        wv_s = _split8(_interleave_kdim(WSCALE * Wv[gsl, :].T))
        bt = np.zeros((nbias, P, P), np.float32)
        for (i, j), slot in bias_slots.items():
            bt[slot] = bias_data[b][(i, j)]
        bt = bt.astype(ml_dtypes.bfloat16)
        in_maps.append({
            "xqh": xsp[("q", b)][0], "xql": xsp[("q", b)][1],
            "xkh": xsp[("k", b)][0], "xkl": xsp[("k", b)][1],
            "xvh": xsp[("v", b)][0], "xvl": xsp[("v", b)][1],
            "wqh": wq_s[0], "wql": wq_s[1],
            "wkh": wk_s[0], "wkl": wk_s[1],
            "wvh": wv_s[0], "wvl": wv_s[1],
            "woT": np.ascontiguousarray(Wo[:, gsl].T.astype(ml_dtypes.bfloat16)),
            "bqd": np.ascontiguousarray(bq[qk_rows].reshape(2, P).T),
            "bkd": np.ascontiguousarray(bk[qk_rows].reshape(2, P).T),
            "idd": np.eye(P, dtype=ml_dtypes.bfloat16),
            "biasT": bt,
        })

    trace = bool(int(os.environ.get("KERNEL_TRACE", "0")))
    res = bass_utils.run_bass_kernel_spmd(
        nc, in_maps, core_ids=list(range(8)), trace=trace)

    out = np.zeros((B, S, D), np.float32)
    for core in range(8):
        out[core // G] += res.results[core]["out"].astype(np.float32)
    out += (bo + bv @ Wo.T)[None, None, :]
    if _return_perf:
        return out, res
    return out


# revision 4
# speedup vs baseline: 1.0468x; 1.0011x over previous
"""Trainium2 Bass kernel for MultiHeadAttention (B=2, S=2048, D=1024, H=16).

Sharding: 8 cores = 2 (batch) x 4 (head groups of 4 heads / 256 proj cols).
Each core computes attention for its batch + head group and a partial
output projection [S, D]; host sums the 4 partials per batch and adds
bo' = bo + bv @ Wo.T (the V bias is folded into the host-side constant).

v2 pipeline (fp8e4m3 DoubleRow matmuls wherever the cost permits):
  1. Projections: 3-pass error-compensated fp8 DoubleRow
     (x_hi@W_hi + x_lo@W_hi + x_hi@W_lo), weights pre-scaled by 32 on the
     host so the fp8 residuals stay inside e4m3's dynamic range. Q/K are
     descaled (x1/32) + biased on DVE and written straight to fp8 SBUF in
     a permuted (head, halfdim) layout: psum partition p = head*32 +
     (dim%32), slot dim = dim//32. V keeps the 32x scale (the softmax
     ones-row is 32 so the reciprocal absorbs it) and is split hi/lo on
     device for an error-compensated 2-pass PV.
  2. QK^T: fp8 DoubleRow per head over 32 partitions at base head*32
     (contraction 64 = 32x2 slots), output S.T psum [sk, 2 heads, sq].
     Additive -3e4 mask bias on partial blocks as before.
  3. exp (Act, scale=1/8) -> fp8 P.T tiles with a j-pair slot dim; PV is
     a 2-pass (V hi/lo) fp8 DoubleRow over j-tile pairs. Row 64 of the
     PV psum is 32*denominator.
  4. Per head pair: DVE reciprocal of both denominators -> [2, CH], one
     PE broadcast matmul with a 0/1 selector -> [128, CH], DVE multiply
     (psum x psum) -> scaled Z.T in SBUF (f32r).
  5. Out-proj per s-tile: f32r matmuls, psum -> bf16 SBUF -> DMA out.

PSUM: "st" tag [128,2,CH] f32 (2 banks) x2 bufs + "rot" tag [128,CH]
f32 (1 bank) x4 bufs rotating over proj psums, PV accumulators, the
broadcast, and out-proj psums = 8 banks exactly.
"""

import math
import os
import sys

import numpy as np

sys.path.insert(0, "/opt/trn_rl_repo")
sys.path.insert(0, "/opt/trn_rl_repo/concourse")

B, S, D, H = 2, 2048, 1024, 16
HD = D // H  # 64
G = 4  # head groups (cores per batch)
OG = D // G  # 256 proj cols per core
HPG = H // G  # 4 heads per core
P = 128
NT = S // P  # 16 s-tiles
CH = 512  # sq chunk width
NCH = S // CH  # 4 chunks
KT2 = 4  # fp8 DoubleRow contraction steps (256 dims each)
WSCALE = 32.0  # host pre-scale on all projection weights
NEG = -30000.0  # additive mask bias (pre-scale)

_cache = {}


def _block_structure(mask, key_padding_mask):
    """Classify each 128x128 block of the [S,S] score matrix per batch.

    Returns (process, biased, bias_data) where
      process[i,j]  : bool  -- any batch needs block (sq-tile i, sk-tile j)
      biased[i,j]   : bool  -- some processed batch needs a bias on (i,j)
      bias_data[b]  : {(i,j): [128,128] f32 bias (TRANSPOSED: [sk,sq])}
    """
    mask = np.asarray(mask)
    kpm = np.asarray(key_padding_mask)
    full = np.zeros((B, NT, NT), dtype=bool)
    partial = np.zeros((B, NT, NT), dtype=bool)
    blocks = {}
    for b in range(B):
        for i in range(NT):
            mrow = mask[i * P:(i + 1) * P]
            for j in range(NT):
                mb = mrow[:, j * P:(j + 1) * P] | kpm[b, None, j * P:(j + 1) * P]
                if mb.all():
                    full[b, i, j] = True
                elif mb.any():
                    partial[b, i, j] = True
                    blocks[(b, i, j)] = mb
                else:
                    blocks[(b, i, j)] = None
    process = (~full).any(axis=0)
    biased = process & (full | partial).any(axis=0)
    bias_data = []
    for b in range(B):
        d = {}
        for i in range(NT):
            for j in range(NT):
                if not (process[i, j] and biased[i, j]):
                    continue
                if full[b, i, j]:
                    d[(i, j)] = np.full((P, P), NEG, np.float32)
                elif partial[b, i, j]:
                    d[(i, j)] = (blocks[(b, i, j)].T * NEG).astype(np.float32)
                else:
                    d[(i, j)] = np.zeros((P, P), np.float32)
        bias_data.append(d)
    return process, biased, bias_data


def _build_bass(process, biased, bias_slots):
    """Trace the Tile kernel. bias_slots: {(i,j): slot} for biased blocks."""
    import concourse.bass as bass
    import concourse.tile as tile
    from concourse import bacc, mybir

    f32 = mybir.dt.float32
    f32r = mybir.dt.float32r
    f8 = mybir.dt.float8e4
    bf16 = mybir.dt.bfloat16
    DR = mybir.MatmulPerfMode.DoubleRow
    AL = mybir.AluOpType
    EXPS = 1.0 / math.sqrt(HD)
    nc = bacc.Bacc("TRN2", target_bir_lowering=False, debug=False,
                   enable_asserts=False)

    # Host supplies fp8 hi/lo splits, contraction-interleaved:
    # x*: [p, t, slot, s] with input dim d = t*256 + slot*128 + p
    # w*: [p, t, slot, o] same d mapping; o permuted for Q/K (head*32+halfdim)
    xqh = nc.dram_tensor("xqh", [P, KT2, 2, S], f8, kind="ExternalInput").ap()
    xql = nc.dram_tensor("xql", [P, KT2, 2, S], f8, kind="ExternalInput").ap()
    xkh = nc.dram_tensor("xkh", [P, KT2, 2, S], f8, kind="ExternalInput").ap()
    xkl = nc.dram_tensor("xkl", [P, KT2, 2, S], f8, kind="ExternalInput").ap()
    xvh = nc.dram_tensor("xvh", [P, KT2, 2, S], f8, kind="ExternalInput").ap()
    xvl = nc.dram_tensor("xvl", [P, KT2, 2, S], f8, kind="ExternalInput").ap()
    wqh = nc.dram_tensor("wqh", [P, KT2, 2, OG], f8, kind="ExternalInput").ap()
    wql = nc.dram_tensor("wql", [P, KT2, 2, OG], f8, kind="ExternalInput").ap()
    wkh = nc.dram_tensor("wkh", [P, KT2, 2, OG], f8, kind="ExternalInput").ap()
    wkl = nc.dram_tensor("wkl", [P, KT2, 2, OG], f8, kind="ExternalInput").ap()
    wvh = nc.dram_tensor("wvh", [P, KT2, 2, OG], f8, kind="ExternalInput").ap()
    wvl = nc.dram_tensor("wvl", [P, KT2, 2, OG], f8, kind="ExternalInput").ap()
    woT = nc.dram_tensor("woT", [OG, D], bf16, kind="ExternalInput").ap()
    bqd = nc.dram_tensor("bqd", [P, 2], f32, kind="ExternalInput").ap()
    bkd = nc.dram_tensor("bkd", [P, 2], f32, kind="ExternalInput").ap()
    idd = nc.dram_tensor("idd", [P, P], bf16, kind="ExternalInput").ap()
    nbias = max(1, len(bias_slots))
    biasT = nc.dram_tensor("biasT", [nbias, P, P], bf16,
                           kind="ExternalInput").ap()
    out = nc.dram_tensor("out", [S, D], bf16, kind="ExternalOutput").ap()

    with tile.TileContext(nc) as tc:
        with tc.tile_pool(name="persist", bufs=1) as persist, \
             tc.tile_pool(name="const", bufs=1) as const:
            # Persistent SBUF tensors. Q/K layout: 4 slots; head h lives at
            # partition base HBASE[h], slot pair HSP[h]..HSP[h]+2 (matmul
            # operand bases must be in {0,32,64}, so head 3 wraps to base 0
            # on the second slot pair).
            HBASE = [0, 32, 64, 0]
            HSP = [0, 0, 0, 2]
            qT8 = persist.tile([P, 4, S], f8)
            kT8 = persist.tile([P, 4, S], f8)
            # V tiles padded to 128 output dims per head: cols 0:64 = v,
            # col 64 = 32.0 (denominator ones-row), cols 65:128 = 0 — the
            # dual-fp8 ldweights ISA check requires M=128 (M=65 is illegal)
            vaug_h = persist.tile([P, NT, HPG, P], f8)
            vaug_l = persist.tile([P, NT, HPG, P], f8)
            zt01 = persist.tile([P, S], bf16)    # heads 0,1 Z.T scaled
            zt23 = persist.tile([P, S], bf16)
            woT_sb = persist.tile([P, 2, D], bf16)
            bias_sb = persist.tile([P, nbias, P], bf16)

            bqs = const.tile([P, 2], f32)
            bks = const.tile([P, 2], f32)
            ident = const.tile([P, P], bf16)
            # ones-row: 32.0 in the hi V (absorbed by the reciprocal), 0 in lo
            nc.gpsimd.memset(vaug_h[:, :, :, HD:P], 0.0)
            nc.gpsimd.memset(vaug_h[:, :, :, HD:HD + 1], WSCALE)
            nc.vector.memset(vaug_l[:, :, :, HD:P], 0.0)

            # ---- Flat pools ----
            osb = tc.alloc_tile_pool(name="osb", bufs=3)
            xTp = tc.alloc_tile_pool(name="xT", bufs=4)
            wsb = tc.alloc_tile_pool(name="wsb", bufs=1)
            psum = tc.alloc_tile_pool(name="psum", bufs=1, space="PSUM")
            ptp = tc.alloc_tile_pool(name="pt", bufs=4)
            small = tc.alloc_tile_pool(name="small", bufs=2)

            wqh_sb = wsb.tile([P, KT2, 2, OG], f8, tag="wqh")
            wql_sb = wsb.tile([P, KT2, 2, OG], f8, tag="wql")
            wkh_sb = wsb.tile([P, KT2, 2, OG], f8, tag="wkh")
            wkl_sb = wsb.tile([P, KT2, 2, OG], f8, tag="wkl")
            wvh_sb = wsb.tile([P, KT2, 2, OG], f8, tag="wvh")
            wvl_sb = wsb.tile([P, KT2, 2, OG], f8, tag="wvl")
            # K weights first (first projections), split for early start;
            # tiny const loads are deferred behind the first matmul's deps
            nc.sync.dma_start(wkh_sb[:, 0:2], wkh[:, 0:2])

            # PSUM: "st" tag [P,2,CH] f32 (2 banks) x2 bufs for attention
            # scores; "rot" tag [P,CH] f32 (1 bank) x4 bufs rotating over
            # proj psums, PV accumulators, and out-proj psums = 8 banks.
            def st_tile(name):
                return psum.tile([P, 2, CH], f32, tag="st", bufs=2, name=name)

            def rot(name):
                return psum.tile([P, CH], f32, tag="rot", bufs=4, name=name)

            srcs = {0: (xkh, xkl, wkh_sb, wkl_sb),
                    1: (xvh, xvl, wvh_sb, wvl_sb),
                    2: (xqh, xql, wqh_sb, wql_sb)}

            def prep_proj(which, c, step):
                """Issue this chunk-projection's DMAs now; return per-psum
                emission closures to be interleaved into the PE stream."""
                # deferred constant loads, spread through the stream
                if step == 0:
                    nc.sync.dma_start(wkl_sb, wkl)
                    nc.sync.dma_start(bks, bkd)
                    nc.sync.dma_start(wqh_sb, wqh)
                    nc.sync.dma_start(bqs, bqd)
                    nc.sync.dma_start(ident, idd)
                elif step == 1:
                    nc.sync.dma_start(wql_sb, wql)
                    nc.sync.dma_start(wvh_sb, wvh)
                    nc.sync.dma_start(wvl_sb, wvl)
                elif step == 2:
                    nc.sync.dma_start(bias_sb,
                                      biasT.rearrange("n p q -> p n q"))
                elif step == 3:
                    nc.sync.dma_start(
                        woT_sb, woT.rearrange("(t p) d -> p t d", p=P))
                xh_dr, xl_dr, w_h, w_l = srcs[which]
                xh_t = xTp.tile([P, KT2, 2, CH], f8, tag="xT", name="xh")
                xl_t = xTp.tile([P, KT2, 2, CH], f8, tag="xT", name="xl")
                csl = slice(c * CH, (c + 1) * CH)
                if step == 0:
                    # fine-grained + issued on the idle Act queue so the
                    # first matmul's two DMAs run in parallel
                    nc.scalar.dma_start(xh_t[:, 0:2], xh_dr[:, 0:2, :, csl])
                    nc.sync.dma_start(wkh_sb[:, 2:4], wkh[:, 2:4])
                    nc.sync.dma_start(xh_t[:, 2:4], xh_dr[:, 2:4, :, csl])
                else:
                    nc.sync.dma_start(xh_t, xh_dr[:, :, :, csl])
                nc.sync.dma_start(xl_t, xl_dr[:, :, :, csl])
                units = []
                if which != 1:
                    dst8 = kT8 if which == 0 else qT8
                    bias_ap = bks if which == 0 else bqs

                    def qk_unit(ot):
                        def run():
                            ps = rot("psqk")
                            osl = slice(ot * P, (ot + 1) * P)
                            for t in range(KT2):
                                nc.tensor.matmul(ps, w_h[:, t, :, osl],
                                                 xh_t[:, t], start=(t == 0),
                                                 stop=False, perf_mode=DR)
                            for t in range(KT2):
                                nc.tensor.matmul(ps, w_l[:, t, :, osl],
                                                 xh_t[:, t], start=False,
                                                 stop=False, perf_mode=DR)
                            for t in range(KT2):
                                nc.tensor.matmul(ps, w_h[:, t, :, osl],
                                                 xl_t[:, t], start=False,
                                                 stop=(t == KT2 - 1),
                                                 perf_mode=DR)
                            nc.vector.tensor_scalar(
                                dst8[0:96, ot, csl], ps[0:96, :],
                                1.0 / WSCALE, bias_ap[0:96, ot:ot + 1],
                                op0=AL.mult, op1=AL.add)
                            nc.vector.tensor_scalar(
                                dst8[0:32, 2 + ot, csl], ps[96:128, :],
                                1.0 / WSCALE, bias_ap[96:128, ot:ot + 1],
                                op0=AL.mult, op1=AL.add)
                        return run
                    units = [qk_unit(0), qk_unit(1)]
                else:
                    def v_unit(st):
                        def run():
                            pv = rot("psv")[:, 0:OG]
                            ssl = slice(st * P, (st + 1) * P)
                            for t in range(KT2):
                                nc.tensor.matmul(pv, xh_t[:, t, :, ssl],
                                                 w_h[:, t], start=(t == 0),
                                                 stop=False, perf_mode=DR)
                            for t in range(KT2):
                                nc.tensor.matmul(pv, xh_t[:, t, :, ssl],
                                                 w_l[:, t], start=False,
                                                 stop=False, perf_mode=DR)
                            for t in range(KT2):
                                nc.tensor.matmul(pv, xl_t[:, t, :, ssl],
                                                 w_h[:, t], start=False,
                                                 stop=(t == KT2 - 1),
                                                 perf_mode=DR)
                            pv_re = pv.rearrange("p (h d) -> p h d", h=HPG)
                            vh_view = vaug_h[:, c * 4 + st, :, 0:HD]
                            nc.vector.tensor_copy(vh_view, pv_re)
                            nc.vector.tensor_tensor(
                                vaug_l[:, c * 4 + st, :, 0:HD], pv_re,
                                vh_view, op=AL.subtract)
                        return run
                    units = [v_unit(st) for st in range(CH // P)]
                return units

            # ---- Attention + out-proj, per sq-chunk ----
            # Out-proj for chunk c-1 is emitted mid-way through chunk c so
            # the (in-order) PE stream never stalls on the epilogue; the
            # epilogue itself is PE-free (DVE recip -> Pool partition
            # broadcast -> DVE multiply). Mask biases are added on the PE
            # (identity-matmul accumulate) to keep DVE off the exp path.
            ATTN_OFF = [0, 512, 1024, 1536]
            ATTN_W = [512, 512, 512, 512]
            NAC = len(ATTN_W)

            def oproj_unit(sg):
                def run():
                    ob = osb.tile([P, D], bf16, tag="ob", name="ob")
                    for nk in range(2):
                        ps = rot("psop")
                        for kk, zsrc in enumerate((zt01, zt23)):
                            nc.tensor.matmul(
                                ps, zsrc[:, sg * P:(sg + 1) * P],
                                woT_sb[:, kk, nk * CH:(nk + 1) * CH],
                                start=(kk == 0), stop=(kk == 1))
                        osl = slice(nk * CH, (nk + 1) * CH)
                        if nk == 0:
                            nc.scalar.copy(ob[:, osl], ps)
                        else:
                            nc.vector.tensor_copy(ob[:, osl], ps)
                    nc.sync.dma_start(out[sg * P:(sg + 1) * P, :], ob)
                return run

            def oproj_units(ci):
                i0 = ATTN_OFF[ci] // P
                return [oproj_unit(sg)
                        for sg in range(i0, i0 + ATTN_W[ci] // P)]

            def emit_attn(ci, fill=None, opro=None):
                fill = fill if fill is not None else []
                opro = opro if opro is not None else []
                coff, cw = ATTN_OFF[ci], ATTN_W[ci]
                i0 = coff // P
                tiles_i = list(range(i0, i0 + cw // P))
                jlist = []
                for j in range(NT):
                    ii = [i for i in tiles_i if process[i, j]]
                    if ii:
                        jlist.append((j, min(ii) - i0,
                                      max(ii) - i0 + 1))
                # pair adjacent j's for the 2-slot PV DoubleRow
                jpairs = []
                idx = 0
                while idx < len(jlist):
                    if (idx + 1 < len(jlist)
                            and jlist[idx + 1][0] == jlist[idx][0] + 1):
                        jpairs.append((jlist[idx], jlist[idx + 1]))
                        idx += 2
                    else:
                        jpairs.append((jlist[idx], None))
                        idx += 1
                for hp in range(2):  # head pairs (2*hp, 2*hp+1)
                    h0, h1 = 2 * hp, 2 * hp + 1
                    zta = {h: rot(f"zta{h}") for h in (h0, h1)}
                    first = True
                    npop = 0
                    for pi, (pa, pb) in enumerate(jpairs):
                        ja, loa, hia = pa
                        if pb is not None:
                            jb, lob, hib = pb
                            lo_u, hi_u = min(loa, lob), max(hia, hib)
                        else:
                            lo_u, hi_u = loa, hia
                        offu, wu = lo_u * P, (hi_u - lo_u) * P
                        pt = ptp.tile([P, 2, 2, CH], f8, tag="pt", name="pt")
                        for jj, ent in enumerate([pa] + ([pb] if pb else [])):
                            j_, lo_, hi_ = ent
                            off, w = lo_ * P, (hi_ - lo_) * P
                            st_ = st_tile("st_")
                            bis = [i for i in range(i0 + lo_, i0 + hi_)
                                   if biased[i, j_]]
                            for hh, h in enumerate((h0, h1)):
                                pb_, sp = HBASE[h], HSP[h]
                                nc.tensor.matmul(
                                    st_[:, hh, off:off + w],
                                    kT8[pb_:pb_ + 32, sp:sp + 2,
                                        j_ * P:(j_ + 1) * P],
                                    qT8[pb_:pb_ + 32, sp:sp + 2,
                                        coff + off:coff + off + w],
                                    start=True, stop=(not bis),
                                    perf_mode=DR)
                            # mask biases via PE identity-matmul accumulate
                            for bn, i in enumerate(bis):
                                sl = bias_slots[(i, j_)]
                                so = (i - i0) * P
                                lastb = bn == len(bis) - 1
                                for hh in range(2):
                                    nc.tensor.matmul(
                                        st_[:, hh, so:so + P], ident,
                                        bias_sb[:, sl, :], start=False,
                                        stop=(lastb and hh == 1),
                                        skip_group_check=True)
                            # zero pt where this j's band is narrower than
                            # the pair's union (PV reads the union)
                            if off > offu:
                                nc.gpsimd.memset(
                                    pt[:, jj, :, offu:off], 0.0)
                            if off + w < offu + wu:
                                nc.gpsimd.memset(
                                    pt[:, jj, :, off + w:offu + wu], 0.0)
                            nc.scalar.activation(
                                pt[:, jj, :, off:off + w],
                                st_[:, :, off:off + w],
                                mybir.ActivationFunctionType.Exp,
                                scale=EXPS)
                        last = pi == len(jpairs) - 1
                        for hh, h in enumerate((h0, h1)):
                            zo = zta[h]
                            if pb is not None:
                                nc.tensor.matmul(
                                    zo[:, offu:offu + wu],
                                    vaug_h[:, ja:ja + 2, h, :],
                                    pt[:, :, hh, offu:offu + wu],
                                    start=first, stop=False, perf_mode=DR)
                                nc.tensor.matmul(
                                    zo[:, offu:offu + wu],
                                    vaug_l[:, ja:ja + 2, h, :],
                                    pt[:, :, hh, offu:offu + wu],
                                    start=False, stop=last, perf_mode=DR)
                            else:
                                nc.tensor.matmul(
                                    zo[:, offu:offu + wu],
                                    vaug_h[:, ja, h, :],
                                    pt[:, 0, hh, offu:offu + wu],
                                    start=first, stop=False)
                                nc.tensor.matmul(
                                    zo[:, offu:offu + wu],
                                    vaug_l[:, ja, h, :],
                                    pt[:, 0, hh, offu:offu + wu],
                                    start=False, stop=last)
                        first = False
                        # interleave a projection unit into the exp-wait
                        # gap; cap 2 per head pair so the rot rotation
                        # never lands on a live PV accumulator
                        if npop < 2 and fill:
                            fill.pop(0)()
                            npop += 1
                    # epilogue (PE-free): reciprocal of 32*denominators,
                    # Pool partition-broadcast, DVE scale into SBUF
                    recs = small.tile([1, 2, CH], f32, tag="recs", bufs=2,
                                      name="recs")
                    bcs0 = small.tile([HD, CH], f32, tag="bcs0", bufs=2,
                                      name="bcs0")
                    bcs1 = small.tile([HD, CH], f32, tag="bcs1", bufs=2,
                                      name="bcs1")
                    with nc.allow_low_precision(reason="fp22 recip"):
                        nc.vector.reciprocal(recs[0:1, 0, 0:cw],
                                             zta[h0][HD:HD + 1, 0:cw])
                        nc.vector.reciprocal(recs[0:1, 1, 0:cw],
                                             zta[h1][HD:HD + 1, 0:cw])
                    # (partition_broadcast only writes at base partition 0)
                    nc.gpsimd.partition_broadcast(bcs0[:, 0:cw],
                                                  recs[0:1, 0, 0:cw],
                                                  channels=HD)
                    nc.gpsimd.partition_broadcast(bcs1[:, 0:cw],
                                                  recs[0:1, 1, 0:cw],
                                                  channels=HD)
                    zdst = zt01 if hp == 0 else zt23
                    for hh, h in enumerate((h0, h1)):
                        zpo = hh * HD
                        nc.vector.tensor_mul(
                            zdst[zpo:zpo + HD, coff:coff + cw],
                            zta[h][0:HD, 0:cw],
                            (bcs0 if hh == 0 else bcs1)[:, 0:cw])
                    # post-epilogue: both PV accumulators are drained, so
                    # any number of rot-allocating units is safe here
                    npost = 2 if hp == 0 else len(fill) + len(opro)
                    for _ in range(min(2, len(fill)) if hp == 0 else
                                   len(fill)):
                        fill.pop(0)()
                    for _ in range(min(2, len(opro)) if hp == 0 else
                                   len(opro)):
                        opro.pop(0)()

            # Interleaved schedule: causal attention chunk c needs only
            # K/V chunks 0..c and Q chunk c. Chunk c+1's projections are
            # emitted BETWEEN chunk c's head pairs so the PE keeps working
            # while the Act engine (exp, the critical resource) chews on
            # chunk c, and Act never starves at chunk boundaries.
            for u in (prep_proj(0, 0, 0) + prep_proj(2, 0, 1)
                      + prep_proj(1, 0, 2)):
                u()
            for ci in range(NAC):
                fill = []
                if ci + 1 < NCH:
                    fill += prep_proj(0, ci + 1, 3 + ci)
                    fill += prep_proj(2, ci + 1, 99)
                    fill += prep_proj(1, ci + 1, 99)
                opro = oproj_units(ci - 1) if ci > 0 else []
                emit_attn(ci, fill=fill, opro=opro)
            for u in oproj_units(NAC - 1):
                u()
            for pool_ in (small, ptp, psum, wsb, xTp, osb):
                pool_.release()
    nc.compile()
    # Belt-and-braces: any write-only preamble registers that survive DCE
    # but never get ids from alloc_regs would fail walrus birverifier
    # (reg_id == -1). They are write-only, so engine-unique ids are safe;
    # keep _lo/_hi pairs adjacent and even-aligned.
    from collections import defaultdict
    from concourse import mybir
    ctr = defaultdict(int)
    for f_ in nc.m.functions:
        for a in f_.allocations:
            if isinstance(a, mybir.Register) and a.reg_id >= 0:
                ctr[a.engine] = max(ctr[a.engine], a.reg_id + 1)
    for f_ in nc.m.functions:
        for a in f_.allocations:
            if isinstance(a, mybir.Register) and a.reg_id == -1:
                if a.name.endswith("_lo") and ctr[a.engine] % 2:
                    ctr[a.engine] += 1
                a.reg_id = ctr[a.engine]
                ctr[a.engine] += 1
    return nc


def _interleave_kdim(arr):
    """[1024 in-dim, N] -> [128 p, 4 t, 2 slot, N] with d = t*256+slot*128+p."""
    n = arr.shape[1]
    return np.ascontiguousarray(
        arr.reshape(KT2, 2, P, n).transpose(2, 0, 1, 3))


def _split8(arr):
    import ml_dtypes
    e4 = ml_dtypes.float8_e4m3
    hi = arr.astype(e4)
    lo = (arr - hi.astype(np.float32)).astype(e4)
    return np.ascontiguousarray(hi), np.ascontiguousarray(lo)


def kernel(query, key, value, mask, key_padding_mask,
           Wq, bq, Wk, bk, Wv, bv, Wo, bo, _return_perf=False):
    import ml_dtypes
    from concourse import bass_utils

    query = np.asarray(query, np.float32)
    key_ = np.asarray(key, np.float32)
    value = np.asarray(value, np.float32)
    Wq, Wk, Wv, Wo = (np.asarray(w, np.float32) for w in (Wq, Wk, Wv, Wo))
    bq, bk, bv, bo = (np.asarray(b_, np.float32) for b_ in (bq, bk, bv, bo))

    process, biased, bias_data = _block_structure(mask, key_padding_mask)
    bias_slots = {}
    for i in range(NT):
        for j in range(NT):
            if process[i, j] and biased[i, j]:
                bias_slots[(i, j)] = len(bias_slots)

    key_struct = (process.tobytes(), biased.tobytes())
    if key_struct not in _cache:
        _cache[key_struct] = _build_bass(process, biased, bias_slots)
    nc = _cache[key_struct]

    nbias = max(1, len(bias_slots))
    # x splits: shared across the 4 cores of each batch
    xsp = {}
    for b in range(B):
        for nm, x in (("q", query[b]), ("k", key_[b]), ("v", value[b])):
            xsp[(nm, b)] = _split8(_interleave_kdim(
                np.ascontiguousarray(x.T)))

    # Q/K output-dim permutation: psum partition p = head*32 + dim%32,
    # slot ot = dim//32  (head/dim within this core's 4-head group)
    perm = np.zeros((2, P), np.int64)
    for ot in range(2):
        for p_ in range(P):
            perm[ot, p_] = (p_ // 32) * HD + ot * 32 + (p_ % 32)

    in_maps = []
    for core in range(8):
        b, g = core // G, core % G
        gsl = np.arange(g * OG, (g + 1) * OG)
        qk_rows = gsl.reshape(1, OG)[0][perm.reshape(-1)]  # [256] perm'd
        wq_s = _split8(_interleave_kdim(WSCALE * Wq[qk_rows, :].T))
        wk_s = _split8(_interleave_kdim(WSCALE * Wk[qk_rows, :].T))
        wv_s = _split8(_interleave_kdim(WSCALE * Wv[gsl, :].T))
        bt = np.zeros((nbias, P, P), np.float32)
        for (i, j), slot in bias_slots.items():
            bt[slot] = bias_data[b][(i, j)]
        bt = bt.astype(ml_dtypes.bfloat16)
        in_maps.append({
            "xqh": xsp[("q", b)][0], "xql": xsp[("q", b)][1],
            "xkh": xsp[("k", b)][0], "xkl": xsp[("k", b)][1],
            "xvh": xsp[("v", b)][0], "xvl": xsp[("v", b)][1],
            "wqh": wq_s[0], "wql": wq_s[1],
            "wkh": wk_s[0], "wkl": wk_s[1],
            "wvh": wv_s[0], "wvl": wv_s[1],
            "woT": np.ascontiguousarray(Wo[:, gsl].T.astype(ml_dtypes.bfloat16)),
            "bqd": np.ascontiguousarray(bq[qk_rows].reshape(2, P).T),
            "bkd": np.ascontiguousarray(bk[qk_rows].reshape(2, P).T),
            "idd": np.eye(P, dtype=ml_dtypes.bfloat16),
            "biasT": bt,
        })

    trace = bool(int(os.environ.get("KERNEL_TRACE", "0")))
    res = bass_utils.run_bass_kernel_spmd(
        nc, in_maps, core_ids=list(range(8)), trace=trace)

    out = np.zeros((B, S, D), np.float32)
    for core in range(8):
        out[core // G] += res.results[core]["out"].astype(np.float32)
    out += (bo + bv @ Wo.T)[None, None, :]
    if _return_perf:
        return out, res
    return out


# revision 5
# speedup vs baseline: 1.0701x; 1.0223x over previous
"""Trainium2 Bass kernel for MultiHeadAttention (B=2, S=2048, D=1024, H=16).

Sharding: 8 cores = 2 (batch) x 4 (head groups of 4 heads / 256 proj cols).
Each core computes attention for its batch + head group and a partial
output projection [S, D]; host sums the 4 partials per batch and adds
bo' = bo + bv @ Wo.T (the V bias is folded into the host-side constant).

v2 pipeline (fp8e4m3 DoubleRow matmuls wherever the cost permits):
  1. Projections: 3-pass error-compensated fp8 DoubleRow
     (x_hi@W_hi + x_lo@W_hi + x_hi@W_lo), weights pre-scaled by 32 on the
     host so the fp8 residuals stay inside e4m3's dynamic range. Q/K are
     descaled (x1/32) + biased on DVE and written straight to fp8 SBUF in
     a permuted (head, halfdim) layout: psum partition p = head*32 +
     (dim%32), slot dim = dim//32. V keeps the 32x scale (the softmax
     ones-row is 32 so the reciprocal absorbs it) and is split hi/lo on
     device for an error-compensated 2-pass PV.
  2. QK^T: fp8 DoubleRow per head over 32 partitions at base head*32
     (contraction 64 = 32x2 slots), output S.T psum [sk, 2 heads, sq].
     Additive -3e4 mask bias on partial blocks as before.
  3. exp (Act, scale=1/8) -> fp8 P.T tiles with a j-pair slot dim; PV is
     a 2-pass (V hi/lo) fp8 DoubleRow over j-tile pairs. Row 64 of the
     PV psum is 32*denominator.
  4. Per head pair: DVE reciprocal of both denominators -> [2, CH], one
     PE broadcast matmul with a 0/1 selector -> [128, CH], DVE multiply
     (psum x psum) -> scaled Z.T in SBUF (f32r).
  5. Out-proj per s-tile: f32r matmuls, psum -> bf16 SBUF -> DMA out.

PSUM: "st" tag [128,2,CH] f32 (2 banks) x2 bufs + "rot" tag [128,CH]
f32 (1 bank) x4 bufs rotating over proj psums, PV accumulators, the
broadcast, and out-proj psums = 8 banks exactly.
"""

import math
import os
import sys

import numpy as np

sys.path.insert(0, "/opt/trn_rl_repo")
sys.path.insert(0, "/opt/trn_rl_repo/concourse")

B, S, D, H = 2, 2048, 1024, 16
HD = D // H  # 64
G = 4  # head groups (cores per batch)
OG = D // G  # 256 proj cols per core
HPG = H // G  # 4 heads per core
P = 128
NT = S // P  # 16 s-tiles
CH = 512  # sq chunk width
NCH = S // CH  # 4 chunks
KT2 = 4  # fp8 DoubleRow contraction steps (256 dims each)
WSCALE = 32.0  # host pre-scale on all projection weights
NEG = -30000.0  # additive mask bias (pre-scale)

_cache = {}


def _block_structure(mask, key_padding_mask):
    """Classify each 128x128 block of the [S,S] score matrix per batch.

    Returns (process, biased, bias_data) where
      process[i,j]  : bool  -- any batch needs block (sq-tile i, sk-tile j)
      biased[i,j]   : bool  -- some processed batch needs a bias on (i,j)
      bias_data[b]  : {(i,j): [128,128] f32 bias (TRANSPOSED: [sk,sq])}
    """
    mask = np.asarray(mask)
    kpm = np.asarray(key_padding_mask)
    full = np.zeros((B, NT, NT), dtype=bool)
    partial = np.zeros((B, NT, NT), dtype=bool)
    blocks = {}
    for b in range(B):
        for i in range(NT):
            mrow = mask[i * P:(i + 1) * P]
            for j in range(NT):
                mb = mrow[:, j * P:(j + 1) * P] | kpm[b, None, j * P:(j + 1) * P]
                if mb.all():
                    full[b, i, j] = True
                elif mb.any():
                    partial[b, i, j] = True
                    blocks[(b, i, j)] = mb
                else:
                    blocks[(b, i, j)] = None
    process = (~full).any(axis=0)
    biased = process & (full | partial).any(axis=0)
    bias_data = []
    for b in range(B):
        d = {}
        for i in range(NT):
            for j in range(NT):
                if not (process[i, j] and biased[i, j]):
                    continue
                if full[b, i, j]:
                    d[(i, j)] = np.full((P, P), NEG, np.float32)
                elif partial[b, i, j]:
                    d[(i, j)] = (blocks[(b, i, j)].T * NEG).astype(np.float32)
                else:
                    d[(i, j)] = np.zeros((P, P), np.float32)
        bias_data.append(d)
    return process, biased, bias_data


def _build_bass(process, biased, bias_slots):
    """Trace the Tile kernel. bias_slots: {(i,j): slot} for biased blocks."""
    import concourse.bass as bass
    import concourse.tile as tile
    from concourse import bacc, mybir

    f32 = mybir.dt.float32
    f32r = mybir.dt.float32r
    f8 = mybir.dt.float8e4
    bf16 = mybir.dt.bfloat16
    DR = mybir.MatmulPerfMode.DoubleRow
    AL = mybir.AluOpType
    EXPS = 1.0 / math.sqrt(HD)
    nc = bacc.Bacc("TRN2", target_bir_lowering=False, debug=False,
                   enable_asserts=False)

    # Host supplies fp8 hi/lo splits, contraction-interleaved:
    # x*: [p, t, slot, s] with input dim d = t*256 + slot*128 + p
    # w*: [p, t, slot, o] same d mapping; o permuted for Q/K (head*32+halfdim)
    xqh = nc.dram_tensor("xqh", [P, KT2, 2, S], f8, kind="ExternalInput").ap()
    xql = nc.dram_tensor("xql", [P, KT2, 2, S], f8, kind="ExternalInput").ap()
    xkh = nc.dram_tensor("xkh", [P, KT2, 2, S], f8, kind="ExternalInput").ap()
    xkl = nc.dram_tensor("xkl", [P, KT2, 2, S], f8, kind="ExternalInput").ap()
    xvh = nc.dram_tensor("xvh", [P, KT2, 2, S], f8, kind="ExternalInput").ap()
    xvl = nc.dram_tensor("xvl", [P, KT2, 2, S], f8, kind="ExternalInput").ap()
    wqh = nc.dram_tensor("wqh", [P, KT2, 2, OG], f8, kind="ExternalInput").ap()
    wql = nc.dram_tensor("wql", [P, KT2, 2, OG], f8, kind="ExternalInput").ap()
    wkh = nc.dram_tensor("wkh", [P, KT2, 2, OG], f8, kind="ExternalInput").ap()
    wkl = nc.dram_tensor("wkl", [P, KT2, 2, OG], f8, kind="ExternalInput").ap()
    wvh = nc.dram_tensor("wvh", [P, KT2, 2, OG], f8, kind="ExternalInput").ap()
    wvl = nc.dram_tensor("wvl", [P, KT2, 2, OG], f8, kind="ExternalInput").ap()
    woT = nc.dram_tensor("woT", [OG, D], bf16, kind="ExternalInput").ap()
    bqd = nc.dram_tensor("bqd", [P, 2], f32, kind="ExternalInput").ap()
    bkd = nc.dram_tensor("bkd", [P, 2], f32, kind="ExternalInput").ap()
    idd = nc.dram_tensor("idd", [P, P], bf16, kind="ExternalInput").ap()
    nbias = max(1, len(bias_slots))
    biasT = nc.dram_tensor("biasT", [nbias, P, P], bf16,
                           kind="ExternalInput").ap()
    out = nc.dram_tensor("out", [S, D], bf16, kind="ExternalOutput").ap()

    with tile.TileContext(nc) as tc:
        with tc.tile_pool(name="persist", bufs=1) as persist, \
             tc.tile_pool(name="const", bufs=1) as const:
            # Persistent SBUF tensors. Q/K layout: 4 slots; head h lives at
            # partition base HBASE[h], slot pair HSP[h]..HSP[h]+2 (matmul
            # operand bases must be in {0,32,64}, so head 3 wraps to base 0
            # on the second slot pair).
            HBASE = [0, 32, 64, 0]
            HSP = [0, 0, 0, 2]
            qT8 = persist.tile([P, 4, S], f8)
            kT8 = persist.tile([P, 4, S], f8)
            # V tiles padded to 128 output dims per head: cols 0:64 = v,
            # col 64 = 32.0 (denominator ones-row), cols 65:128 = 0 — the
            # dual-fp8 ldweights ISA check requires M=128 (M=65 is illegal)
            vaug_h = persist.tile([P, NT, HPG, P], f8)
            vaug_l = persist.tile([P, NT, HPG, P], f8)
            zt01 = persist.tile([P, S], bf16)    # heads 0,1 Z.T scaled
            zt23 = persist.tile([P, S], bf16)
            woT_sb = persist.tile([P, 2, D], bf16)
            bias_sb = persist.tile([P, nbias, P], bf16)

            bqs = const.tile([P, 2], f32)
            bks = const.tile([P, 2], f32)
            ident = const.tile([P, P], bf16)
            # ones-row: 32.0 in the hi V (absorbed by the reciprocal), 0 in lo
            nc.gpsimd.memset(vaug_h[:, :, :, HD:P], 0.0)
            nc.gpsimd.memset(vaug_h[:, :, :, HD:HD + 1], WSCALE)
            nc.vector.memset(vaug_l[:, :, :, HD:P], 0.0)

            # ---- Flat pools ----
            osb = tc.alloc_tile_pool(name="osb", bufs=3)
            xTp = tc.alloc_tile_pool(name="xT", bufs=4)
            wsb = tc.alloc_tile_pool(name="wsb", bufs=1)
            psum = tc.alloc_tile_pool(name="psum", bufs=1, space="PSUM")
            ptp = tc.alloc_tile_pool(name="pt", bufs=4)
            small = tc.alloc_tile_pool(name="small", bufs=2)

            wqh_sb = wsb.tile([P, KT2, 2, OG], f8, tag="wqh")
            wql_sb = wsb.tile([P, KT2, 2, OG], f8, tag="wql")
            wkh_sb = wsb.tile([P, KT2, 2, OG], f8, tag="wkh")
            wkl_sb = wsb.tile([P, KT2, 2, OG], f8, tag="wkl")
            wvh_sb = wsb.tile([P, KT2, 2, OG], f8, tag="wvh")
            wvl_sb = wsb.tile([P, KT2, 2, OG], f8, tag="wvl")
            # K weights first (first projections), split for early start;
            # tiny const loads are deferred behind the first matmul's deps
            nc.sync.dma_start(wkh_sb[:, 0:2], wkh[:, 0:2])

            # PSUM: "st" tag [P,2,CH] f32 (2 banks) x2 bufs for attention
            # scores; "rot" tag [P,CH] f32 (1 bank) x4 bufs rotating over
            # proj psums, PV accumulators, and out-proj psums = 8 banks.
            def st_tile(name):
                return psum.tile([P, 2, CH], f32, tag="st", bufs=2, name=name)

            def rot(name):
                return psum.tile([P, CH], f32, tag="rot", bufs=4, name=name)

            srcs = {0: (xkh, xkl, wkh_sb, wkl_sb),
                    1: (xvh, xvl, wvh_sb, wvl_sb),
                    2: (xqh, xql, wqh_sb, wql_sb)}

            def prep_proj(which, c, step):
                """Issue this chunk-projection's DMAs now; return per-psum
                emission closures to be interleaved into the PE stream."""
                # deferred constant loads, spread through the stream
                if step == 1:
                    nc.sync.dma_start(wql_sb, wql)
                    nc.sync.dma_start(wvh_sb, wvh)
                    nc.sync.dma_start(wvl_sb, wvl)
                elif step == 2:
                    nc.sync.dma_start(bias_sb,
                                      biasT.rearrange("n p q -> p n q"))
                elif step == 3:
                    nc.sync.dma_start(
                        woT_sb, woT.rearrange("(t p) d -> p t d", p=P))
                xh_dr, xl_dr, w_h, w_l = srcs[which]
                xh_t = xTp.tile([P, KT2, 2, CH], f8, tag="xT", bufs=6, name="xh")
                xl_t = xTp.tile([P, KT2, 2, CH], f8, tag="xT", bufs=6, name="xl")
                csl = slice(c * CH, (c + 1) * CH)
                if step == 0:
                    # fine-grained, first-needed-first; the x halves go out
                    # on the idle Act queue in parallel with the SP stream
                    nc.scalar.dma_start(xh_t[:, 0:2], xh_dr[:, 0:2, :, csl])
                    nc.sync.dma_start(wkh_sb[:, 2:4], wkh[:, 2:4])
                    nc.scalar.dma_start(xh_t[:, 2:4], xh_dr[:, 2:4, :, csl])
                    nc.sync.dma_start(wkl_sb, wkl)
                    nc.sync.dma_start(bks, bkd)
                    nc.sync.dma_start(wqh_sb, wqh)
                    nc.sync.dma_start(bqs, bqd)
                    nc.sync.dma_start(ident, idd)
                else:
                    nc.sync.dma_start(xh_t, xh_dr[:, :, :, csl])
                nc.sync.dma_start(xl_t, xl_dr[:, :, :, csl])
                units = []
                if which != 1:
                    dst8 = kT8 if which == 0 else qT8
                    bias_ap = bks if which == 0 else bqs

                    def qk_unit(ot):
                        def run():
                            ps = rot("psqk")
                            osl = slice(ot * P, (ot + 1) * P)
                            for t in range(KT2):
                                nc.tensor.matmul(ps, w_h[:, t, :, osl],
                                                 xh_t[:, t], start=(t == 0),
                                                 stop=False, perf_mode=DR)
                            for t in range(KT2):
                                nc.tensor.matmul(ps, w_l[:, t, :, osl],
                                                 xh_t[:, t], start=False,
                                                 stop=False, perf_mode=DR)
                            for t in range(KT2):
                                nc.tensor.matmul(ps, w_h[:, t, :, osl],
                                                 xl_t[:, t], start=False,
                                                 stop=(t == KT2 - 1),
                                                 perf_mode=DR)
                            nc.vector.tensor_scalar(
                                dst8[0:96, ot, csl], ps[0:96, :],
                                1.0 / WSCALE, bias_ap[0:96, ot:ot + 1],
                                op0=AL.mult, op1=AL.add)
                            nc.vector.tensor_scalar(
                                dst8[0:32, 2 + ot, csl], ps[96:128, :],
                                1.0 / WSCALE, bias_ap[96:128, ot:ot + 1],
                                op0=AL.mult, op1=AL.add)
                        return run
                    units = [qk_unit(0), qk_unit(1)]
                else:
                    def v_unit(st):
                        def run():
                            pv = rot("psv")[:, 0:OG]
                            ssl = slice(st * P, (st + 1) * P)
                            for t in range(KT2):
                                nc.tensor.matmul(pv, xh_t[:, t, :, ssl],
                                                 w_h[:, t], start=(t == 0),
                                                 stop=False, perf_mode=DR)
                            for t in range(KT2):
                                nc.tensor.matmul(pv, xh_t[:, t, :, ssl],
                                                 w_l[:, t], start=False,
                                                 stop=False, perf_mode=DR)
                            for t in range(KT2):
                                nc.tensor.matmul(pv, xl_t[:, t, :, ssl],
                                                 w_h[:, t], start=False,
                                                 stop=(t == KT2 - 1),
                                                 perf_mode=DR)
                            pv_re = pv.rearrange("p (h d) -> p h d", h=HPG)
                            vh_view = vaug_h[:, c * 4 + st, :, 0:HD]
                            nc.vector.tensor_copy(vh_view, pv_re)
                            nc.vector.tensor_tensor(
                                vaug_l[:, c * 4 + st, :, 0:HD], pv_re,
                                vh_view, op=AL.subtract)
                        return run
                    units = [v_unit(st) for st in range(CH // P)]
                return units

            # ---- Attention + out-proj, per sq-chunk ----
            # Out-proj for chunk c-1 is emitted mid-way through chunk c so
            # the (in-order) PE stream never stalls on the epilogue; the
            # epilogue itself is PE-free (DVE recip -> Pool partition
            # broadcast -> DVE multiply). Mask biases are added on the PE
            # (identity-matmul accumulate) to keep DVE off the exp path.
            ATTN_OFF = [0, 512, 1024, 1536]
            ATTN_W = [512, 512, 512, 512]
            NAC = len(ATTN_W)

            def oproj_unit(sg):
                def run():
                    ob = osb.tile([P, D], bf16, tag="ob", name="ob")
                    for nk in range(2):
                        ps = rot("psop")
                        for kk, zsrc in enumerate((zt01, zt23)):
                            nc.tensor.matmul(
                                ps, zsrc[:, sg * P:(sg + 1) * P],
                                woT_sb[:, kk, nk * CH:(nk + 1) * CH],
                                start=(kk == 0), stop=(kk == 1))
                        osl = slice(nk * CH, (nk + 1) * CH)
                        if nk == 0:
                            nc.scalar.copy(ob[:, osl], ps)
                        else:
                            nc.vector.tensor_copy(ob[:, osl], ps)
                    nc.sync.dma_start(out[sg * P:(sg + 1) * P, :], ob)
                return run

            def oproj_units(ci):
                i0 = ATTN_OFF[ci] // P
                return [oproj_unit(sg)
                        for sg in range(i0, i0 + ATTN_W[ci] // P)]

            def emit_attn(ci, fill=None, opro=None):
                fill = fill if fill is not None else []
                opro = opro if opro is not None else []
                coff, cw = ATTN_OFF[ci], ATTN_W[ci]
                i0 = coff // P
                tiles_i = list(range(i0, i0 + cw // P))
                jlist = []
                for j in range(NT):
                    ii = [i for i in tiles_i if process[i, j]]
                    if ii:
                        jlist.append((j, min(ii) - i0,
                                      max(ii) - i0 + 1))
                # pair adjacent j's for the 2-slot PV DoubleRow
                jpairs = []
                idx = 0
                while idx < len(jlist):
                    if (idx + 1 < len(jlist)
                            and jlist[idx + 1][0] == jlist[idx][0] + 1):
                        jpairs.append((jlist[idx], jlist[idx + 1]))
                        idx += 2
                    else:
                        jpairs.append((jlist[idx], None))
                        idx += 1
                for hp in range(2):  # head pairs (2*hp, 2*hp+1)
                    h0, h1 = 2 * hp, 2 * hp + 1
                    zta = {h: rot(f"zta{h}") for h in (h0, h1)}
                    first = True
                    npop = 0
                    for pi, (pa, pb) in enumerate(jpairs):
                        ja, loa, hia = pa
                        if pb is not None:
                            jb, lob, hib = pb
                            lo_u, hi_u = min(loa, lob), max(hia, hib)
                        else:
                            lo_u, hi_u = loa, hia
                        offu, wu = lo_u * P, (hi_u - lo_u) * P
                        pt = ptp.tile([P, 2, 2, CH], f8, tag="pt", bufs=6, name="pt")
                        for jj, ent in enumerate([pa] + ([pb] if pb else [])):
                            j_, lo_, hi_ = ent
                            off, w = lo_ * P, (hi_ - lo_) * P
                            st_ = st_tile("st_")
                            bis = [i for i in range(i0 + lo_, i0 + hi_)
                                   if biased[i, j_]]
                            for hh, h in enumerate((h0, h1)):
                                pb_, sp = HBASE[h], HSP[h]
                                nc.tensor.matmul(
                                    st_[:, hh, off:off + w],
                                    kT8[pb_:pb_ + 32, sp:sp + 2,
                                        j_ * P:(j_ + 1) * P],
                                    qT8[pb_:pb_ + 32, sp:sp + 2,
                                        coff + off:coff + off + w],
                                    start=True, stop=(not bis),
                                    perf_mode=DR)
                            # mask biases via PE identity-matmul accumulate
                            for bn, i in enumerate(bis):
                                sl = bias_slots[(i, j_)]
                                so = (i - i0) * P
                                lastb = bn == len(bis) - 1
                                for hh in range(2):
                                    nc.tensor.matmul(
                                        st_[:, hh, so:so + P], ident,
                                        bias_sb[:, sl, :], start=False,
                                        stop=(lastb and hh == 1),
                                        skip_group_check=True)
                            # zero pt where this j's band is narrower than
                            # the pair's union (PV reads the union)
                            if off > offu:
                                nc.gpsimd.memset(
                                    pt[:, jj, :, offu:off], 0.0)
                            if off + w < offu + wu:
                                nc.gpsimd.memset(
                                    pt[:, jj, :, off + w:offu + wu], 0.0)
                            nc.scalar.activation(
                                pt[:, jj, :, off:off + w],
                                st_[:, :, off:off + w],
                                mybir.ActivationFunctionType.Exp,
                                scale=EXPS)
                        last = pi == len(jpairs) - 1
                        for hh, h in enumerate((h0, h1)):
                            zo = zta[h]
                            if pb is not None:
                                nc.tensor.matmul(
                                    zo[:, offu:offu + wu],
                                    vaug_h[:, ja:ja + 2, h, :],
                                    pt[:, :, hh, offu:offu + wu],
                                    start=first, stop=False, perf_mode=DR)
                                nc.tensor.matmul(
                                    zo[:, offu:offu + wu],
                                    vaug_l[:, ja:ja + 2, h, :],
                                    pt[:, :, hh, offu:offu + wu],
                                    start=False, stop=last, perf_mode=DR)
                            else:
                                nc.tensor.matmul(
                                    zo[:, offu:offu + wu],
                                    vaug_h[:, ja, h, :],
                                    pt[:, 0, hh, offu:offu + wu],
                                    start=first, stop=False)
                                nc.tensor.matmul(
                                    zo[:, offu:offu + wu],
                                    vaug_l[:, ja, h, :],
                                    pt[:, 0, hh, offu:offu + wu],
                                    start=False, stop=last)
                        first = False
                        # interleave a projection unit into the exp-wait
                        # gap; cap 2 per head pair so the rot rotation
                        # never lands on a live PV accumulator
                        if npop < 2 and fill:
                            fill.pop(0)()
                            npop += 1
                    # epilogue (PE-free): reciprocal of 32*denominators,
                    # Pool partition-broadcast, DVE scale into SBUF
                    recs = small.tile([1, 2, CH], f32, tag="recs", bufs=2,
                                      name="recs")
                    bcs0 = small.tile([HD, CH], f32, tag="bcs0", bufs=2,
                                      name="bcs0")
                    bcs1 = small.tile([HD, CH], f32, tag="bcs1", bufs=2,
                                      name="bcs1")
                    with nc.allow_low_precision(reason="fp22 recip"):
                        nc.vector.reciprocal(recs[0:1, 0, 0:cw],
                                             zta[h0][HD:HD + 1, 0:cw])
                        nc.vector.reciprocal(recs[0:1, 1, 0:cw],
                                             zta[h1][HD:HD + 1, 0:cw])
                    # (partition_broadcast only writes at base partition 0)
                    nc.gpsimd.partition_broadcast(bcs0[:, 0:cw],
                                                  recs[0:1, 0, 0:cw],
                                                  channels=HD)
                    nc.gpsimd.partition_broadcast(bcs1[:, 0:cw],
                                                  recs[0:1, 1, 0:cw],
                                                  channels=HD)
                    zdst = zt01 if hp == 0 else zt23
                    for hh, h in enumerate((h0, h1)):
                        zpo = hh * HD
                        nc.vector.tensor_mul(
                            zdst[zpo:zpo + HD, coff:coff + cw],
                            zta[h][0:HD, 0:cw],
                            (bcs0 if hh == 0 else bcs1)[:, 0:cw])
                    # post-epilogue: both PV accumulators are drained, so
                    # any number of rot-allocating units is safe here
                    npost = 2 if hp == 0 else len(fill) + len(opro)
                    for _ in range(min(2, len(fill)) if hp == 0 else
                                   len(fill)):
                        fill.pop(0)()
                    for _ in range(min(2, len(opro)) if hp == 0 else
                                   len(opro)):
                        opro.pop(0)()

            # Interleaved schedule: causal attention chunk c needs only
            # K/V chunks 0..c and Q chunk c. Chunk c+1's projections are
            # emitted BETWEEN chunk c's head pairs so the PE keeps working
            # while the Act engine (exp, the critical resource) chews on
            # chunk c, and Act never starves at chunk boundaries.
            for u in (prep_proj(0, 0, 0) + prep_proj(2, 0, 1)
                      + prep_proj(1, 0, 2)):
                u()
            for ci in range(NAC):
                fill = []
                if ci + 1 < NCH:
                    fill += prep_proj(0, ci + 1, 3 + ci)
                    fill += prep_proj(2, ci + 1, 99)
                    fill += prep_proj(1, ci + 1, 99)
                opro = oproj_units(ci - 1) if ci > 0 else []
                emit_attn(ci, fill=fill, opro=opro)
            for u in oproj_units(NAC - 1):
                u()
            for pool_ in (small, ptp, psum, wsb, xTp, osb):
                pool_.release()
    nc.compile()
    # Belt-and-braces: any write-only preamble registers that survive DCE
    # but never get ids from alloc_regs would fail walrus birverifier
    # (reg_id == -1). They are write-only, so engine-unique ids are safe;
    # keep _lo/_hi pairs adjacent and even-aligned.
    from collections import defaultdict
    from concourse import mybir
    ctr = defaultdict(int)
    for f_ in nc.m.functions:
        for a in f_.allocations:
            if isinstance(a, mybir.Register) and a.reg_id >= 0:
                ctr[a.engine] = max(ctr[a.engine], a.reg_id + 1)
    for f_ in nc.m.functions:
        for a in f_.allocations:
            if isinstance(a, mybir.Register) and a.reg_id == -1:
                if a.name.endswith("_lo") and ctr[a.engine] % 2:
                    ctr[a.engine] += 1
                a.reg_id = ctr[a.engine]
                ctr[a.engine] += 1
    return nc


def _interleave_kdim(arr):
    """[1024 in-dim, N] -> [128 p, 4 t, 2 slot, N] with d = t*256+slot*128+p."""
    n = arr.shape[1]
    return np.ascontiguousarray(
        arr.reshape(KT2, 2, P, n).transpose(2, 0, 1, 3))


def _split8(arr):
    import ml_dtypes
    e4 = ml_dtypes.float8_e4m3
    hi = arr.astype(e4)
    lo = (arr - hi.astype(np.float32)).astype(e4)
    return np.ascontiguousarray(hi), np.ascontiguousarray(lo)


def kernel(query, key, value, mask, key_padding_mask,
           Wq, bq, Wk, bk, Wv, bv, Wo, bo, _return_perf=False):
    import ml_dtypes
    from concourse import bass_utils

    query = np.asarray(query, np.float32)
    key_ = np.asarray(key, np.float32)
    value = np.asarray(value, np.float32)
    Wq, Wk, Wv, Wo = (np.asarray(w, np.float32) for w in (Wq, Wk, Wv, Wo))
    bq, bk, bv, bo = (np.asarray(b_, np.float32) for b_ in (bq, bk, bv, bo))

    process, biased, bias_data = _block_structure(mask, key_padding_mask)
    bias_slots = {}
    for i in range(NT):
        for j in range(NT):
            if process[i, j] and biased[i, j]:
                bias_slots[(i, j)] = len(bias_slots)

    key_struct = (process.tobytes(), biased.tobytes())
    if key_struct not in _cache:
        _cache[key_struct] = _build_bass(process, biased, bias_slots)
    nc = _cache[key_struct]

    nbias = max(1, len(bias_slots))
    # x splits: shared across the 4 cores of each batch
    xsp = {}
    for b in range(B):
        for nm, x in (("q", query[b]), ("k", key_[b]), ("v", value[b])):
            xsp[(nm, b)] = _split8(_interleave_kdim(
                np.ascontiguousarray(x.T)))

    # Q/K output-dim permutation: psum partition p = head*32 + dim%32,
    # slot ot = dim//32  (head/dim within this core's 4-head group)
    perm = np.zeros((2, P), np.int64)
    for ot in range(2):
        for p_ in range(P):
            perm[ot, p_] = (p_ // 32) * HD + ot * 32 + (p_ % 32)

    in_maps = []
    for core in range(8):
        b, g = core // G, core % G
        gsl = np.arange(g * OG, (g + 1) * OG)
        qk_rows = gsl.reshape(1, OG)[0][perm.reshape(-1)]  # [256] perm'd
        wq_s = _split8(_interleave_kdim(WSCALE * Wq[qk_rows, :].T))
        wk_s = _split8(_interleave_kdim(WSCALE * Wk[qk_rows, :].T))
        wv_s = _split8(_interleave_kdim(WSCALE * Wv[gsl, :].T))
        bt = np.zeros((nbias, P, P), np.float32)
        for (i, j), slot in bias_slots.items():
            bt[slot] = bias_data[b][(i, j)]
        bt = bt.astype(ml_dtypes.bfloat16)
        in_maps.append({
            "xqh": xsp[("q", b)][0], "xql": xsp[("q", b)][1],
            "xkh": xsp[("k", b)][0], "xkl": xsp[("k", b)][1],
            "xvh": xsp[("v", b)][0], "xvl": xsp[("v", b)][1],
            "wqh": wq_s[0], "wql": wq_s[1],
            "wkh": wk_s[0], "wkl": wk_s[1],
            "wvh": wv_s[0], "wvl": wv_s[1],
            "woT": np.ascontiguousarray(Wo[:, gsl].T.astype(ml_dtypes.bfloat16)),
            "bqd": np.ascontiguousarray(bq[qk_rows].reshape(2, P).T),
            "bkd": np.ascontiguousarray(bk[qk_rows].reshape(2, P).T),
            "idd": np.eye(P, dtype=ml_dtypes.bfloat16),
            "biasT": bt,
        })

    trace = bool(int(os.environ.get("KERNEL_TRACE", "0")))
    res = bass_utils.run_bass_kernel_spmd(
        nc, in_maps, core_ids=list(range(8)), trace=trace)

    out = np.zeros((B, S, D), np.float32)
    for core in range(8):
        out[core // G] += res.results[core]["out"].astype(np.float32)
    out += (bo + bv @ Wo.T)[None, None, :]
    if _return_perf:
        return out, res
    return out


# revision 6
# speedup vs baseline: 1.0837x; 1.0127x over previous
"""Trainium2 Bass kernel for MultiHeadAttention (B=2, S=2048, D=1024, H=16).

Sharding: 8 cores = 2 (batch) x 4 (head groups of 4 heads / 256 proj cols).
Each core computes attention for its batch + head group and a partial
output projection [S, D]; host sums the 4 partials per batch and adds
bo' = bo + bv @ Wo.T (the V bias is folded into the host-side constant).

v2 pipeline (fp8e4m3 DoubleRow matmuls wherever the cost permits):
  1. Projections: 3-pass error-compensated fp8 DoubleRow
     (x_hi@W_hi + x_lo@W_hi + x_hi@W_lo), weights pre-scaled by 32 on the
     host so the fp8 residuals stay inside e4m3's dynamic range. Q/K are
     descaled (x1/32) + biased on DVE and written straight to fp8 SBUF in
     a permuted (head, halfdim) layout: psum partition p = head*32 +
     (dim%32), slot dim = dim//32. V keeps the 32x scale (the softmax
     ones-row is 32 so the reciprocal absorbs it) and is split hi/lo on
     device for an error-compensated 2-pass PV.
  2. QK^T: fp8 DoubleRow per head over 32 partitions at base head*32
     (contraction 64 = 32x2 slots), output S.T psum [sk, 2 heads, sq].
     Additive -3e4 mask bias on partial blocks as before.
  3. exp (Act, scale=1/8) -> fp8 P.T tiles with a j-pair slot dim; PV is
     a 2-pass (V hi/lo) fp8 DoubleRow over j-tile pairs. Row 64 of the
     PV psum is 32*denominator.
  4. Per head pair: DVE reciprocal of both denominators -> [2, CH], one
     PE broadcast matmul with a 0/1 selector -> [128, CH], DVE multiply
     (psum x psum) -> scaled Z.T in SBUF (f32r).
  5. Out-proj per s-tile: f32r matmuls, psum -> bf16 SBUF -> DMA out.

PSUM: "st" tag [128,2,CH] f32 (2 banks) x2 bufs + "rot" tag [128,CH]
f32 (1 bank) x4 bufs rotating over proj psums, PV accumulators, the
broadcast, and out-proj psums = 8 banks exactly.
"""

import math
import os
import sys

import numpy as np

sys.path.insert(0, "/opt/trn_rl_repo")
sys.path.insert(0, "/opt/trn_rl_repo/concourse")

B, S, D, H = 2, 2048, 1024, 16
HD = D // H  # 64
G = 4  # head groups (cores per batch)
OG = D // G  # 256 proj cols per core
HPG = H // G  # 4 heads per core
P = 128
NT = S // P  # 16 s-tiles
CH = 512  # sq chunk width
NCH = S // CH  # 4 chunks
KT2 = 4  # fp8 DoubleRow contraction steps (256 dims each)
WSCALE = 32.0  # host pre-scale on all projection weights
NEG = -30000.0  # additive mask bias (pre-scale)

_cache = {}


def _block_structure(mask, key_padding_mask):
    """Classify each 128x128 block of the [S,S] score matrix per batch.

    Returns (process, biased, bias_data) where
      process[i,j]  : bool  -- any batch needs block (sq-tile i, sk-tile j)
      biased[i,j]   : bool  -- some processed batch needs a bias on (i,j)
      bias_data[b]  : {(i,j): [128,128] f32 bias (TRANSPOSED: [sk,sq])}
    """
    mask = np.asarray(mask)
    kpm = np.asarray(key_padding_mask)
    full = np.zeros((B, NT, NT), dtype=bool)
    partial = np.zeros((B, NT, NT), dtype=bool)
    blocks = {}
    for b in range(B):
        for i in range(NT):
            mrow = mask[i * P:(i + 1) * P]
            for j in range(NT):
                mb = mrow[:, j * P:(j + 1) * P] | kpm[b, None, j * P:(j + 1) * P]
                if mb.all():
                    full[b, i, j] = True
                elif mb.any():
                    partial[b, i, j] = True
                    blocks[(b, i, j)] = mb
                else:
                    blocks[(b, i, j)] = None
    process = (~full).any(axis=0)
    biased = process & (full | partial).any(axis=0)
    bias_data = []
    for b in range(B):
        d = {}
        for i in range(NT):
            for j in range(NT):
                if not (process[i, j] and biased[i, j]):
                    continue
                if full[b, i, j]:
                    d[(i, j)] = np.full((P, P), NEG, np.float32)
                elif partial[b, i, j]:
                    d[(i, j)] = (blocks[(b, i, j)].T * NEG).astype(np.float32)
                else:
                    d[(i, j)] = np.zeros((P, P), np.float32)
        bias_data.append(d)
    return process, biased, bias_data


def _build_bass(process, biased, bias_slots):
    """Trace the Tile kernel. bias_slots: {(i,j): slot} for biased blocks."""
    import concourse.bass as bass
    import concourse.tile as tile
    from concourse import bacc, mybir

    f32 = mybir.dt.float32
    f32r = mybir.dt.float32r
    f8 = mybir.dt.float8e4
    bf16 = mybir.dt.bfloat16
    DR = mybir.MatmulPerfMode.DoubleRow
    AL = mybir.AluOpType
    EXPS = 1.0 / math.sqrt(HD)
    nc = bacc.Bacc("TRN2", target_bir_lowering=False, debug=False,
                   enable_asserts=False)

    # Host supplies fp8 hi/lo splits, contraction-interleaved:
    # x*: [p, t, slot, s] with input dim d = t*256 + slot*128 + p
    # w*: [p, t, slot, o] same d mapping; o permuted for Q/K (head*32+halfdim)
    xqh = nc.dram_tensor("xqh", [P, KT2, 2, S], f8, kind="ExternalInput").ap()
    xql = nc.dram_tensor("xql", [P, KT2, 2, S], f8, kind="ExternalInput").ap()
    xkh = nc.dram_tensor("xkh", [P, KT2, 2, S], f8, kind="ExternalInput").ap()
    xkl = nc.dram_tensor("xkl", [P, KT2, 2, S], f8, kind="ExternalInput").ap()
    xvh = nc.dram_tensor("xvh", [P, KT2, 2, S], f8, kind="ExternalInput").ap()
    xvl = nc.dram_tensor("xvl", [P, KT2, 2, S], f8, kind="ExternalInput").ap()
    wqh = nc.dram_tensor("wqh", [P, KT2, 2, OG], f8, kind="ExternalInput").ap()
    wql = nc.dram_tensor("wql", [P, KT2, 2, OG], f8, kind="ExternalInput").ap()
    wkh = nc.dram_tensor("wkh", [P, KT2, 2, OG], f8, kind="ExternalInput").ap()
    wkl = nc.dram_tensor("wkl", [P, KT2, 2, OG], f8, kind="ExternalInput").ap()
    wvh = nc.dram_tensor("wvh", [P, KT2, 2, OG], f8, kind="ExternalInput").ap()
    wvl = nc.dram_tensor("wvl", [P, KT2, 2, OG], f8, kind="ExternalInput").ap()
    woT = nc.dram_tensor("woT", [OG, D], bf16, kind="ExternalInput").ap()
    bqd = nc.dram_tensor("bqd", [P, 2], f32, kind="ExternalInput").ap()
    bkd = nc.dram_tensor("bkd", [P, 2], f32, kind="ExternalInput").ap()
    idd = nc.dram_tensor("idd", [P, P], bf16, kind="ExternalInput").ap()
    nbias = max(1, len(bias_slots))
    biasT = nc.dram_tensor("biasT", [nbias, P, P], bf16,
                           kind="ExternalInput").ap()
    out = nc.dram_tensor("out", [S, D], bf16, kind="ExternalOutput").ap()

    with tile.TileContext(nc) as tc:
        with tc.tile_pool(name="persist", bufs=1) as persist, \
             tc.tile_pool(name="const", bufs=1) as const:
            # Persistent SBUF tensors. Q/K layout: 4 slots; head h lives at
            # partition base HBASE[h], slot pair HSP[h]..HSP[h]+2 (matmul
            # operand bases must be in {0,32,64}, so head 3 wraps to base 0
            # on the second slot pair).
            HBASE = [0, 32, 64, 0]
            HSP = [0, 0, 0, 2]
            qT8 = persist.tile([P, 4, S], f8)
            kT8 = persist.tile([P, 4, S], f8)
            # V tiles padded to 128 output dims per head: cols 0:64 = v,
            # col 64 = 32.0 (denominator ones-row), cols 65:128 = 0 — the
            # dual-fp8 ldweights ISA check requires M=128 (M=65 is illegal)
            vaug_h = persist.tile([P, NT, HPG, P], f8)
            vaug_l = persist.tile([P, NT, HPG, P], f8)
            zt01 = persist.tile([P, S], bf16)    # heads 0,1 Z.T scaled
            zt23 = persist.tile([P, S], bf16)
            woT_sb = persist.tile([P, 2, D], bf16)
            bias_sb = persist.tile([P, nbias, P], bf16)

            bqs = const.tile([P, 2], f32)
            bks = const.tile([P, 2], f32)
            ident = const.tile([P, P], bf16)
            # ones-row: 32.0 in the hi V (absorbed by the reciprocal), 0 in lo
            nc.gpsimd.memset(vaug_h[:, :, :, HD:P], 0.0)
            nc.gpsimd.memset(vaug_h[:, :, :, HD:HD + 1], WSCALE)
            nc.vector.memset(vaug_l[:, :, :, HD:P], 0.0)

            # ---- Flat pools ----
            osb = tc.alloc_tile_pool(name="osb", bufs=3)
            xTp = tc.alloc_tile_pool(name="xT", bufs=4)
            wsb = tc.alloc_tile_pool(name="wsb", bufs=1)
            psum = tc.alloc_tile_pool(name="psum", bufs=1, space="PSUM")
            ptp = tc.alloc_tile_pool(name="pt", bufs=4)
            small = tc.alloc_tile_pool(name="small", bufs=2)

            wqh_sb = wsb.tile([P, KT2, 2, OG], f8, tag="wqh")
            wql_sb = wsb.tile([P, KT2, 2, OG], f8, tag="wql")
            wkh_sb = wsb.tile([P, KT2, 2, OG], f8, tag="wkh")
            wkl_sb = wsb.tile([P, KT2, 2, OG], f8, tag="wkl")
            wvh_sb = wsb.tile([P, KT2, 2, OG], f8, tag="wvh")
            wvl_sb = wsb.tile([P, KT2, 2, OG], f8, tag="wvl")
            # K weights first (first projections), split for early start;
            # tiny const loads are deferred behind the first matmul's deps
            nc.sync.dma_start(wkh_sb[:, 0:2], wkh[:, 0:2])

            # PSUM: "st" tag [P,2,CH] f32 (2 banks) x2 bufs for attention
            # scores; "rot" tag [P,CH] f32 (1 bank) x4 bufs rotating over
            # proj psums, PV accumulators, and out-proj psums = 8 banks.
            def st_tile(name):
                return psum.tile([P, 2, CH], f32, tag="st", bufs=2, name=name)

            def rot(name):
                return psum.tile([P, CH], f32, tag="rot", bufs=4, name=name)

            srcs = {0: (xkh, xkl, wkh_sb, wkl_sb),
                    1: (xvh, xvl, wvh_sb, wvl_sb),
                    2: (xqh, xql, wqh_sb, wql_sb)}

            def prep_proj(which, c, step):
                """Issue this chunk-projection's DMAs now; return per-psum
                emission closures to be interleaved into the PE stream."""
                # deferred constant loads, spread through the stream
                if step == 1:
                    nc.sync.dma_start(wql_sb, wql)
                    nc.sync.dma_start(wvh_sb, wvh)
                    nc.sync.dma_start(wvl_sb, wvl)
                elif step == 2:
                    nc.sync.dma_start(bias_sb,
                                      biasT.rearrange("n p q -> p n q"))
                elif step == 3:
                    nc.sync.dma_start(
                        woT_sb, woT.rearrange("(t p) d -> p t d", p=P))
                xh_dr, xl_dr, w_h, w_l = srcs[which]
                xh_t = xTp.tile([P, KT2, 2, CH], f8, tag="xT", bufs=6, name="xh")
                xl_t = xTp.tile([P, KT2, 2, CH], f8, tag="xT", bufs=6, name="xl")
                csl = slice(c * CH, (c + 1) * CH)
                if step == 0:
                    # fine-grained, first-needed-first; the x halves go out
                    # on the idle Act queue in parallel with the SP stream
                    nc.scalar.dma_start(xh_t[:, 0:2], xh_dr[:, 0:2, :, csl])
                    nc.sync.dma_start(wkh_sb[:, 2:4], wkh[:, 2:4])
                    nc.scalar.dma_start(xh_t[:, 2:4], xh_dr[:, 2:4, :, csl])
                    nc.sync.dma_start(wkl_sb, wkl)
                    nc.sync.dma_start(bks, bkd)
                    nc.sync.dma_start(wqh_sb, wqh)
                    nc.sync.dma_start(bqs, bqd)
                    nc.sync.dma_start(ident, idd)
                else:
                    nc.sync.dma_start(xh_t, xh_dr[:, :, :, csl])
                nc.sync.dma_start(xl_t, xl_dr[:, :, :, csl])
                units = []
                if which != 1:
                    dst8 = kT8 if which == 0 else qT8
                    bias_ap = bks if which == 0 else bqs

                    def qk_unit(ot):
                        def run():
                            ps = rot("psqk")
                            osl = slice(ot * P, (ot + 1) * P)
                            for t in range(KT2):
                                nc.tensor.matmul(ps, w_h[:, t, :, osl],
                                                 xh_t[:, t], start=(t == 0),
                                                 stop=False, perf_mode=DR)
                            for t in range(KT2):
                                nc.tensor.matmul(ps, w_l[:, t, :, osl],
                                                 xh_t[:, t], start=False,
                                                 stop=False, perf_mode=DR)
                            for t in range(KT2):
                                nc.tensor.matmul(ps, w_h[:, t, :, osl],
                                                 xl_t[:, t], start=False,
                                                 stop=(t == KT2 - 1),
                                                 perf_mode=DR)
                            nc.vector.tensor_scalar(
                                dst8[0:96, ot, csl], ps[0:96, :],
                                1.0 / WSCALE, bias_ap[0:96, ot:ot + 1],
                                op0=AL.mult, op1=AL.add)
                            nc.vector.tensor_scalar(
                                dst8[0:32, 2 + ot, csl], ps[96:128, :],
                                1.0 / WSCALE, bias_ap[96:128, ot:ot + 1],
                                op0=AL.mult, op1=AL.add)
                        return run
                    units = [qk_unit(0), qk_unit(1)]
                else:
                    def v_unit(st):
                        def run():
                            pv = rot("psv")[:, 0:OG]
                            ssl = slice(st * P, (st + 1) * P)
                            for t in range(KT2):
                                nc.tensor.matmul(pv, xh_t[:, t, :, ssl],
                                                 w_h[:, t], start=(t == 0),
                                                 stop=False, perf_mode=DR)
                            for t in range(KT2):
                                nc.tensor.matmul(pv, xh_t[:, t, :, ssl],
                                                 w_l[:, t], start=False,
                                                 stop=False, perf_mode=DR)
                            for t in range(KT2):
                                nc.tensor.matmul(pv, xl_t[:, t, :, ssl],
                                                 w_h[:, t], start=False,
                                                 stop=(t == KT2 - 1),
                                                 perf_mode=DR)
                            pv_re = pv.rearrange("p (h d) -> p h d", h=HPG)
                            vh_view = vaug_h[:, c * 4 + st, :, 0:HD]
                            nc.vector.tensor_copy(vh_view, pv_re)
                            nc.vector.tensor_tensor(
                                vaug_l[:, c * 4 + st, :, 0:HD], pv_re,
                                vh_view, op=AL.subtract)
                        return run
                    units = [v_unit(st) for st in range(CH // P)]
                return units

            # ---- Attention + out-proj, per sq-chunk ----
            # Out-proj for chunk c-1 is emitted mid-way through chunk c so
            # the (in-order) PE stream never stalls on the epilogue; the
            # epilogue itself is PE-free (DVE recip -> Pool partition
            # broadcast -> DVE multiply). Mask biases are added on the PE
            # (identity-matmul accumulate) to keep DVE off the exp path.
            ATTN_OFF = [0, 512, 1024, 1536]
            ATTN_W = [512, 512, 512, 512]
            NAC = len(ATTN_W)

            def oproj_unit(sg):
                def run():
                    ob = osb.tile([P, D], bf16, tag="ob", name="ob")
                    for nk in range(2):
                        ps = rot("psop")
                        for kk, zsrc in enumerate((zt01, zt23)):
                            nc.tensor.matmul(
                                ps, zsrc[:, sg * P:(sg + 1) * P],
                                woT_sb[:, kk, nk * CH:(nk + 1) * CH],
                                start=(kk == 0), stop=(kk == 1))
                        osl = slice(nk * CH, (nk + 1) * CH)
                        if nk == 0:
                            nc.scalar.copy(ob[:, osl], ps)
                        else:
                            nc.vector.tensor_copy(ob[:, osl], ps)
                    nc.sync.dma_start(out[sg * P:(sg + 1) * P, :], ob)
                return run

            def oproj_units(ci):
                i0 = ATTN_OFF[ci] // P
                return [oproj_unit(sg)
                        for sg in range(i0, i0 + ATTN_W[ci] // P)]

            def emit_attn(ci, fill=None, opro=None):
                fill = fill if fill is not None else []
                opro = opro if opro is not None else []
                coff, cw = ATTN_OFF[ci], ATTN_W[ci]
                i0 = coff // P
                tiles_i = list(range(i0, i0 + cw // P))
                jlist = []
                for j in range(NT):
                    ii = [i for i in tiles_i if process[i, j]]
                    if ii:
                        jlist.append((j, min(ii) - i0,
                                      max(ii) - i0 + 1))
                # pair adjacent j's for the 2-slot PV DoubleRow
                jpairs = []
                idx = 0
                while idx < len(jlist):
                    if (idx + 1 < len(jlist)
                            and jlist[idx + 1][0] == jlist[idx][0] + 1):
                        jpairs.append((jlist[idx], jlist[idx + 1]))
                        idx += 2
                    else:
                        jpairs.append((jlist[idx], None))
                        idx += 1
                for hp in range(2):  # head pairs (2*hp, 2*hp+1)
                    h0, h1 = 2 * hp, 2 * hp + 1
                    zta = {h: rot(f"zta{h}") for h in (h0, h1)}
                    npop = 0
                    pend = None  # deferred PV: next pair's S.T/exp goes
                    # into the PE stream BEFORE this PV so the (in-order)
                    # PE never head-of-line stalls on the exp it waits for

                    def emit_pv(args, last):
                        pa_, pb_, offu_, wu_, pt_, first_ = args
                        ja_ = pa_[0]
                        for hh, h in enumerate((h0, h1)):
                            zo = zta[h]
                            if pb_ is not None:
                                nc.tensor.matmul(
                                    zo[:, offu_:offu_ + wu_],
                                    vaug_h[:, ja_:ja_ + 2, h, :],
                                    pt_[:, :, hh, offu_:offu_ + wu_],
                                    start=first_, stop=False, perf_mode=DR)
                                nc.tensor.matmul(
                                    zo[:, offu_:offu_ + wu_],
                                    vaug_l[:, ja_:ja_ + 2, h, :],
                                    pt_[:, :, hh, offu_:offu_ + wu_],
                                    start=False, stop=last, perf_mode=DR)
                            else:
                                nc.tensor.matmul(
                                    zo[:, offu_:offu_ + wu_],
                                    vaug_h[:, ja_, h, :],
                                    pt_[:, 0, hh, offu_:offu_ + wu_],
                                    start=first_, stop=False)
                                nc.tensor.matmul(
                                    zo[:, offu_:offu_ + wu_],
                                    vaug_l[:, ja_, h, :],
                                    pt_[:, 0, hh, offu_:offu_ + wu_],
                                    start=False, stop=last)

                    for pi, (pa, pb) in enumerate(jpairs):
                        ja, loa, hia = pa
                        if pb is not None:
                            jb, lob, hib = pb
                            lo_u, hi_u = min(loa, lob), max(hia, hib)
                        else:
                            lo_u, hi_u = loa, hia
                        offu, wu = lo_u * P, (hi_u - lo_u) * P
                        pt = ptp.tile([P, 2, 2, CH], f8, tag="pt", bufs=6, name="pt")
                        for jj, ent in enumerate([pa] + ([pb] if pb else [])):
                            j_, lo_, hi_ = ent
                            off, w = lo_ * P, (hi_ - lo_) * P
                            st_ = st_tile("st_")
                            bis = [i for i in range(i0 + lo_, i0 + hi_)
                                   if biased[i, j_]]
                            for hh, h in enumerate((h0, h1)):
                                pb_, sp = HBASE[h], HSP[h]
                                nc.tensor.matmul(
                                    st_[:, hh, off:off + w],
                                    kT8[pb_:pb_ + 32, sp:sp + 2,
                                        j_ * P:(j_ + 1) * P],
                                    qT8[pb_:pb_ + 32, sp:sp + 2,
                                        coff + off:coff + off + w],
                                    start=True, stop=(not bis),
                                    perf_mode=DR)
                            # mask biases via PE identity-matmul accumulate
                            for bn, i in enumerate(bis):
                                sl = bias_slots[(i, j_)]
                                so = (i - i0) * P
                                lastb = bn == len(bis) - 1
                                for hh in range(2):
                                    nc.tensor.matmul(
                                        st_[:, hh, so:so + P], ident,
                                        bias_sb[:, sl, :], start=False,
                                        stop=(lastb and hh == 1),
                                        skip_group_check=True)
                            # zero pt where this j's band is narrower than
                            # the pair's union (PV reads the union)
                            if off > offu:
                                nc.gpsimd.memset(
                                    pt[:, jj, :, offu:off], 0.0)
                            if off + w < offu + wu:
                                nc.gpsimd.memset(
                                    pt[:, jj, :, off + w:offu + wu], 0.0)
                            nc.scalar.activation(
                                pt[:, jj, :, off:off + w],
                                st_[:, :, off:off + w],
                                mybir.ActivationFunctionType.Exp,
                                scale=EXPS)
                        if pend is not None:
                            emit_pv(pend, last=False)
                            # interleave a projection unit into the exp-wait
                            # gap; cap 2 per head pair so the rot rotation
                            # never lands on a live PV accumulator
                            if npop < 2 and fill:
                                fill.pop(0)()
                                npop += 1
                        pend = (pa, pb, offu, wu, pt, pi == 0)
                    emit_pv(pend, last=True)
                    # epilogue (PE-free): reciprocal of 32*denominators,
                    # Pool partition-broadcast, DVE scale into SBUF
                    recs = small.tile([1, 2, CH], f32, tag="recs", bufs=2,
                                      name="recs")
                    bcs0 = small.tile([HD, CH], f32, tag="bcs0", bufs=2,
                                      name="bcs0")
                    bcs1 = small.tile([HD, CH], f32, tag="bcs1", bufs=2,
                                      name="bcs1")
                    with nc.allow_low_precision(reason="fp22 recip"):
                        nc.vector.reciprocal(recs[0:1, 0, 0:cw],
                                             zta[h0][HD:HD + 1, 0:cw])
                        nc.vector.reciprocal(recs[0:1, 1, 0:cw],
                                             zta[h1][HD:HD + 1, 0:cw])
                    # (partition_broadcast only writes at base partition 0)
                    nc.gpsimd.partition_broadcast(bcs0[:, 0:cw],
                                                  recs[0:1, 0, 0:cw],
                                                  channels=HD)
                    nc.gpsimd.partition_broadcast(bcs1[:, 0:cw],
                                                  recs[0:1, 1, 0:cw],
                                                  channels=HD)
                    zdst = zt01 if hp == 0 else zt23
                    for hh, h in enumerate((h0, h1)):
                        zpo = hh * HD
                        nc.vector.tensor_mul(
                            zdst[zpo:zpo + HD, coff:coff + cw],
                            zta[h][0:HD, 0:cw],
                            (bcs0 if hh == 0 else bcs1)[:, 0:cw])
                    # post-epilogue: both PV accumulators are drained, so
                    # any number of rot-allocating units is safe here
                    npost = 2 if hp == 0 else len(fill) + len(opro)
                    for _ in range(min(2, len(fill)) if hp == 0 else
                                   len(fill)):
                        fill.pop(0)()
                    for _ in range(min(2, len(opro)) if hp == 0 else
                                   len(opro)):
                        opro.pop(0)()

            # Interleaved schedule: causal attention chunk c needs only
            # K/V chunks 0..c and Q chunk c. Chunk c+1's projections are
            # emitted BETWEEN chunk c's head pairs so the PE keeps working
            # while the Act engine (exp, the critical resource) chews on
            # chunk c, and Act never starves at chunk boundaries.
            for u in (prep_proj(0, 0, 0) + prep_proj(2, 0, 1)
                      + prep_proj(1, 0, 2)):
                u()
            # out-proj units are deferred one extra chunk so they land in
            # the Act-bound (exp-heavy) late windows where the PE starves
            opro_sched = {2: oproj_units(0), 3: oproj_units(1)}
            for ci in range(NAC):
                fill = []
                if ci + 1 < NCH:
                    fill += prep_proj(0, ci + 1, 3 + ci)
                    fill += prep_proj(2, ci + 1, 99)
                    fill += prep_proj(1, ci + 1, 99)
                if ci == NAC - 1:
                    fill += oproj_units(NAC - 2)
                emit_attn(ci, fill=fill, opro=opro_sched.get(ci, []))
            for u in oproj_units(NAC - 1):
                u()
            for pool_ in (small, ptp, psum, wsb, xTp, osb):
                pool_.release()
    nc.compile()
    # Belt-and-braces: any write-only preamble registers that survive DCE
    # but never get ids from alloc_regs would fail walrus birverifier
    # (reg_id == -1). They are write-only, so engine-unique ids are safe;
    # keep _lo/_hi pairs adjacent and even-aligned.
    from collections import defaultdict
    from concourse import mybir
    ctr = defaultdict(int)
    for f_ in nc.m.functions:
        for a in f_.allocations:
            if isinstance(a, mybir.Register) and a.reg_id >= 0:
                ctr[a.engine] = max(ctr[a.engine], a.reg_id + 1)
    for f_ in nc.m.functions:
        for a in f_.allocations:
            if isinstance(a, mybir.Register) and a.reg_id == -1:
                if a.name.endswith("_lo") and ctr[a.engine] % 2:
                    ctr[a.engine] += 1
                a.reg_id = ctr[a.engine]
                ctr[a.engine] += 1
    return nc


def _interleave_kdim(arr):
    """[1024 in-dim, N] -> [128 p, 4 t, 2 slot, N] with d = t*256+slot*128+p."""
    n = arr.shape[1]
    return np.ascontiguousarray(
        arr.reshape(KT2, 2, P, n).transpose(2, 0, 1, 3))


def _split8(arr):
    import ml_dtypes
    e4 = ml_dtypes.float8_e4m3
    hi = arr.astype(e4)
    lo = (arr - hi.astype(np.float32)).astype(e4)
    return np.ascontiguousarray(hi), np.ascontiguousarray(lo)


def kernel(query, key, value, mask, key_padding_mask,
           Wq, bq, Wk, bk, Wv, bv, Wo, bo, _return_perf=False):
    import ml_dtypes
    from concourse import bass_utils

    query = np.asarray(query, np.float32)
    key_ = np.asarray(key, np.float32)
    value = np.asarray(value, np.float32)
    Wq, Wk, Wv, Wo = (np.asarray(w, np.float32) for w in (Wq, Wk, Wv, Wo))
    bq, bk, bv, bo = (np.asarray(b_, np.float32) for b_ in (bq, bk, bv, bo))

    process, biased, bias_data = _block_structure(mask, key_padding_mask)
    bias_slots = {}
    for i in range(NT):
        for j in range(NT):
            if process[i, j] and biased[i, j]:
                bias_slots[(i, j)] = len(bias_slots)

    key_struct = (process.tobytes(), biased.tobytes())
    if key_struct not in _cache:
        _cache[key_struct] = _build_bass(process, biased, bias_slots)
    nc = _cache[key_struct]

    nbias = max(1, len(bias_slots))
    # x splits: shared across the 4 cores of each batch
    xsp = {}
    for b in range(B):
        for nm, x in (("q", query[b]), ("k", key_[b]), ("v", value[b])):
            xsp[(nm, b)] = _split8(_interleave_kdim(
                np.ascontiguousarray(x.T)))

    # Q/K output-dim permutation: psum partition p = head*32 + dim%32,
    # slot ot = dim//32  (head/dim within this core's 4-head group)
    perm = np.zeros((2, P), np.int64)
    for ot in range(2):
        for p_ in range(P):
            perm[ot, p_] = (p_ // 32) * HD + ot * 32 + (p_ % 32)

    in_maps = []
    for core in range(8):
        b, g = core // G, core % G
        gsl = np.arange(g * OG, (g + 1) * OG)
        qk_rows = gsl.reshape(1, OG)[0][perm.reshape(-1)]  # [256] perm'd
        wq_s = _split8(_interleave_kdim(WSCALE * Wq[qk_rows, :].T))
        wk_s = _split8(_interleave_kdim(WSCALE * Wk[qk_rows, :].T))
        wv_s = _split8(_interleave_kdim(WSCALE * Wv[gsl, :].T))
        bt = np.zeros((nbias, P, P), np.float32)
        for (i, j), slot in bias_slots.items():
            bt[slot] = bias_data[b][(i, j)]
        bt = bt.astype(ml_dtypes.bfloat16)
        in_maps.append({
            "xqh": xsp[("q", b)][0], "xql": xsp[("q", b)][1],
            "xkh": xsp[("k", b)][0], "xkl": xsp[("k", b)][1],
            "xvh": xsp[("v", b)][0], "xvl": xsp[("v", b)][1],
            "wqh": wq_s[0], "wql": wq_s[1],
            "wkh": wk_s[0], "wkl": wk_s[1],
            "wvh": wv_s[0], "wvl": wv_s[1],
            "woT": np.ascontiguousarray(Wo[:, gsl].T.astype(ml_dtypes.bfloat16)),
            "bqd": np.ascontiguousarray(bq[qk_rows].reshape(2, P).T),
            "bkd": np.ascontiguousarray(bk[qk_rows].reshape(2, P).T),
            "idd": np.eye(P, dtype=ml_dtypes.bfloat16),
            "biasT": bt,
        })

    trace = bool(int(os.environ.get("KERNEL_TRACE", "0")))
    res = bass_utils.run_bass_kernel_spmd(
        nc, in_maps, core_ids=list(range(8)), trace=trace)

    out = np.zeros((B, S, D), np.float32)
    for core in range(8):
        out[core // G] += res.results[core]["out"].astype(np.float32)
    out += (bo + bv @ Wo.T)[None, None, :]
    if _return_perf:
        return out, res
    return out


# revision 7
# speedup vs baseline: 1.0839x; 1.0001x over previous
"""Trainium2 Bass kernel for MultiHeadAttention (B=2, S=2048, D=1024, H=16).

Sharding: 8 cores = 2 (batch) x 4 (head groups of 4 heads / 256 proj cols).
Each core computes attention for its batch + head group and a partial
output projection [S, D]; host sums the 4 partials per batch and adds
bo' = bo + bv @ Wo.T (the V bias is folded into the host-side constant).

v2 pipeline (fp8e4m3 DoubleRow matmuls wherever the cost permits):
  1. Projections: 3-pass error-compensated fp8 DoubleRow
     (x_hi@W_hi + x_lo@W_hi + x_hi@W_lo), weights pre-scaled by 32 on the
     host so the fp8 residuals stay inside e4m3's dynamic range. Q/K are
     descaled (x1/32) + biased on DVE and written straight to fp8 SBUF in
     a permuted (head, halfdim) layout: psum partition p = head*32 +
     (dim%32), slot dim = dim//32. V keeps the 32x scale (the softmax
     ones-row is 32 so the reciprocal absorbs it) and is split hi/lo on
     device for an error-compensated 2-pass PV.
  2. QK^T: fp8 DoubleRow per head over 32 partitions at base head*32
     (contraction 64 = 32x2 slots), output S.T psum [sk, 2 heads, sq].
     Additive -3e4 mask bias on partial blocks as before.
  3. exp (Act, scale=1/8) -> fp8 P.T tiles with a j-pair slot dim; PV is
     a 2-pass (V hi/lo) fp8 DoubleRow over j-tile pairs. Row 64 of the
     PV psum is 32*denominator.
  4. Per head pair: DVE reciprocal of both denominators -> [2, CH], one
     PE broadcast matmul with a 0/1 selector -> [128, CH], DVE multiply
     (psum x psum) -> scaled Z.T in SBUF (f32r).
  5. Out-proj per s-tile: f32r matmuls, psum -> bf16 SBUF -> DMA out.

PSUM: "st" tag [128,2,CH] f32 (2 banks) x2 bufs + "rot" tag [128,CH]
f32 (1 bank) x4 bufs rotating over proj psums, PV accumulators, the
broadcast, and out-proj psums = 8 banks exactly.
"""

import math
import os
import sys

import numpy as np

sys.path.insert(0, "/opt/trn_rl_repo")
sys.path.insert(0, "/opt/trn_rl_repo/concourse")

B, S, D, H = 2, 2048, 1024, 16
HD = D // H  # 64
G = 4  # head groups (cores per batch)
OG = D // G  # 256 proj cols per core
HPG = H // G  # 4 heads per core
P = 128
NT = S // P  # 16 s-tiles
CH = 512  # sq chunk width
NCH = S // CH  # 4 chunks
KT2 = 4  # fp8 DoubleRow contraction steps (256 dims each)
WSCALE = 32.0  # host pre-scale on all projection weights
NEG = -30000.0  # additive mask bias (pre-scale)

_cache = {}


def _block_structure(mask, key_padding_mask):
    """Classify each 128x128 block of the [S,S] score matrix per batch.

    Returns (process, biased, bias_data) where
      process[i,j]  : bool  -- any batch needs block (sq-tile i, sk-tile j)
      biased[i,j]   : bool  -- some processed batch needs a bias on (i,j)
      bias_data[b]  : {(i,j): [128,128] f32 bias (TRANSPOSED: [sk,sq])}
    """
    mask = np.asarray(mask)
    kpm = np.asarray(key_padding_mask)
    full = np.zeros((B, NT, NT), dtype=bool)
    partial = np.zeros((B, NT, NT), dtype=bool)
    blocks = {}
    for b in range(B):
        for i in range(NT):
            mrow = mask[i * P:(i + 1) * P]
            for j in range(NT):
                mb = mrow[:, j * P:(j + 1) * P] | kpm[b, None, j * P:(j + 1) * P]
                if mb.all():
                    full[b, i, j] = True
                elif mb.any():
                    partial[b, i, j] = True
                    blocks[(b, i, j)] = mb
                else:
                    blocks[(b, i, j)] = None
    process = (~full).any(axis=0)
    biased = process & (full | partial).any(axis=0)
    bias_data = []
    for b in range(B):
        d = {}
        for i in range(NT):
            for j in range(NT):
                if not (process[i, j] and biased[i, j]):
                    continue
                if full[b, i, j]:
                    d[(i, j)] = np.full((P, P), NEG, np.float32)
                elif partial[b, i, j]:
                    d[(i, j)] = (blocks[(b, i, j)].T * NEG).astype(np.float32)
                else:
                    d[(i, j)] = np.zeros((P, P), np.float32)
        bias_data.append(d)
    return process, biased, bias_data


def _build_bass(process, biased, bias_slots):
    """Trace the Tile kernel. bias_slots: {(i,j): slot} for biased blocks."""
    import concourse.bass as bass
    import concourse.tile as tile
    from concourse import bacc, mybir

    f32 = mybir.dt.float32
    f32r = mybir.dt.float32r
    f8 = mybir.dt.float8e4
    bf16 = mybir.dt.bfloat16
    DR = mybir.MatmulPerfMode.DoubleRow
    AL = mybir.AluOpType
    EXPS = 1.0 / math.sqrt(HD)
    nc = bacc.Bacc("TRN2", target_bir_lowering=False, debug=False,
                   enable_asserts=False)

    # Host supplies fp8 hi/lo splits, contraction-interleaved:
    # x*: [p, t, slot, s] with input dim d = t*256 + slot*128 + p
    # w*: [p, t, slot, o] same d mapping; o permuted for Q/K (head*32+halfdim)
    xqh = nc.dram_tensor("xqh", [P, KT2, 2, S], f8, kind="ExternalInput").ap()
    xql = nc.dram_tensor("xql", [P, KT2, 2, S], f8, kind="ExternalInput").ap()
    xkh = nc.dram_tensor("xkh", [P, KT2, 2, S], f8, kind="ExternalInput").ap()
    xkl = nc.dram_tensor("xkl", [P, KT2, 2, S], f8, kind="ExternalInput").ap()
    xvh = nc.dram_tensor("xvh", [P, KT2, 2, S], f8, kind="ExternalInput").ap()
    xvl = nc.dram_tensor("xvl", [P, KT2, 2, S], f8, kind="ExternalInput").ap()
    wqh = nc.dram_tensor("wqh", [P, KT2, 2, OG], f8, kind="ExternalInput").ap()
    wql = nc.dram_tensor("wql", [P, KT2, 2, OG], f8, kind="ExternalInput").ap()
    wkh = nc.dram_tensor("wkh", [P, KT2, 2, OG], f8, kind="ExternalInput").ap()
    wkl = nc.dram_tensor("wkl", [P, KT2, 2, OG], f8, kind="ExternalInput").ap()
    wvh = nc.dram_tensor("wvh", [P, KT2, 2, OG], f8, kind="ExternalInput").ap()
    wvl = nc.dram_tensor("wvl", [P, KT2, 2, OG], f8, kind="ExternalInput").ap()
    woT = nc.dram_tensor("woT", [OG, D], bf16, kind="ExternalInput").ap()
    bqd = nc.dram_tensor("bqd", [P, 2], f32, kind="ExternalInput").ap()
    bkd = nc.dram_tensor("bkd", [P, 2], f32, kind="ExternalInput").ap()
    idd = nc.dram_tensor("idd", [P, P], bf16, kind="ExternalInput").ap()
    nbias = max(1, len(bias_slots))
    biasT = nc.dram_tensor("biasT", [nbias, P, P], bf16,
                           kind="ExternalInput").ap()
    out = nc.dram_tensor("out", [S, D], bf16, kind="ExternalOutput").ap()

    with tile.TileContext(nc) as tc:
        with tc.tile_pool(name="persist", bufs=1) as persist, \
             tc.tile_pool(name="const", bufs=1) as const:
            # Persistent SBUF tensors. Q/K layout: 4 slots; head h lives at
            # partition base HBASE[h], slot pair HSP[h]..HSP[h]+2 (matmul
            # operand bases must be in {0,32,64}, so head 3 wraps to base 0
            # on the second slot pair).
            HBASE = [0, 32, 64, 0]
            HSP = [0, 0, 0, 2]
            qT8 = persist.tile([P, 4, S], f8)
            kT8 = persist.tile([P, 4, S], f8)
            # V tiles padded to 128 output dims per head: cols 0:64 = v,
            # col 64 = 32.0 (denominator ones-row), cols 65:128 = 0 — the
            # dual-fp8 ldweights ISA check requires M=128 (M=65 is illegal)
            vaug_h = persist.tile([P, NT, HPG, P], f8)
            vaug_l = persist.tile([P, NT, HPG, P], f8)
            zt01 = persist.tile([P, S], bf16)    # heads 0,1 Z.T scaled
            zt23 = persist.tile([P, S], bf16)
            woT_sb = persist.tile([P, 2, D], bf16)
            bias_sb = persist.tile([P, nbias, P], bf16)

            bqs = const.tile([P, 2], f32)
            bks = const.tile([P, 2], f32)
            ident = const.tile([P, P], bf16)
            # ones-row: 32.0 in the hi V (absorbed by the reciprocal), 0 in lo
            nc.gpsimd.memset(vaug_h[:, :, :, HD:P], 0.0)
            nc.gpsimd.memset(vaug_h[:, :, :, HD:HD + 1], WSCALE)
            nc.vector.memset(vaug_l[:, :, :, HD:P], 0.0)

            # ---- Flat pools ----
            osb = tc.alloc_tile_pool(name="osb", bufs=6)
            xTp = tc.alloc_tile_pool(name="xT", bufs=4)
            wsb = tc.alloc_tile_pool(name="wsb", bufs=1)
            psum = tc.alloc_tile_pool(name="psum", bufs=1, space="PSUM")
            ptp = tc.alloc_tile_pool(name="pt", bufs=4)
            small = tc.alloc_tile_pool(name="small", bufs=2)

            wqh_sb = wsb.tile([P, KT2, 2, OG], f8, tag="wqh")
            wql_sb = wsb.tile([P, KT2, 2, OG], f8, tag="wql")
            wkh_sb = wsb.tile([P, KT2, 2, OG], f8, tag="wkh")
            wkl_sb = wsb.tile([P, KT2, 2, OG], f8, tag="wkl")
            wvh_sb = wsb.tile([P, KT2, 2, OG], f8, tag="wvh")
            wvl_sb = wsb.tile([P, KT2, 2, OG], f8, tag="wvl")
            # K weights first (first projections), split for early start;
            # tiny const loads are deferred behind the first matmul's deps
            nc.sync.dma_start(wkh_sb[:, 0:2], wkh[:, 0:2])

            # PSUM: "st" tag [P,2,CH] f32 (2 banks) x2 bufs for attention
            # scores; "rot" tag [P,CH] f32 (1 bank) x4 bufs rotating over
            # proj psums, PV accumulators, and out-proj psums = 8 banks.
            def st_tile(name):
                return psum.tile([P, 2, CH], f32, tag="st", bufs=2, name=name)

            def rot(name):
                return psum.tile([P, CH], f32, tag="rot", bufs=4, name=name)

            srcs = {0: (xkh, xkl, wkh_sb, wkl_sb),
                    1: (xvh, xvl, wvh_sb, wvl_sb),
                    2: (xqh, xql, wqh_sb, wql_sb)}

            def prep_proj(which, c, step):
                """Issue this chunk-projection's DMAs now; return per-psum
                emission closures to be interleaved into the PE stream."""
                # deferred constant loads, spread through the stream
                if step == 1:
                    nc.sync.dma_start(wql_sb, wql)
                    nc.sync.dma_start(wvh_sb, wvh)
                    nc.sync.dma_start(wvl_sb, wvl)
                elif step == 2:
                    nc.sync.dma_start(bias_sb,
                                      biasT.rearrange("n p q -> p n q"))
                elif step == 3:
                    nc.sync.dma_start(
                        woT_sb, woT.rearrange("(t p) d -> p t d", p=P))
                xh_dr, xl_dr, w_h, w_l = srcs[which]
                xh_t = xTp.tile([P, KT2, 2, CH], f8, tag="xT", bufs=8, name="xh")
                xl_t = xTp.tile([P, KT2, 2, CH], f8, tag="xT", bufs=8, name="xl")
                csl = slice(c * CH, (c + 1) * CH)
                if step == 0:
                    # fine-grained, first-needed-first; the x halves go out
                    # on the idle Act queue in parallel with the SP stream
                    nc.scalar.dma_start(xh_t[:, 0:2], xh_dr[:, 0:2, :, csl])
                    nc.sync.dma_start(wkh_sb[:, 2:4], wkh[:, 2:4])
                    nc.scalar.dma_start(xh_t[:, 2:4], xh_dr[:, 2:4, :, csl])
                    nc.sync.dma_start(wkl_sb, wkl)
                    nc.sync.dma_start(bks, bkd)
                    nc.sync.dma_start(wqh_sb, wqh)
                    nc.sync.dma_start(bqs, bqd)
                    nc.sync.dma_start(ident, idd)
                else:
                    nc.sync.dma_start(xh_t, xh_dr[:, :, :, csl])
                nc.sync.dma_start(xl_t, xl_dr[:, :, :, csl])
                units = []
                if which != 1:
                    dst8 = kT8 if which == 0 else qT8
                    bias_ap = bks if which == 0 else bqs

                    def qk_unit(ot):
                        def run():
                            ps = rot("psqk")
                            osl = slice(ot * P, (ot + 1) * P)
                            for t in range(KT2):
                                nc.tensor.matmul(ps, w_h[:, t, :, osl],
                                                 xh_t[:, t], start=(t == 0),
                                                 stop=False, perf_mode=DR)
                            for t in range(KT2):
                                nc.tensor.matmul(ps, w_l[:, t, :, osl],
                                                 xh_t[:, t], start=False,
                                                 stop=False, perf_mode=DR)
                            for t in range(KT2):
                                nc.tensor.matmul(ps, w_h[:, t, :, osl],
                                                 xl_t[:, t], start=False,
                                                 stop=(t == KT2 - 1),
                                                 perf_mode=DR)
                            nc.vector.tensor_scalar(
                                dst8[0:96, ot, csl], ps[0:96, :],
                                1.0 / WSCALE, bias_ap[0:96, ot:ot + 1],
                                op0=AL.mult, op1=AL.add)
                            nc.vector.tensor_scalar(
                                dst8[0:32, 2 + ot, csl], ps[96:128, :],
                                1.0 / WSCALE, bias_ap[96:128, ot:ot + 1],
                                op0=AL.mult, op1=AL.add)
                        return run
                    units = [qk_unit(0), qk_unit(1)]
                else:
                    def v_unit(st):
                        def run():
                            pv = rot("psv")[:, 0:OG]
                            ssl = slice(st * P, (st + 1) * P)
                            for t in range(KT2):
                                nc.tensor.matmul(pv, xh_t[:, t, :, ssl],
                                                 w_h[:, t], start=(t == 0),
                                                 stop=False, perf_mode=DR)
                            for t in range(KT2):
                                nc.tensor.matmul(pv, xh_t[:, t, :, ssl],
                                                 w_l[:, t], start=False,
                                                 stop=False, perf_mode=DR)
                            for t in range(KT2):
                                nc.tensor.matmul(pv, xl_t[:, t, :, ssl],
                                                 w_h[:, t], start=False,
                                                 stop=(t == KT2 - 1),
                                                 perf_mode=DR)
                            pv_re = pv.rearrange("p (h d) -> p h d", h=HPG)
                            vh_view = vaug_h[:, c * 4 + st, :, 0:HD]
                            nc.vector.tensor_copy(vh_view, pv_re)
                            nc.vector.tensor_tensor(
                                vaug_l[:, c * 4 + st, :, 0:HD], pv_re,
                                vh_view, op=AL.subtract)
                        return run
                    units = [v_unit(st) for st in range(CH // P)]
                return units

            # ---- Attention + out-proj, per sq-chunk ----
            # Out-proj for chunk c-1 is emitted mid-way through chunk c so
            # the (in-order) PE stream never stalls on the epilogue; the
            # epilogue itself is PE-free (DVE recip -> Pool partition
            # broadcast -> DVE multiply). Mask biases are added on the PE
            # (identity-matmul accumulate) to keep DVE off the exp path.
            ATTN_OFF = [0, 512, 1024, 1536]
            ATTN_W = [512, 512, 512, 512]
            NAC = len(ATTN_W)

            def oproj_unit(sg):
                def run():
                    ob = osb.tile([P, D], bf16, tag="ob", name="ob")
                    for nk in range(2):
                        ps = rot("psop")
                        for kk, zsrc in enumerate((zt01, zt23)):
                            nc.tensor.matmul(
                                ps, zsrc[:, sg * P:(sg + 1) * P],
                                woT_sb[:, kk, nk * CH:(nk + 1) * CH],
                                start=(kk == 0), stop=(kk == 1))
                        osl = slice(nk * CH, (nk + 1) * CH)
                        if nk == 0:
                            nc.scalar.copy(ob[:, osl], ps)
                        else:
                            nc.vector.tensor_copy(ob[:, osl], ps)
                    nc.sync.dma_start(out[sg * P:(sg + 1) * P, :], ob)
                return run

            def oproj_units(ci):
                i0 = ATTN_OFF[ci] // P
                return [oproj_unit(sg)
                        for sg in range(i0, i0 + ATTN_W[ci] // P)]

            def emit_attn(ci, fill=None, opro=None):
                fill = fill if fill is not None else []
                opro = opro if opro is not None else []
                coff, cw = ATTN_OFF[ci], ATTN_W[ci]
                i0 = coff // P
                tiles_i = list(range(i0, i0 + cw // P))
                jlist = []
                for j in range(NT):
                    ii = [i for i in tiles_i if process[i, j]]
                    if ii:
                        jlist.append((j, min(ii) - i0,
                                      max(ii) - i0 + 1))
                # pair adjacent j's for the 2-slot PV DoubleRow
                jpairs = []
                idx = 0
                while idx < len(jlist):
                    if (idx + 1 < len(jlist)
                            and jlist[idx + 1][0] == jlist[idx][0] + 1):
                        jpairs.append((jlist[idx], jlist[idx + 1]))
                        idx += 2
                    else:
                        jpairs.append((jlist[idx], None))
                        idx += 1
                for hp in range(2):  # head pairs (2*hp, 2*hp+1)
                    h0, h1 = 2 * hp, 2 * hp + 1
                    zta = {h: rot(f"zta{h}") for h in (h0, h1)}
                    npop = 0
                    pend = None  # deferred PV: next pair's S.T/exp goes
                    # into the PE stream BEFORE this PV so the (in-order)
                    # PE never head-of-line stalls on the exp it waits for

                    def emit_pv(args, last):
                        pa_, pb_, offu_, wu_, pt_, first_ = args
                        ja_ = pa_[0]
                        for hh, h in enumerate((h0, h1)):
                            zo = zta[h]
                            if pb_ is not None:
                                nc.tensor.matmul(
                                    zo[:, offu_:offu_ + wu_],
                                    vaug_h[:, ja_:ja_ + 2, h, :],
                                    pt_[:, :, hh, offu_:offu_ + wu_],
                                    start=first_, stop=False, perf_mode=DR)
                                nc.tensor.matmul(
                                    zo[:, offu_:offu_ + wu_],
                                    vaug_l[:, ja_:ja_ + 2, h, :],
                                    pt_[:, :, hh, offu_:offu_ + wu_],
                                    start=False, stop=last, perf_mode=DR)
                            else:
                                nc.tensor.matmul(
                                    zo[:, offu_:offu_ + wu_],
                                    vaug_h[:, ja_, h, :],
                                    pt_[:, 0, hh, offu_:offu_ + wu_],
                                    start=first_, stop=False)
                                nc.tensor.matmul(
                                    zo[:, offu_:offu_ + wu_],
                                    vaug_l[:, ja_, h, :],
                                    pt_[:, 0, hh, offu_:offu_ + wu_],
                                    start=False, stop=last)

                    for pi, (pa, pb) in enumerate(jpairs):
                        ja, loa, hia = pa
                        if pb is not None:
                            jb, lob, hib = pb
                            lo_u, hi_u = min(loa, lob), max(hia, hib)
                        else:
                            lo_u, hi_u = loa, hia
                        offu, wu = lo_u * P, (hi_u - lo_u) * P
                        pt = ptp.tile([P, 2, 2, CH], f8, tag="pt", bufs=10, name="pt")
                        for jj, ent in enumerate([pa] + ([pb] if pb else [])):
                            j_, lo_, hi_ = ent
                            off, w = lo_ * P, (hi_ - lo_) * P
                            st_ = st_tile("st_")
                            bis = [i for i in range(i0 + lo_, i0 + hi_)
                                   if biased[i, j_]]
                            for hh, h in enumerate((h0, h1)):
                                pb_, sp = HBASE[h], HSP[h]
                                nc.tensor.matmul(
                                    st_[:, hh, off:off + w],
                                    kT8[pb_:pb_ + 32, sp:sp + 2,
                                        j_ * P:(j_ + 1) * P],
                                    qT8[pb_:pb_ + 32, sp:sp + 2,
                                        coff + off:coff + off + w],
                                    start=True, stop=(not bis),
                                    perf_mode=DR)
                            # mask biases via PE identity-matmul accumulate
                            for bn, i in enumerate(bis):
                                sl = bias_slots[(i, j_)]
                                so = (i - i0) * P
                                lastb = bn == len(bis) - 1
                                for hh in range(2):
                                    nc.tensor.matmul(
                                        st_[:, hh, so:so + P], ident,
                                        bias_sb[:, sl, :], start=False,
                                        stop=(lastb and hh == 1),
                                        skip_group_check=True)
                            # zero pt where this j's band is narrower than
                            # the pair's union (PV reads the union)
                            if off > offu:
                                nc.gpsimd.memset(
                                    pt[:, jj, :, offu:off], 0.0)
                            if off + w < offu + wu:
                                nc.gpsimd.memset(
                                    pt[:, jj, :, off + w:offu + wu], 0.0)
                            nc.scalar.activation(
                                pt[:, jj, :, off:off + w],
                                st_[:, :, off:off + w],
                                mybir.ActivationFunctionType.Exp,
                                scale=EXPS)
                        if pend is not None:
                            emit_pv(pend, last=False)
                            # interleave a projection unit into the exp-wait
                            # gap; cap 2 per head pair so the rot rotation
                            # never lands on a live PV accumulator
                            if npop < 2 and fill:
                                fill.pop(0)()
                                npop += 1
                        pend = (pa, pb, offu, wu, pt, pi == 0)
                    emit_pv(pend, last=True)
                    # epilogue (PE-free): reciprocal of 32*denominators,
                    # Pool partition-broadcast, DVE scale into SBUF
                    recs = small.tile([1, 2, CH], f32, tag="recs", bufs=4,
                                      name="recs")
                    bcs0 = small.tile([HD, CH], f32, tag="bcs0", bufs=4,
                                      name="bcs0")
                    bcs1 = small.tile([HD, CH], f32, tag="bcs1", bufs=4,
                                      name="bcs1")
                    with nc.allow_low_precision(reason="fp22 recip"):
                        nc.vector.reciprocal(recs[0:1, 0, 0:cw],
                                             zta[h0][HD:HD + 1, 0:cw])
                        nc.vector.reciprocal(recs[0:1, 1, 0:cw],
                                             zta[h1][HD:HD + 1, 0:cw])
                    # (partition_broadcast only writes at base partition 0)
                    nc.gpsimd.partition_broadcast(bcs0[:, 0:cw],
                                                  recs[0:1, 0, 0:cw],
                                                  channels=HD)
                    nc.gpsimd.partition_broadcast(bcs1[:, 0:cw],
                                                  recs[0:1, 1, 0:cw],
                                                  channels=HD)
                    zdst = zt01 if hp == 0 else zt23
                    for hh, h in enumerate((h0, h1)):
                        zpo = hh * HD
                        nc.vector.tensor_mul(
                            zdst[zpo:zpo + HD, coff:coff + cw],
                            zta[h][0:HD, 0:cw],
                            (bcs0 if hh == 0 else bcs1)[:, 0:cw])
                    # post-epilogue: both PV accumulators are drained, so
                    # any number of rot-allocating units is safe here
                    npost = 2 if hp == 0 else len(fill) + len(opro)
                    for _ in range(min(2, len(fill)) if hp == 0 else
                                   len(fill)):
                        fill.pop(0)()
                    for _ in range(min(2, len(opro)) if hp == 0 else
                                   len(opro)):
                        opro.pop(0)()

            # Interleaved schedule: causal attention chunk c needs only
            # K/V chunks 0..c and Q chunk c. Chunk c+1's projections are
            # emitted BETWEEN chunk c's head pairs so the PE keeps working
            # while the Act engine (exp, the critical resource) chews on
            # chunk c, and Act never starves at chunk boundaries.
            for u in (prep_proj(0, 0, 0) + prep_proj(2, 0, 1)
                      + prep_proj(1, 0, 2)):
                u()
            # out-proj units are deferred one extra chunk so they land in
            # the Act-bound (exp-heavy) late windows where the PE starves
            opro_sched = {2: oproj_units(0), 3: oproj_units(1)}
            for ci in range(NAC):
                fill = []
                if ci + 1 < NCH:
                    fill += prep_proj(0, ci + 1, 3 + ci)
                    fill += prep_proj(2, ci + 1, 99)
                    fill += prep_proj(1, ci + 1, 99)
                if ci == NAC - 1:
                    fill += oproj_units(NAC - 2)
                emit_attn(ci, fill=fill, opro=opro_sched.get(ci, []))
            for u in oproj_units(NAC - 1):
                u()
            for pool_ in (small, ptp, psum, wsb, xTp, osb):
                pool_.release()
    nc.compile()
    # Belt-and-braces: any write-only preamble registers that survive DCE
    # but never get ids from alloc_regs would fail walrus birverifier
    # (reg_id == -1). They are write-only, so engine-unique ids are safe;
    # keep _lo/_hi pairs adjacent and even-aligned.
    from collections import defaultdict
    from concourse import mybir
    ctr = defaultdict(int)
    for f_ in nc.m.functions:
        for a in f_.allocations:
            if isinstance(a, mybir.Register) and a.reg_id >= 0:
                ctr[a.engine] = max(ctr[a.engine], a.reg_id + 1)
    for f_ in nc.m.functions:
        for a in f_.allocations:
            if isinstance(a, mybir.Register) and a.reg_id == -1:
                if a.name.endswith("_lo") and ctr[a.engine] % 2:
                    ctr[a.engine] += 1
                a.reg_id = ctr[a.engine]
                ctr[a.engine] += 1
    return nc


def _interleave_kdim(arr):
    """[1024 in-dim, N] -> [128 p, 4 t, 2 slot, N] with d = t*256+slot*128+p."""
    n = arr.shape[1]
    return np.ascontiguousarray(
        arr.reshape(KT2, 2, P, n).transpose(2, 0, 1, 3))


def _split8(arr):
    import ml_dtypes
    e4 = ml_dtypes.float8_e4m3
    hi = arr.astype(e4)
    lo = (arr - hi.astype(np.float32)).astype(e4)
    return np.ascontiguousarray(hi), np.ascontiguousarray(lo)


def kernel(query, key, value, mask, key_padding_mask,
           Wq, bq, Wk, bk, Wv, bv, Wo, bo, _return_perf=False):
    import ml_dtypes
    from concourse import bass_utils

    query = np.asarray(query, np.float32)
    key_ = np.asarray(key, np.float32)
    value = np.asarray(value, np.float32)
    Wq, Wk, Wv, Wo = (np.asarray(w, np.float32) for w in (Wq, Wk, Wv, Wo))
    bq, bk, bv, bo = (np.asarray(b_, np.float32) for b_ in (bq, bk, bv, bo))

    process, biased, bias_data = _block_structure(mask, key_padding_mask)
    bias_slots = {}
    for i in range(NT):
        for j in range(NT):
            if process[i, j] and biased[i, j]:
                bias_slots[(i, j)] = len(bias_slots)

    key_struct = (process.tobytes(), biased.tobytes())
    if key_struct not in _cache:
        _cache[key_struct] = _build_bass(process, biased, bias_slots)
    nc = _cache[key_struct]

    nbias = max(1, len(bias_slots))
    # x splits: shared across the 4 cores of each batch
    xsp = {}
    for b in range(B):
        for nm, x in (("q", query[b]), ("k", key_[b]), ("v", value[b])):
            xsp[(nm, b)] = _split8(_interleave_kdim(
                np.ascontiguousarray(x.T)))

    # Q/K output-dim permutation: psum partition p = head*32 + dim%32,
    # slot ot = dim//32  (head/dim within this core's 4-head group)
    perm = np.zeros((2, P), np.int64)
    for ot in range(2):
        for p_ in range(P):
            perm[ot, p_] = (p_ // 32) * HD + ot * 32 + (p_ % 32)

    in_maps = []
    for core in range(8):
        b, g = core // G, core % G
        gsl = np.arange(g * OG, (g + 1) * OG)
        qk_rows = gsl.reshape(1, OG)[0][perm.reshape(-1)]  # [256] perm'd
        wq_s = _split8(_interleave_kdim(WSCALE * Wq[qk_rows, :].T))
        wk_s = _split8(_interleave_kdim(WSCALE * Wk[qk_rows, :].T))
        wv_s = _split8(_interleave_kdim(WSCALE * Wv[gsl, :].T))
        bt = np.zeros((nbias, P, P), np.float32)
        for (i, j), slot in bias_slots.items():
            bt[slot] = bias_data[b][(i, j)]
        bt = bt.astype(ml_dtypes.bfloat16)
        in_maps.append({
            "xqh": xsp[("q", b)][0], "xql": xsp[("q", b)][1],
            "xkh": xsp[("k", b)][0], "xkl": xsp[("k", b)][1],
            "xvh": xsp[("v", b)][0], "xvl": xsp[("v", b)][1],
            "wqh": wq_s[0], "wql": wq_s[1],
            "wkh": wk_s[0], "wkl": wk_s[1],
            "wvh": wv_s[0], "wvl": wv_s[1],
            "woT": np.ascontiguousarray(Wo[:, gsl].T.astype(ml_dtypes.bfloat16)),
            "bqd": np.ascontiguousarray(bq[qk_rows].reshape(2, P).T),
            "bkd": np.ascontiguousarray(bk[qk_rows].reshape(2, P).T),
            "idd": np.eye(P, dtype=ml_dtypes.bfloat16),
            "biasT": bt,
        })

    trace = bool(int(os.environ.get("KERNEL_TRACE", "0")))
    res = bass_utils.run_bass_kernel_spmd(
        nc, in_maps, core_ids=list(range(8)), trace=trace)

    out = np.zeros((B, S, D), np.float32)
    for core in range(8):
        out[core // G] += res.results[core]["out"].astype(np.float32)
    out += (bo + bv @ Wo.T)[None, None, :]
    if _return_perf:
        return out, res
    return out


# revision 9
# speedup vs baseline: 1.0868x; 1.0026x over previous
"""Trainium2 Bass kernel for MultiHeadAttention (B=2, S=2048, D=1024, H=16).

Sharding: 8 cores = 2 (batch) x 4 (head groups of 4 heads / 256 proj cols).
Each core computes attention for its batch + head group and a partial
output projection [S, D]; host sums the 4 partials per batch and adds
bo' = bo + bv @ Wo.T (the V bias is folded into the host-side constant).

v2 pipeline (fp8e4m3 DoubleRow matmuls wherever the cost permits):
  1. Projections: 3-pass error-compensated fp8 DoubleRow
     (x_hi@W_hi + x_lo@W_hi + x_hi@W_lo), weights pre-scaled by 32 on the
     host so the fp8 residuals stay inside e4m3's dynamic range. Q/K are
     descaled (x1/32) + biased on DVE and written straight to fp8 SBUF in
     a permuted (head, halfdim) layout: psum partition p = head*32 +
     (dim%32), slot dim = dim//32. V keeps the 32x scale (the softmax
     ones-row is 32 so the reciprocal absorbs it) and is split hi/lo on
     device for an error-compensated 2-pass PV.
  2. QK^T: fp8 DoubleRow per head over 32 partitions at base head*32
     (contraction 64 = 32x2 slots), output S.T psum [sk, 2 heads, sq].
     Additive -3e4 mask bias on partial blocks as before.
  3. exp (Act, scale=1/8) -> fp8 P.T tiles with a j-pair slot dim; PV is
     a 2-pass (V hi/lo) fp8 DoubleRow over j-tile pairs. Row 64 of the
     PV psum is 32*denominator.
  4. Per head pair: DVE reciprocal of both denominators -> [2, CH], one
     PE broadcast matmul with a 0/1 selector -> [128, CH], DVE multiply
     (psum x psum) -> scaled Z.T in SBUF (f32r).
  5. Out-proj per s-tile: f32r matmuls, psum -> bf16 SBUF -> DMA out.

PSUM: "st" tag [128,2,CH] f32 (2 banks) x2 bufs + "rot" tag [128,CH]
f32 (1 bank) x4 bufs rotating over proj psums, PV accumulators, the
broadcast, and out-proj psums = 8 banks exactly.
"""

import math
import os
import sys

import numpy as np

sys.path.insert(0, "/opt/trn_rl_repo")
sys.path.insert(0, "/opt/trn_rl_repo/concourse")

B, S, D, H = 2, 2048, 1024, 16
HD = D // H  # 64
G = 4  # head groups (cores per batch)
OG = D // G  # 256 proj cols per core
HPG = H // G  # 4 heads per core
P = 128
NT = S // P  # 16 s-tiles
CH = 512  # sq chunk width
NCH = S // CH  # 4 chunks
KT2 = 4  # fp8 DoubleRow contraction steps (256 dims each)
WSCALE = 32.0  # host pre-scale on all projection weights
NEG = -30000.0  # additive mask bias (pre-scale)

_cache = {}


def _block_structure(mask, key_padding_mask):
    """Classify each 128x128 block of the [S,S] score matrix per batch.

    Returns (process, biased, bias_data) where
      process[i,j]  : bool  -- any batch needs block (sq-tile i, sk-tile j)
      biased[i,j]   : bool  -- some processed batch needs a bias on (i,j)
      bias_data[b]  : {(i,j): [128,128] f32 bias (TRANSPOSED: [sk,sq])}
    """
    mask = np.asarray(mask)
    kpm = np.asarray(key_padding_mask)
    full = np.zeros((B, NT, NT), dtype=bool)
    partial = np.zeros((B, NT, NT), dtype=bool)
    blocks = {}
    for b in range(B):
        for i in range(NT):
            mrow = mask[i * P:(i + 1) * P]
            for j in range(NT):
                mb = mrow[:, j * P:(j + 1) * P] | kpm[b, None, j * P:(j + 1) * P]
                if mb.all():
                    full[b, i, j] = True
                elif mb.any():
                    partial[b, i, j] = True
                    blocks[(b, i, j)] = mb
                else:
                    blocks[(b, i, j)] = None
    process = (~full).any(axis=0)
    biased = process & (full | partial).any(axis=0)
    bias_data = []
    for b in range(B):
        d = {}
        for i in range(NT):
            for j in range(NT):
                if not (process[i, j] and biased[i, j]):
                    continue
                if full[b, i, j]:
                    d[(i, j)] = np.full((P, P), NEG, np.float32)
                elif partial[b, i, j]:
                    d[(i, j)] = (blocks[(b, i, j)].T * NEG).astype(np.float32)
                else:
                    d[(i, j)] = np.zeros((P, P), np.float32)
        bias_data.append(d)
    return process, biased, bias_data


def _build_bass(process, biased, bias_slots):
    """Trace the Tile kernel. bias_slots: {(i,j): slot} for biased blocks."""
    import concourse.bass as bass
    import concourse.tile as tile
    from concourse import bacc, mybir

    f32 = mybir.dt.float32
    f32r = mybir.dt.float32r
    f8 = mybir.dt.float8e4
    bf16 = mybir.dt.bfloat16
    DR = mybir.MatmulPerfMode.DoubleRow
    AL = mybir.AluOpType
    EXPS = 1.0 / math.sqrt(HD)
    nc = bacc.Bacc("TRN2", target_bir_lowering=False, debug=False,
                   enable_asserts=False)

    # Host supplies fp8 hi/lo splits, contraction-interleaved:
    # x*: [p, t, slot, s] with input dim d = t*256 + slot*128 + p
    # w*: [p, t, slot, o] same d mapping; o permuted for Q/K (head*32+halfdim)
    xqh = nc.dram_tensor("xqh", [P, KT2, 2, S], f8, kind="ExternalInput").ap()
    xql = nc.dram_tensor("xql", [P, KT2, 2, S], f8, kind="ExternalInput").ap()
    xkh = nc.dram_tensor("xkh", [P, KT2, 2, S], f8, kind="ExternalInput").ap()
    xkl = nc.dram_tensor("xkl", [P, KT2, 2, S], f8, kind="ExternalInput").ap()
    xvh = nc.dram_tensor("xvh", [P, KT2, 2, S], f8, kind="ExternalInput").ap()
    xvl = nc.dram_tensor("xvl", [P, KT2, 2, S], f8, kind="ExternalInput").ap()
    wqh = nc.dram_tensor("wqh", [P, KT2, 2, OG], f8, kind="ExternalInput").ap()
    wql = nc.dram_tensor("wql", [P, KT2, 2, OG], f8, kind="ExternalInput").ap()
    wkh = nc.dram_tensor("wkh", [P, KT2, 2, OG], f8, kind="ExternalInput").ap()
    wkl = nc.dram_tensor("wkl", [P, KT2, 2, OG], f8, kind="ExternalInput").ap()
    wvh = nc.dram_tensor("wvh", [P, KT2, 2, OG], f8, kind="ExternalInput").ap()
    wvl = nc.dram_tensor("wvl", [P, KT2, 2, OG], f8, kind="ExternalInput").ap()
    woT = nc.dram_tensor("woT", [OG, D], bf16, kind="ExternalInput").ap()
    bqd = nc.dram_tensor("bqd", [P, 2], f32, kind="ExternalInput").ap()
    bkd = nc.dram_tensor("bkd", [P, 2], f32, kind="ExternalInput").ap()
    idd = nc.dram_tensor("idd", [P, P], bf16, kind="ExternalInput").ap()
    nbias = max(1, len(bias_slots))
    biasT = nc.dram_tensor("biasT", [nbias, P, P], bf16,
                           kind="ExternalInput").ap()
    out = nc.dram_tensor("out", [S, D], bf16, kind="ExternalOutput").ap()

    with tile.TileContext(nc) as tc:
        with tc.tile_pool(name="persist", bufs=1) as persist, \
             tc.tile_pool(name="const", bufs=1) as const:
            # Persistent SBUF tensors. Q/K layout: 4 slots; head h lives at
            # partition base HBASE[h], slot pair HSP[h]..HSP[h]+2 (matmul
            # operand bases must be in {0,32,64}, so head 3 wraps to base 0
            # on the second slot pair).
            HBASE = [0, 32, 64, 0]
            HSP = [0, 0, 0, 2]
            qT8 = persist.tile([P, 4, S], f8)
            kT8 = persist.tile([P, 4, S], f8)
            # V tiles padded to 128 output dims per head: cols 0:64 = v,
            # col 64 = 32.0 (denominator ones-row), cols 65:128 = 0 — the
            # dual-fp8 ldweights ISA check requires M=128 (M=65 is illegal)
            vaug_h = persist.tile([P, NT, HPG, P], f8)
            vaug_l = persist.tile([P, NT, HPG, P], f8)
            zt01 = persist.tile([P, S], bf16)    # heads 0,1 Z.T scaled
            zt23 = persist.tile([P, S], bf16)
            woT_sb = persist.tile([P, 2, D], bf16)
            bias_sb = persist.tile([P, nbias, P], bf16)

            bqs = const.tile([P, 2], f32)
            bks = const.tile([P, 2], f32)
            ident = const.tile([P, P], bf16)
            # ones-row: 32.0 in the hi V (absorbed by the reciprocal), 0 in lo
            nc.gpsimd.memset(vaug_h[:, :, :, HD:P], 0.0)
            nc.gpsimd.memset(vaug_h[:, :, :, HD:HD + 1], WSCALE)
            nc.vector.memset(vaug_l[:, :, :, HD:P], 0.0)

            # ---- Flat pools ----
            osb = tc.alloc_tile_pool(name="osb", bufs=8)
            xTp = tc.alloc_tile_pool(name="xT", bufs=4)
            wsb = tc.alloc_tile_pool(name="wsb", bufs=1)
            psum = tc.alloc_tile_pool(name="psum", bufs=1, space="PSUM")
            ptp = tc.alloc_tile_pool(name="pt", bufs=4)
            small = tc.alloc_tile_pool(name="small", bufs=2)

            wqh_sb = wsb.tile([P, KT2, 2, OG], f8, tag="wqh")
            wql_sb = wsb.tile([P, KT2, 2, OG], f8, tag="wql")
            wkh_sb = wsb.tile([P, KT2, 2, OG], f8, tag="wkh")
            wkl_sb = wsb.tile([P, KT2, 2, OG], f8, tag="wkl")
            wvh_sb = wsb.tile([P, KT2, 2, OG], f8, tag="wvh")
            wvl_sb = wsb.tile([P, KT2, 2, OG], f8, tag="wvl")
            # K weights first (first projections), split for early start;
            # tiny const loads are deferred behind the first matmul's deps
            nc.sync.dma_start(wkh_sb[:, 0:2], wkh[:, 0:2])

            # PSUM: "st" tag [P,2,CH] f32 (2 banks) x2 bufs for attention
            # scores; "rot" tag [P,CH] f32 (1 bank) x4 bufs rotating over
            # proj psums, PV accumulators, and out-proj psums = 8 banks.
            def st_tile(name):
                return psum.tile([P, 2, CH], f32, tag="st", bufs=2, name=name)

            def rot(name):
                return psum.tile([P, CH], f32, tag="rot", bufs=4, name=name)

            srcs = {0: (xkh, xkl, wkh_sb, wkl_sb),
                    1: (xvh, xvl, wvh_sb, wvl_sb),
                    2: (xqh, xql, wqh_sb, wql_sb)}

            def prep_proj(which, c, step):
                """Issue this chunk-projection's DMAs now; return per-psum
                emission closures to be interleaved into the PE stream."""
                # deferred constant loads, spread through the stream
                if step == 1:
                    nc.sync.dma_start(wql_sb, wql)
                    nc.sync.dma_start(wvh_sb, wvh)
                    nc.sync.dma_start(wvl_sb, wvl)
                elif step == 2:
                    nc.sync.dma_start(bias_sb,
                                      biasT.rearrange("n p q -> p n q"))
                elif step == 3:
                    nc.sync.dma_start(
                        woT_sb, woT.rearrange("(t p) d -> p t d", p=P))
                xh_dr, xl_dr, w_h, w_l = srcs[which]
                xh_t = xTp.tile([P, KT2, 2, CH], f8, tag="xT", bufs=10, name="xh")
                xl_t = xTp.tile([P, KT2, 2, CH], f8, tag="xT", bufs=10, name="xl")
                csl = slice(c * CH, (c + 1) * CH)
                if step == 0:
                    # fine-grained, first-needed-first; the x halves go out
                    # on the idle Act queue in parallel with the SP stream
                    nc.scalar.dma_start(xh_t[:, 0:2], xh_dr[:, 0:2, :, csl])
                    nc.sync.dma_start(wkh_sb[:, 2:4], wkh[:, 2:4])
                    nc.scalar.dma_start(xh_t[:, 2:4], xh_dr[:, 2:4, :, csl])
                    nc.sync.dma_start(wkl_sb, wkl)
                    nc.sync.dma_start(bks, bkd)
                    nc.sync.dma_start(wqh_sb, wqh)
                    nc.sync.dma_start(bqs, bqd)
                    nc.sync.dma_start(ident, idd)
                else:
                    nc.sync.dma_start(xh_t, xh_dr[:, :, :, csl])
                nc.sync.dma_start(xl_t, xl_dr[:, :, :, csl])
                units = []
                if which != 1:
                    dst8 = kT8 if which == 0 else qT8
                    bias_ap = bks if which == 0 else bqs

                    def qk_unit(ot):
                        box = {}

                        def run_a():  # allocates the psum slot
                            ps = rot("psqk")
                            box["ps"] = ps
                            osl = slice(ot * P, (ot + 1) * P)
                            for t in range(KT2):
                                nc.tensor.matmul(ps, w_h[:, t, :, osl],
                                                 xh_t[:, t], start=(t == 0),
                                                 stop=False, perf_mode=DR)
                            for t in range(KT2):
                                nc.tensor.matmul(ps, w_l[:, t, :, osl],
                                                 xh_t[:, t], start=False,
                                                 stop=False, perf_mode=DR)

                        def run_b():  # completes group + drains
                            ps = box["ps"]
                            osl = slice(ot * P, (ot + 1) * P)
                            for t in range(KT2):
                                nc.tensor.matmul(ps, w_h[:, t, :, osl],
                                                 xl_t[:, t], start=False,
                                                 stop=(t == KT2 - 1),
                                                 perf_mode=DR)
                            nc.vector.tensor_scalar(
                                dst8[0:96, ot, csl], ps[0:96, :],
                                1.0 / WSCALE, bias_ap[0:96, ot:ot + 1],
                                op0=AL.mult, op1=AL.add)
                            nc.vector.tensor_scalar(
                                dst8[0:32, 2 + ot, csl], ps[96:128, :],
                                1.0 / WSCALE, bias_ap[96:128, ot:ot + 1],
                                op0=AL.mult, op1=AL.add)
                        run_a.alloc = True
                        run_b.alloc = False
                        return [run_a, run_b]
                    units = qk_unit(0) + qk_unit(1)
                else:
                    def v_unit(st):
                        def run():
                            pv = rot("psv")[:, 0:OG]
                            ssl = slice(st * P, (st + 1) * P)
                            for t in range(KT2):
                                nc.tensor.matmul(pv, xh_t[:, t, :, ssl],
                                                 w_h[:, t], start=(t == 0),
                                                 stop=False, perf_mode=DR)
                            for t in range(KT2):
                                nc.tensor.matmul(pv, xh_t[:, t, :, ssl],
                                                 w_l[:, t], start=False,
                                                 stop=False, perf_mode=DR)
                            for t in range(KT2):
                                nc.tensor.matmul(pv, xl_t[:, t, :, ssl],
                                                 w_h[:, t], start=False,
                                                 stop=(t == KT2 - 1),
                                                 perf_mode=DR)
                            pv_re = pv.rearrange("p (h d) -> p h d", h=HPG)
                            vh_view = vaug_h[:, c * 4 + st, :, 0:HD]
                            nc.vector.tensor_copy(vh_view, pv_re)
                            nc.vector.tensor_tensor(
                                vaug_l[:, c * 4 + st, :, 0:HD], pv_re,
                                vh_view, op=AL.subtract)
                        run.alloc = True
                        return run
                    units = [v_unit(st) for st in range(CH // P)]
                return units

            # ---- Attention + out-proj, per sq-chunk ----
            # Out-proj for chunk c-1 is emitted mid-way through chunk c so
            # the (in-order) PE stream never stalls on the epilogue; the
            # epilogue itself is PE-free (DVE recip -> Pool partition
            # broadcast -> DVE multiply). Mask biases are added on the PE
            # (identity-matmul accumulate) to keep DVE off the exp path.
            ATTN_OFF = [0, 512, 1024, 1536]
            ATTN_W = [512, 512, 512, 512]
            NAC = len(ATTN_W)

            def oproj_unit(sg):
                def run():
                    ob = osb.tile([P, D], bf16, tag="ob", name="ob")
                    for nk in range(2):
                        ps = rot("psop")
                        for kk, zsrc in enumerate((zt01, zt23)):
                            nc.tensor.matmul(
                                ps, zsrc[:, sg * P:(sg + 1) * P],
                                woT_sb[:, kk, nk * CH:(nk + 1) * CH],
                                start=(kk == 0), stop=(kk == 1))
                        osl = slice(nk * CH, (nk + 1) * CH)
                        if nk == 0:
                            nc.scalar.copy(ob[:, osl], ps)
                        else:
                            nc.vector.tensor_copy(ob[:, osl], ps)
                    nc.sync.dma_start(out[sg * P:(sg + 1) * P, :], ob)
                return run

            def oproj_units(ci):
                i0 = ATTN_OFF[ci] // P
                return [oproj_unit(sg)
                        for sg in range(i0, i0 + ATTN_W[ci] // P)]

            def emit_attn(ci, fill=None, opro=None):
                fill = fill if fill is not None else []
                opro = opro if opro is not None else []
                coff, cw = ATTN_OFF[ci], ATTN_W[ci]
                i0 = coff // P
                tiles_i = list(range(i0, i0 + cw // P))
                jlist = []
                for j in range(NT):
                    ii = [i for i in tiles_i if process[i, j]]
                    if ii:
                        jlist.append((j, min(ii) - i0,
                                      max(ii) - i0 + 1))
                # pair adjacent j's for the 2-slot PV DoubleRow
                jpairs = []
                idx = 0
                while idx < len(jlist):
                    if (idx + 1 < len(jlist)
                            and jlist[idx + 1][0] == jlist[idx][0] + 1):
                        jpairs.append((jlist[idx], jlist[idx + 1]))
                        idx += 2
                    else:
                        jpairs.append((jlist[idx], None))
                        idx += 1
                for hp in range(2):  # head pairs (2*hp, 2*hp+1)
                    h0, h1 = 2 * hp, 2 * hp + 1
                    zta = {h: rot(f"zta{h}") for h in (h0, h1)}
                    npop = 0
                    pend = None  # deferred PV: next pair's S.T/exp goes
                    # into the PE stream BEFORE this PV so the (in-order)
                    # PE never head-of-line stalls on the exp it waits for

                    def emit_pv(args, last):
                        pa_, pb_, offu_, wu_, pt_, first_ = args
                        ja_ = pa_[0]
                        for hh, h in enumerate((h0, h1)):
                            zo = zta[h]
                            if pb_ is not None:
                                nc.tensor.matmul(
                                    zo[:, offu_:offu_ + wu_],
                                    vaug_h[:, ja_:ja_ + 2, h, :],
                                    pt_[:, :, hh, offu_:offu_ + wu_],
                                    start=first_, stop=False, perf_mode=DR)
                                nc.tensor.matmul(
                                    zo[:, offu_:offu_ + wu_],
                                    vaug_l[:, ja_:ja_ + 2, h, :],
                                    pt_[:, :, hh, offu_:offu_ + wu_],
                                    start=False, stop=last, perf_mode=DR)
                            else:
                                nc.tensor.matmul(
                                    zo[:, offu_:offu_ + wu_],
                                    vaug_h[:, ja_, h, :],
                                    pt_[:, 0, hh, offu_:offu_ + wu_],
                                    start=first_, stop=False)
                                nc.tensor.matmul(
                                    zo[:, offu_:offu_ + wu_],
                                    vaug_l[:, ja_, h, :],
                                    pt_[:, 0, hh, offu_:offu_ + wu_],
                                    start=False, stop=last)

                    for pi, (pa, pb) in enumerate(jpairs):
                        ja, loa, hia = pa
                        if pb is not None:
                            jb, lob, hib = pb
                            lo_u, hi_u = min(loa, lob), max(hia, hib)
                        else:
                            lo_u, hi_u = loa, hia
                        offu, wu = lo_u * P, (hi_u - lo_u) * P
                        pt = ptp.tile([P, 2, 2, CH], f8, tag="pt", bufs=12, name="pt")
                        for jj, ent in enumerate([pa] + ([pb] if pb else [])):
                            j_, lo_, hi_ = ent
                            off, w = lo_ * P, (hi_ - lo_) * P
                            st_ = st_tile("st_")
                            bis = [i for i in range(i0 + lo_, i0 + hi_)
                                   if biased[i, j_]]
                            for hh, h in enumerate((h0, h1)):
                                pb_, sp = HBASE[h], HSP[h]
                                nc.tensor.matmul(
                                    st_[:, hh, off:off + w],
                                    kT8[pb_:pb_ + 32, sp:sp + 2,
                                        j_ * P:(j_ + 1) * P],
                                    qT8[pb_:pb_ + 32, sp:sp + 2,
                                        coff + off:coff + off + w],
                                    start=True, stop=(not bis),
                                    perf_mode=DR)
                            # mask biases via PE identity-matmul accumulate
                            for bn, i in enumerate(bis):
                                sl = bias_slots[(i, j_)]
                                so = (i - i0) * P
                                lastb = bn == len(bis) - 1
                                for hh in range(2):
                                    nc.tensor.matmul(
                                        st_[:, hh, so:so + P], ident,
                                        bias_sb[:, sl, :], start=False,
                                        stop=(lastb and hh == 1),
                                        skip_group_check=True)
                            # zero pt where this j's band is narrower than
                            # the pair's union (PV reads the union)
                            if off > offu:
                                nc.gpsimd.memset(
                                    pt[:, jj, :, offu:off], 0.0)
                            if off + w < offu + wu:
                                nc.gpsimd.memset(
                                    pt[:, jj, :, off + w:offu + wu], 0.0)
                            nc.scalar.activation(
                                pt[:, jj, :, off:off + w],
                                st_[:, :, off:off + w],
                                mybir.ActivationFunctionType.Exp,
                                scale=EXPS)
                        if pend is not None:
                            emit_pv(pend, last=False)
                            # interleave a projection unit into the exp-wait
                            # gap; cap 2 per head pair so the rot rotation
                            # never lands on a live PV accumulator
                            if npop < 2 and fill:
                                u = fill.pop(0)
                                u()
                                npop += 1 if getattr(u, "alloc", True) \
                                    else 0
                        pend = (pa, pb, offu, wu, pt, pi == 0)
                    emit_pv(pend, last=True)
                    # epilogue (PE-free): reciprocal of 32*denominators,
                    # Pool partition-broadcast, DVE scale into SBUF
                    recs = small.tile([1, 2, CH], f32, tag="recs", bufs=4,
                                      name="recs")
                    bcs0 = small.tile([HD, CH], f32, tag="bcs0", bufs=4,
                                      name="bcs0")
                    bcs1 = small.tile([HD, CH], f32, tag="bcs1", bufs=4,
                                      name="bcs1")
                    with nc.allow_low_precision(reason="fp22 recip"):
                        nc.vector.reciprocal(recs[0:1, 0, 0:cw],
                                             zta[h0][HD:HD + 1, 0:cw])
                        nc.vector.reciprocal(recs[0:1, 1, 0:cw],
                                             zta[h1][HD:HD + 1, 0:cw])
                    # (partition_broadcast only writes at base partition 0)
                    nc.gpsimd.partition_broadcast(bcs0[:, 0:cw],
                                                  recs[0:1, 0, 0:cw],
                                                  channels=HD)
                    nc.gpsimd.partition_broadcast(bcs1[:, 0:cw],
                                                  recs[0:1, 1, 0:cw],
                                                  channels=HD)
                    zdst = zt01 if hp == 0 else zt23
                    for hh, h in enumerate((h0, h1)):
                        zpo = hh * HD
                        nc.vector.tensor_mul(
                            zdst[zpo:zpo + HD, coff:coff + cw],
                            zta[h][0:HD, 0:cw],
                            (bcs0 if hh == 0 else bcs1)[:, 0:cw])
                    # post-epilogue: both PV accumulators are drained, so
                    # any number of rot-allocating units is safe here
                    npost = 2 if hp == 0 else len(fill) + len(opro)
                    for _ in range(min(2, len(fill)) if hp == 0 else
                                   len(fill)):
                        fill.pop(0)()
                    for _ in range(min(2, len(opro)) if hp == 0 else
                                   len(opro)):
                        opro.pop(0)()

            # Interleaved schedule: causal attention chunk c needs only
            # K/V chunks 0..c and Q chunk c. Chunk c+1's projections are
            # emitted BETWEEN chunk c's head pairs so the PE keeps working
            # while the Act engine (exp, the critical resource) chews on
            # chunk c, and Act never starves at chunk boundaries.
            for u in (prep_proj(0, 0, 0) + prep_proj(2, 0, 1)
                      + prep_proj(1, 0, 2)):
                u()
            # out-proj units are deferred one extra chunk so they land in
            # the Act-bound (exp-heavy) late windows where the PE starves
            opro_sched = {2: oproj_units(0), 3: oproj_units(1)}
            for ci in range(NAC):
                fill = []
                if ci + 1 < NCH:
                    fill += prep_proj(0, ci + 1, 3 + ci)
                    fill += prep_proj(2, ci + 1, 99)
                    fill += prep_proj(1, ci + 1, 99)
                if ci == NAC - 1:
                    fill += oproj_units(NAC - 2)
                emit_attn(ci, fill=fill, opro=opro_sched.get(ci, []))
            for u in oproj_units(NAC - 1):
                u()
            for pool_ in (small, ptp, psum, wsb, xTp, osb):
                pool_.release()
    nc.compile()
    # Belt-and-braces: any write-only preamble registers that survive DCE
    # but never get ids from alloc_regs would fail walrus birverifier
    # (reg_id == -1). They are write-only, so engine-unique ids are safe;
    # keep _lo/_hi pairs adjacent and even-aligned.
    from collections import defaultdict
    from concourse import mybir
    ctr = defaultdict(int)
    for f_ in nc.m.functions:
        for a in f_.allocations:
            if isinstance(a, mybir.Register) and a.reg_id >= 0:
                ctr[a.engine] = max(ctr[a.engine], a.reg_id + 1)
    for f_ in nc.m.functions:
        for a in f_.allocations:
            if isinstance(a, mybir.Register) and a.reg_id == -1:
                if a.name.endswith("_lo") and ctr[a.engine] % 2:
                    ctr[a.engine] += 1
                a.reg_id = ctr[a.engine]
                ctr[a.engine] += 1
    return nc


def _interleave_kdim(arr):
    """[1024 in-dim, N] -> [128 p, 4 t, 2 slot, N] with d = t*256+slot*128+p."""
    n = arr.shape[1]
    return np.ascontiguousarray(
        arr.reshape(KT2, 2, P, n).transpose(2, 0, 1, 3))


def _split8(arr):
    import ml_dtypes
    e4 = ml_dtypes.float8_e4m3
    hi = arr.astype(e4)
    lo = (arr - hi.astype(np.float32)).astype(e4)
    return np.ascontiguousarray(hi), np.ascontiguousarray(lo)


def kernel(query, key, value, mask, key_padding_mask,
           Wq, bq, Wk, bk, Wv, bv, Wo, bo, _return_perf=False):
    import ml_dtypes
    from concourse import bass_utils

    query = np.asarray(query, np.float32)
    key_ = np.asarray(key, np.float32)
    value = np.asarray(value, np.float32)
    Wq, Wk, Wv, Wo = (np.asarray(w, np.float32) for w in (Wq, Wk, Wv, Wo))
    bq, bk, bv, bo = (np.asarray(b_, np.float32) for b_ in (bq, bk, bv, bo))

    process, biased, bias_data = _block_structure(mask, key_padding_mask)
    bias_slots = {}
    for i in range(NT):
        for j in range(NT):
            if process[i, j] and biased[i, j]:
                bias_slots[(i, j)] = len(bias_slots)

    key_struct = (process.tobytes(), biased.tobytes())
    if key_struct not in _cache:
        _cache[key_struct] = _build_bass(process, biased, bias_slots)
    nc = _cache[key_struct]

    nbias = max(1, len(bias_slots))
    # x splits: shared across the 4 cores of each batch
    xsp = {}
    for b in range(B):
        for nm, x in (("q", query[b]), ("k", key_[b]), ("v", value[b])):
            xsp[(nm, b)] = _split8(_interleave_kdim(
                np.ascontiguousarray(x.T)))

    # Q/K output-dim permutation: psum partition p = head*32 + dim%32,
    # slot ot = dim//32  (head/dim within this core's 4-head group)
    perm = np.zeros((2, P), np.int64)
    for ot in range(2):
        for p_ in range(P):
            perm[ot, p_] = (p_ // 32) * HD + ot * 32 + (p_ % 32)

    in_maps = []
    for core in range(8):
        b, g = core // G, core % G
        gsl = np.arange(g * OG, (g + 1) * OG)
        qk_rows = gsl.reshape(1, OG)[0][perm.reshape(-1)]  # [256] perm'd
        wq_s = _split8(_interleave_kdim(WSCALE * Wq[qk_rows, :].T))
        wk_s = _split8(_interleave_kdim(WSCALE * Wk[qk_rows, :].T))
        wv_s = _split8(_interleave_kdim(WSCALE * Wv[gsl, :].T))
        bt = np.zeros((nbias, P, P), np.float32)
        for (i, j), slot in bias_slots.items():
            bt[slot] = bias_data[b][(i, j)]
        bt = bt.astype(ml_dtypes.bfloat16)
        in_maps.append({
            "xqh": xsp[("q", b)][0], "xql": xsp[("q", b)][1],
            "xkh": xsp[("k", b)][0], "xkl": xsp[("k", b)][1],
            "xvh": xsp[("v", b)][0], "xvl": xsp[("v", b)][1],
            "wqh": wq_s[0], "wql": wq_s[1],
            "wkh": wk_s[0], "wkl": wk_s[1],
            "wvh": wv_s[0], "wvl": wv_s[1],
            "woT": np.ascontiguousarray(Wo[:, gsl].T.astype(ml_dtypes.bfloat16)),
            "bqd": np.ascontiguousarray(bq[qk_rows].reshape(2, P).T),
            "bkd": np.ascontiguousarray(bk[qk_rows].reshape(2, P).T),
            "idd": np.eye(P, dtype=ml_dtypes.bfloat16),
            "biasT": bt,
        })

    trace = bool(int(os.environ.get("KERNEL_TRACE", "0")))
    res = bass_utils.run_bass_kernel_spmd(
        nc, in_maps, core_ids=list(range(8)), trace=trace)

    out = np.zeros((B, S, D), np.float32)
    for core in range(8):
        out[core // G] += res.results[core]["out"].astype(np.float32)
    out += (bo + bv @ Wo.T)[None, None, :]
    if _return_perf:
        return out, res
    return out
